# revision 1
# baseline (speedup 1.0000x reference)
"""GCN (5-layer ColorGNN) Bass kernel for 8 TRN2 NeuronCores.

Strategy (node-sharded, SPMD):
  - Nodes row-sharded across 8 cores (6250/core, padded to 6272 = 49*128).
  - Fixed normalized adjacency A (same for all 5 layers, incl self-loops):
      out[d] = dinv[d] * sum_e(dinv[src_e]*ew_e * T[src_e]) (+ bias via
      sqrt(deg) trick inside PSUM), relu fused in the epilogue.
  - Per layer: dense matmul T_own = H @ W (node-parallel, fp16),
      AllGather T_own -> T_full, then aggregation:
      per 128-dst-node tile: dma_gather message rows from T_full (fp16),
      one-hot selection matrices S (built on DVE: is_equal(iota, dstslot)
      * norm) folded through the PE: psum += S_c^T @ msg_c.
  - Layer 1 aggregates X first (A@X) since F_in=512 < F_out=2048.
  - Host preprocessing: sort edges by (dst tile, src region), pad per
    (tile, region) to the max count over cores so all 8 cores run the
    same program (SPMD) with different index/norm data.
"""

import numpy as np
import concourse.bass as bass
import concourse.mybir as mybir
import concourse.tile as tile

FP16 = mybir.dt.float16
F32 = mybir.dt.float32
I16 = mybir.dt.int16

P = 128
SPLIT = 32768  # int16 index limit boundary for gather regions
SBATCH = 8     # chunks per S-build DVE op batch


# ---------------------------------------------------------------- tile patch
def apply_tile_patch():
    """This walrus build allows only 1 sync-wait per Drain; split the tail
    drain's waits across a chain of drains."""
    import bass_rust

    def _drain_and_barrier_split(self, tick_clock, wait_clock):
        from bass_rust import ScopedClock
        drain_inst = self.nc.sync.drain()
        wait_clock.add_sem_waits(
            drain_inst.ins, ScopedClock({None: tick_clock.global_clock})
        )
        si = drain_inst.ins.sync_info
        waits = list(si.on_wait) if si is not None else []
        if len(waits) > 1:
            si.on_wait = [waits[0]]
            for w in waits[1:]:
                extra = self.nc.sync.drain()
                if extra.ins.sync_info is None:
                    extra.ins.sync_info = bass_rust.SyncInfo(
                        on_wait=[w], on_update=[])
                else:
                    extra.ins.sync_info.on_wait = [w]
        self.nc.all_engine_barrier()
        popped = self.nc._tile_sem_poison_stack.pop()
        assert popped is self._sem_poison
        self.nc.clear_and_free_semaphores(list(self.sems.allocated().values()))
        self.nc.all_engine_barrier()

    tile.TileContext._drain_and_barrier = _drain_and_barrier_split


# ------------------------------------------------------------------- config
class Cfg:
    def __init__(self, n_nodes, n_cores, dims_in):
        # dims_in: [512, 2048, 1024, 512, 128, 64] + final 3
        self.n_nodes = n_nodes
        self.n_cores = n_cores
        self.pcn = n_nodes // n_cores               # real nodes per core
        assert self.pcn * n_cores == n_nodes
        self.npc = ((self.pcn + P - 1) // P) * P    # padded nodes per core
        self.nt = self.npc // P                     # dst tiles per core
        self.npt = self.npc * n_cores               # padded total nodes
        # feature widths (pad last hidden 64->128, final out 3->4)
        d = list(dims_in)
        self.dims_real = d
        self.hid = [d[0], d[1], d[2], d[3], d[4], P]    # hidden widths padded
        self.fout = 4                               # padded final width
        # aggregation widths (width of T_l gathered at layer l)
        self.tw = [self.hid[0], self.hid[2], self.hid[3], self.hid[4], self.hid[5]]
        # regions for int16 gather indexing
        if self.npt > SPLIT:
            assert self.npt - SPLIT <= 32768
            self.regions = [(0, SPLIT), (SPLIT, self.npt)]
        else:
            self.regions = [(0, self.npt)]


# ------------------------------------------------------------- preprocess
def preprocess(x, edge_index, edge_attr, Ws, bs, Wp, bp, cfg: Cfg):
    """Host-side: normalization, edge sharding/sorting/packing, input maps.
    Returns (in_maps, meta). meta holds the compile-time structure."""
    N, C = cfg.n_nodes, cfg.n_cores
    src = np.asarray(edge_index[0], dtype=np.int64)
    dst = np.asarray(edge_index[1], dtype=np.int64)
    ew = np.asarray(edge_attr, dtype=np.float32)
    loop = np.arange(N, dtype=np.int64)
    src2 = np.concatenate([src, loop])
    dst2 = np.concatenate([dst, loop])
    ew2 = np.concatenate([ew, np.ones(N, np.float32)])

    deg = np.bincount(dst2, weights=ew2.astype(np.float64), minlength=N)
    deg = deg.astype(np.float32)
    dinv = np.where(deg > 0, 1.0 / np.sqrt(deg), 0.0).astype(np.float32)
    norm_s = (dinv[src2] * ew2).astype(np.float32)  # dinv[dst] applied later

    gpid = (src2 // cfg.pcn) * cfg.npc + (src2 % cfg.pcn)  # padded global ids

    core_of = dst2 // cfg.pcn
    slot = dst2 - core_of * cfg.pcn          # local slot 0..pcn-1
    tile_of = slot // P
    slot_in = slot % P

    NR = len(cfg.regions)
    region_of = np.zeros(len(src2), np.int64)
    if NR == 2:
        region_of = (gpid >= SPLIT).astype(np.int64)

    # bucket edges per (core, tile, region)
    counts = np.zeros((C, cfg.nt, NR), np.int64)
    np.add.at(counts, (core_of, tile_of, region_of), 1)
    kmax = counts.max(axis=0)                      # [nt, NR]
    K = ((kmax + P - 1) // P) * P                  # padded per-call counts
    K[kmax == 0] = 0

    # order edges by (core, tile, region) via lexsort
    order = np.lexsort((region_of, tile_of, core_of))
    so_gpid = gpid[order]
    so_norm = norm_s[order]
    so_slot = slot_in[order]
    so_core = core_of[order]
    so_tile = tile_of[order]
    so_reg = region_of[order]

    # per-(tile,region) call column bases (idx cols and chunk cols)
    icol = np.zeros((cfg.nt, NR), np.int64)
    cbase = np.zeros((cfg.nt, NR), np.int64)
    ic = cc = 0
    for t in range(cfg.nt):
        for r in range(NR):
            icol[t, r] = ic
            cbase[t, r] = cc
            ic += K[t, r] // 16
            cc += K[t, r] // P
    idxcols, nch = ic, cc

    in_maps = []
    x_np = np.asarray(x, dtype=np.float32)
    # weight packs (shared across cores)
    w_packs, b_rows = [], []
    hid = cfg.hid
    w_list = [np.asarray(w, np.float32) for w in Ws] + [np.asarray(Wp, np.float32)]
    b_list = [np.asarray(b, np.float32) for b in bs] + [np.asarray(bp, np.float32)]
    kdims = [hid[0], hid[1], hid[2], hid[3], hid[4], hid[5]]
    mdims = [hid[1], hid[2], hid[3], hid[4], hid[5], cfg.fout]
    for j in range(6):
        Kd, Md = kdims[j], mdims[j]
        wp = np.zeros((Kd, Md), np.float32)
        wr = w_list[j]
        wp[: wr.shape[0], : wr.shape[1]] = wr
        wp = wp.reshape(Kd // P, P, Md).transpose(1, 0, 2).reshape(P, -1)
        w_packs.append(wp.astype(np.float16))
        br = np.zeros((1, Md), np.float32)
        br[0, : b_list[j].shape[0]] = b_list[j]
        b_rows.append(br.astype(np.float16))

    iota = np.tile(np.arange(P, dtype=np.float16), (P, 1))
    ones1 = np.ones((1, P), np.float16)

    # boundaries of each core's edges in the sorted order
    core_starts = np.searchsorted(so_core, np.arange(C + 1))

    for c in range(C):
        lo, hi = core_starts[c], core_starts[c + 1]
        ct, cr = so_tile[lo:hi], so_reg[lo:hi]
        cg, cn, cs = so_gpid[lo:hi], so_norm[lo:hi], so_slot[lo:hi]
        # per (tile, region) start offsets within this core's slice
        idx16 = np.zeros((16, idxcols), np.int16)
        slotp = np.zeros((P, nch), np.float16)
        normp = np.zeros((P, nch), np.float16)
        pos = 0
        for t in range(cfg.nt):
            for r in range(NR):
                k = K[t, r]
                if k == 0:
                    continue
                n_e = counts[c, t, r]
                seg = slice(pos, pos + n_e)
                assert np.all(ct[seg] == t) and np.all(cr[seg] == r), (c, t, r)
                reg_lo = cfg.regions[r][0]
                arr = np.zeros(k, np.int64)
                arr[:n_e] = cg[seg] - reg_lo
                assert arr.max(initial=0) < 32768
                idx16[:, icol[t, r]: icol[t, r] + k // 16] = (
                    arr.reshape(k // 16, 16).T.astype(np.int16))
                sl = np.zeros(k, np.float32)
                sl[:n_e] = cs[seg]
                nm = np.zeros(k, np.float32)
                nm[:n_e] = cn[seg]
                cb = cbase[t, r]
                slotp[:, cb: cb + k // P] = (
                    sl.reshape(k // P, P).T.astype(np.float16))
                normp[:, cb: cb + k // P] = (
                    nm.reshape(k // P, P).T.astype(np.float16))
                pos += n_e
        assert pos == hi - lo

        # x shard (padded, fp16)
        x16 = np.zeros((cfg.npc, hid[0]), np.float16)
        x16[: cfg.pcn] = x_np[c * cfg.pcn: (c + 1) * cfg.pcn].astype(np.float16)
        # dinv per slot [128, nt], sqrt(deg) row [1, npc]
        dloc = np.zeros(cfg.npc, np.float32)
        dloc[: cfg.pcn] = dinv[c * cfg.pcn: (c + 1) * cfg.pcn]
        dinvp = dloc.reshape(cfg.nt, P).T.copy()
        sq = np.zeros((1, cfg.npc), np.float32)
        sq[0, : cfg.pcn] = np.sqrt(deg[c * cfg.pcn: (c + 1) * cfg.pcn])
        sqd = sq.astype(np.float16)

        m = {
            "x16": x16,
            "idx16": np.tile(idx16, (8, 1)),
            "slotp": slotp,
            "normp": normp,
            "dinvp": dinvp,
            "sqd": sqd,
            "iota": iota,
            "ones1": ones1,
        }
        for j in range(6):
            m[f"w{j}"] = w_packs[j]
            m[f"b{j}"] = b_rows[j]
        in_maps.append(m)

    meta = dict(K=K, icol=icol, cbase=cbase, idxcols=idxcols, nch=nch)
    return in_maps, meta


# ---------------------------------------------------------------- program
def _bc3(ap, ncols, inner=P, mode="col"):
    """3D broadcast APs for batched S-build.
    mode 'col': [128, ncols] -> [[p,128],[1,ncols],[0,inner]] (each col
    replicated across inner); mode 'mat': [128, inner] -> insert [0, ncols]."""
    base = ap.ap
    if mode == "col":
        return bass.AP(ap.tensor, ap.offset, [base[0], [1, ncols], [0, inner]])
    else:
        return bass.AP(ap.tensor, ap.offset, [base[0], [0, ncols], base[1]])


def _3d(ap, ncols, inner=P):
    """[128, ncols*inner] contiguous slice -> [[p,128],[inner,ncols],[1,inner]]"""
    return bass.AP(ap.tensor, ap.offset, [ap.ap[0], [inner, ncols], [1, inner]])


def build_program(cfg: Cfg, meta):
    import concourse.bacc as bacc
    nc = bacc.Bacc("TRN2", num_swdge_queues=4)
    hid, tw = cfg.hid, cfg.tw
    K, icol, cbase = meta["K"], meta["icol"], meta["cbase"]
    idxcols, nch = meta["idxcols"], meta["nch"]
    NR = len(cfg.regions)
    NT = cfg.nt
    rg = [list(range(cfg.n_cores))]

    # ---------------- params
    pr = {}
    pr["x16"] = nc.declare_dram_parameter("x16", [cfg.npc, hid[0]], FP16, isOutput=False)
    pr["idx16"] = nc.declare_dram_parameter("idx16", [P, idxcols], I16, isOutput=False)
    pr["slotp"] = nc.declare_dram_parameter("slotp", [P, nch], FP16, isOutput=False)
    pr["normp"] = nc.declare_dram_parameter("normp", [P, nch], FP16, isOutput=False)
    pr["dinvp"] = nc.declare_dram_parameter("dinvp", [P, NT], F32, isOutput=False)
    pr["sqd"] = nc.declare_dram_parameter("sqd", [1, cfg.npc], FP16, isOutput=False)
    pr["iota"] = nc.declare_dram_parameter("iota", [P, P], FP16, isOutput=False)
    pr["ones1"] = nc.declare_dram_parameter("ones1", [1, P], FP16, isOutput=False)
    kdims = [hid[0], hid[1], hid[2], hid[3], hid[4], hid[5]]
    mdims = [hid[1], hid[2], hid[3], hid[4], hid[5], cfg.fout]
    for j in range(6):
        pr[f"w{j}"] = nc.declare_dram_parameter(
            f"w{j}", [P, (kdims[j] // P) * mdims[j]], FP16, isOutput=False)
        pr[f"b{j}"] = nc.declare_dram_parameter(f"b{j}", [1, mdims[j]], FP16, isOutput=False)
    out_ext = nc.declare_dram_parameter("out", [cfg.pcn, 3], F32, isOutput=True)

    # ---------------- internal DRAM
    xb = nc.dram_tensor("xb", [cfg.npc, hid[0]], FP16)
    TF = [nc.dram_tensor(f"tf{l}", [cfg.npt, tw[l]], FP16, addr_space="Shared")
          for l in range(5)]
    town = [None] + [nc.dram_tensor(f"town{l}", [cfg.npc, tw[l]], FP16)
                     for l in range(1, 5)]
    # dense outputs: G1 (agg of x), H1..H5
    G1 = nc.dram_tensor("g1", [cfg.npc, tw[0]], FP16)
    Hs = [nc.dram_tensor(f"h{j}", [cfg.npc, hid[j + 1]], FP16) for j in range(5)]

    with tile.TileContext(nc) as tc:
        import contextlib
        with contextlib.ExitStack() as ctx:
            cpool = ctx.enter_context(tc.tile_pool(name="const", bufs=1))
            msgp = ctx.enter_context(tc.tile_pool(name="msg", bufs=3))
            spool = ctx.enter_context(tc.tile_pool(name="sb", bufs=2))
            pp = ctx.enter_context(tc.tile_pool(name="ps", bufs=2, space="PSUM"))
            hp = ctx.enter_context(tc.tile_pool(name="hout", bufs=3))
            wp_ = ctx.enter_context(tc.tile_pool(name="wts", bufs=1))
            htp = ctx.enter_context(tc.tile_pool(name="ht", bufs=24))

            # ---- resident constants
            idx_sb = cpool.tile([P, idxcols], I16)
            nc.sync.dma_start(out=idx_sb[:], in_=pr["idx16"][:])
            slot_sb = cpool.tile([P, nch], FP16)
            nc.sync.dma_start(out=slot_sb[:], in_=pr["slotp"][:])
            norm_sb = cpool.tile([P, nch], FP16)
            nc.sync.dma_start(out=norm_sb[:], in_=pr["normp"][:])
            dinv_sb = cpool.tile([P, NT], F32)
            nc.sync.dma_start(out=dinv_sb[:], in_=pr["dinvp"][:])
            sqd_sb = cpool.tile([1, cfg.npc], FP16)
            nc.sync.dma_start(out=sqd_sb[:], in_=pr["sqd"][:])
            iota_sb = cpool.tile([P, P], FP16)
            nc.sync.dma_start(out=iota_sb[:], in_=pr["iota"][:])
            ones_sb = cpool.tile([1, P], FP16)
            nc.sync.dma_start(out=ones_sb[:], in_=pr["ones1"][:])
            brow_sb = []
            for j in range(6):
                b_ = cpool.tile([1, mdims[j]], FP16, tag=f"br{j}")
                nc.sync.dma_start(out=b_[:], in_=pr[f"b{j}"][:])
                brow_sb.append(b_)

            GMAX = 8  # max chunks per gather call (ucode caps dma_gather at 1024 idxs)
            qn = [0]  # round-robin SWDGE queue

            # ---- aggregation phase for layer l (0-based): T_full -> dst
            def agg(l, dst_dram, with_bias_relu, bias_idx):
                W = tw[l]
                FC = min(W, 512)
                nfp = W // FC
                for t in range(NT):
                    ct = int(K[t].sum() // P)
                    # gather segments: (region, n_idx, idx_col, chunk_off)
                    segs = []
                    for r in range(NR):
                        k, pos = int(K[t, r]), 0
                        while pos < k:
                            ks = min(GMAX * P, k - pos)
                            segs.append((r, ks, int(icol[t, r]) + pos // 16,
                                         int(cbase[t, r] - cbase[t, 0]) + pos // P))
                            pos += ks
                    # build S for all chunks of this tile
                    s_t = spool.tile([P, max(ct, 1) * P], FP16, tag="s")
                    for b0 in range(0, ct, SBATCH):
                        nb = min(SBATCH, ct - b0)
                        cb0 = int(cbase[t, 0]) + b0
                        o3 = _3d(s_t[:, b0 * P:(b0 + nb) * P], nb)
                        nc.vector.tensor_tensor(
                            out=o3,
                            in0=_bc3(slot_sb[:, cb0:cb0 + nb], nb, mode="col"),
                            in1=_bc3(iota_sb[:], nb, mode="mat"),
                            op=mybir.AluOpType.is_equal)
                        nc.vector.tensor_tensor(
                            out=o3, in0=o3,
                            in1=_bc3(norm_sb[:, cb0:cb0 + nb], nb, mode="col"),
                            op=mybir.AluOpType.mult)
                    ps = pp.tile([P, 2048], F32, tag="ps")
                    for fp in range(nfp):
                        nmm = 0
                        for (r, ks, ic, cb) in segs:
                            reg_lo, reg_hi = cfg.regions[r]
                            src_ap = TF[l][reg_lo:reg_hi, fp * FC:(fp + 1) * FC]
                            msg = msgp.tile([P, GMAX * FC], FP16, tag="msg")
                            nc.gpsimd.dma_gather(
                                out_ap=_3d(msg[:, : (ks // P) * FC],
                                           ks // P, inner=FC),
                                in_ap=src_ap,
                                idxs_ap=idx_sb[:, ic: ic + ks // 16],
                                num_idxs=ks,
                                num_idxs_reg=ks,
                                elem_size=FC,
                                elem_step=W,
                                queue_num=qn[0],
                            )
                            qn[0] = (qn[0] + 1) % 4
                            for ci in range(ks // P):
                                nmm += 1
                                nc.tensor.matmul(
                                    out=ps[:, fp * FC:(fp + 1) * FC],
                                    lhsT=s_t[:, (cb + ci) * P:(cb + ci + 1) * P],
                                    rhs=msg[:, ci * FC:(ci + 1) * FC],
                                    start=(nmm == 1),
                                    stop=(not with_bias_relu and nmm == ct))
                        # bias matmul: psum += sqrt(deg)[:,None] @ b[None,:]
                        if with_bias_relu:
                            nc.tensor.matmul(
                                out=ps[:, fp * FC:(fp + 1) * FC],
                                lhsT=sqd_sb[0:1, t * P:(t + 1) * P],
                                rhs=brow_sb[bias_idx][0:1, fp * FC:(fp + 1) * FC],
                                start=(ct == 0), stop=True)
                        elif ct == 0:
                            nc.vector.memset(ps[:, fp * FC:(fp + 1) * FC], 0.0)
                    h_sb = hp.tile([P, W], FP16, tag="hout")
                    nc.scalar.activation(
                        out=h_sb[:], in_=ps[:, :W],
                        func=(mybir.ActivationFunctionType.Relu if with_bias_relu
                              else mybir.ActivationFunctionType.Copy),
                        scale=dinv_sb[:, t:t + 1])
                    nc.sync.dma_start(
                        out=dst_dram[t * P:(t + 1) * P, :], in_=h_sb[:])

            # ---- dense phase j (0-based): in_dram [npc,K] @ w_j -> out
            def dense(j, in_dram, out_dram, bias_relu, final=False):
                Kd, Md = kdims[j], mdims[j]
                nk = Kd // P
                w_sb = wp_.tile([P, nk * Md], FP16, tag="w")
                nc.sync.dma_start(out=w_sb[:], in_=pr[f"w{j}"][:])
                for t in range(NT):
                    ps = pp.tile([P, 2048], F32, tag="ps")
                    hts = []
                    for k in range(nk):
                        ht = htp.tile([P, P], FP16, tag="ht")
                        nc.sync.dma_start(
                            out=ht[:],
                            in_=in_dram[t * P:(t + 1) * P, k * P:(k + 1) * P],
                            transpose=True)
                        hts.append(ht)
                    has_bias = bias_relu or final
                    for k in range(nk):
                        for m0 in range(0, Md, 512):
                            m1 = min(m0 + 512, Md)
                            nc.tensor.matmul(
                                out=ps[:, m0:m1],
                                lhsT=hts[k][:],
                                rhs=w_sb[:, k * Md + m0: k * Md + m1],
                                start=(k == 0),
                                stop=(k == nk - 1 and not has_bias))
                    if bias_relu or final:
                        for m0 in range(0, Md, 512):
                            m1 = min(m0 + 512, Md)
                            nc.tensor.matmul(
                                out=ps[:, m0:m1],
                                lhsT=ones_sb[0:1, :],
                                rhs=brow_sb[j][0:1, m0:m1],
                                start=False, stop=True)
                    if final:
                        o_sb = hp.tile([P, Md], F32, tag="fout")
                        nc.vector.tensor_copy(out=o_sb[:], in_=ps[:, :Md])
                        r0 = t * P
                        r1 = min((t + 1) * P, cfg.pcn)
                        if r1 > r0:
                            nc.sync.dma_start(
                                out=out_dram[r0:r1, :],
                                in_=o_sb[: r1 - r0, :3])
                    else:
                        h_sb = hp.tile([P, Md], FP16, tag="hout")
                        if bias_relu:
                            nc.scalar.activation(
                                out=h_sb[:], in_=ps[:, :Md],
                                func=mybir.ActivationFunctionType.Relu)
                        else:
                            nc.scalar.copy(out=h_sb[:], in_=ps[:, :Md])
                        nc.sync.dma_start(
                            out=out_dram[t * P:(t + 1) * P, :], in_=h_sb[:])

            # ---------------- the network
            nc.sync.dma_start(out=xb[:], in_=pr["x16"][:])
            nc.gpsimd.collective_compute(
                "AllGather", mybir.AluOpType.bypass, replica_groups=rg,
                ins=[xb[:]], outs=[TF[0][:]])
            agg(0, G1, with_bias_relu=False, bias_idx=None)       # A@X
            dense(0, G1, Hs[0], bias_relu=True)                   # H1
            for l in range(1, 5):
                dense(l, Hs[l - 1], town[l], bias_relu=False)     # T_own
                nc.gpsimd.collective_compute(
                    "AllGather", mybir.AluOpType.bypass, replica_groups=rg,
                    ins=[town[l][:]], outs=[TF[l][:]])
                agg(l, Hs[l], with_bias_relu=True, bias_idx=l)    # H_{l+1}
            dense(5, Hs[4], out_ext, bias_relu=False, final=True)

    nc.finalize()
    return nc


# ------------------------------------------------------------------ driver
def run_numpy_reference(x, edge_index, edge_attr, Ws, bs, Wp, bp):
    """Mirror of reference.py in numpy (float32)."""
    N = x.shape[0]
    src, dst = np.asarray(edge_index[0]), np.asarray(edge_index[1])
    ew = np.asarray(edge_attr, np.float32)
    loop = np.arange(N)
    src2 = np.concatenate([src, loop])
    dst2 = np.concatenate([dst, loop])
    ew2 = np.concatenate([ew, np.ones(N, np.float32)])
    deg = np.bincount(dst2, weights=ew2, minlength=N).astype(np.float32)
    dinv = np.where(deg > 0, 1 / np.sqrt(deg), 0).astype(np.float32)
    norm = dinv[src2] * ew2 * dinv[dst2]

    def conv(h, W, b):
        hw = h @ W
        msg = hw[src2] * norm[:, None]
        out = np.zeros((N, W.shape[1]), np.float32)
        np.add.at(out, dst2, msg)
        return out + b

    h = np.asarray(x, np.float32)
    for W, b in zip(Ws, bs):
        h = np.maximum(conv(h, W, b), 0)
    return h @ Wp + bp


# ===================================================================
# Harness entry point: kernel(**inputs) -> np.ndarray [50000, 3] f32
# ===================================================================
_CACHE = {}


def kernel(x, edge_index, edge_attr, W1, b1, W2, b2, W3, b3, W4, b4, W5, b5,
           Wp, bp):
    apply_tile_patch()
    import os
    from concourse.bass_utils import run_bass_kernel_spmd

    cfg = Cfg(50000, 8, [512, 2048, 1024, 512, 128, 64])
    Ws = [W1, W2, W3, W4, W5]
    bs = [b1, b2, b3, b4, b5]
    in_maps, meta = preprocess(x, edge_index, edge_attr, Ws, bs, Wp, bp, cfg)

    key = (meta["K"].tobytes(), meta["nch"], meta["idxcols"])
    nc = _CACHE.get(key)
    if nc is None:
        nc = build_program(cfg, meta)
        _CACHE[key] = nc

    res = run_bass_kernel_spmd(
        nc, in_maps, core_ids=list(range(cfg.n_cores)),
        trace=bool(int(os.environ.get("TRACE", "0"))))
    if res.exec_time_ns:
        print(f"HW exec time: {res.exec_time_ns} ns")
    out = np.concatenate(
        [res.results[c]["out"] for c in range(cfg.n_cores)], axis=0)
    return np.ascontiguousarray(out.astype(np.float32))



# revision 12
# speedup vs baseline: 1.4312x; 1.4312x over previous
"""GCN (5-layer ColorGNN) Bass kernel for 8 TRN2 NeuronCores — v2.

Pipelined design (node-sharded SPMD, 6272 padded nodes/core = 49 tiles):

  - Full padded X (fp16, gpid layout) is staged on EVERY core: the layer-1
    aggregation gathers it directly -> no AllGather for layer 1.
  - Aggregation outputs are produced TRANSPOSED (H^T tiles: features on
    partitions, 128 dst nodes on free): out^T[f,n] = sum_e msg[e,f]*S[e,n]
    with lhsT=msg (gathered rows), rhs=S (one-hot x norm). The full
    symmetric norm dinv[src]*ew*dinv[dst] is folded into S on the host, and
    the bias is a per-partition ACT bias in the relu epilogue.
  - H^T tiles feed the next dense matmul as lhsT straight from SBUF: no
    transposes, no H round-trips through DRAM. Dense outputs T (node-major)
    go to DRAM only as AllGather input. Layer-1's dense pair runs in the
    transposed orientation (lhsT = W1 blocks) so the chain stays in H^T.
  - Each T AllGather is split in 2 node-chunks (4096+2176 rows/core,
    matching gpid regions [0,32768) and [32768,50176) which also serve as
    the int16 gather-index regions). Chunk0 fires when dense tiles 0..31
    are stored; the next layer's aggregation runs in 2 passes (region A
    then region B) so its region-A gathers need only chunk0 -> chunk1's
    transfer hides under the region-A gather. Pass A's PSUM is staged to
    DRAM as an fp16 partial and re-added in pass B via an identity matmul.
  - Host preprocessing: edges bucketed per (core=dst core, dst tile, src
    region), padded to 128-multiples of the cross-core max so all 8 cores
    run one SPMD program.
"""

import numpy as np
import concourse.bass as bass
import concourse.mybir as mybir
import concourse.tile as tile

FP16 = mybir.dt.float16
F32 = mybir.dt.float32
I16 = mybir.dt.int16

P = 128
NCORE = 8
PCN = 6250            # real nodes per core
NPC = 6272            # padded nodes per core (49 tiles)
NT = 49
C0_ROWS = 4096        # AG chunk 0 rows per core (tiles 0..31)
C1_ROWS = NPC - C0_ROWS  # 2176 rows (tiles 32..48)
T_SPLIT = 32          # first tile of chunk 1
REG0 = NCORE * C0_ROWS   # 32768 = gpid boundary of region 0
NPT = NCORE * NPC        # 50176 padded total nodes
GMAX = 8              # chunks per gather call (1024 idx ucode cap)
SBATCH = 8            # chunks per batched S-build DVE op

FEAT = 512
WG = [512, 1024, 512, 128, 128]   # gather/agg width per layer l = 0..4
# dense widths: d0: 512->2048(T out), d1: 2048->1024, d2: 1024->512,
#               d3: 512->128, d4: 128->128, d5: 128->4


# ---------------------------------------------------------------- tile patch
def apply_tile_patch():
    """This walrus build allows only 1 sync-wait per Drain; split the tail
    drain's waits across a chain of drains."""
    import bass_rust

    def _drain_and_barrier_split(self, tick_clock, wait_clock):
        from bass_rust import ScopedClock
        drain_inst = self.nc.sync.drain()
        wait_clock.add_sem_waits(
            drain_inst.ins, ScopedClock({None: tick_clock.global_clock})
        )
        si = drain_inst.ins.sync_info
        waits = list(si.on_wait) if si is not None else []
        if len(waits) > 1:
            si.on_wait = [waits[0]]
            for w in waits[1:]:
                extra = self.nc.sync.drain()
                if extra.ins.sync_info is None:
                    extra.ins.sync_info = bass_rust.SyncInfo(
                        on_wait=[w], on_update=[])
                else:
                    extra.ins.sync_info.on_wait = [w]
        self.nc.all_engine_barrier()
        popped = self.nc._tile_sem_poison_stack.pop()
        assert popped is self._sem_poison
        self.nc.clear_and_free_semaphores(list(self.sems.allocated().values()))
        self.nc.all_engine_barrier()

    tile.TileContext._drain_and_barrier = _drain_and_barrier_split


def _gpid(core, slot):
    """Chunk-major padded global id (vectorized)."""
    return np.where(
        slot < C0_ROWS,
        core * C0_ROWS + slot,
        REG0 + core * C1_ROWS + (slot - C0_ROWS),
    )


# ------------------------------------------------------------- preprocess
def preprocess(x, edge_index, edge_attr, Ws, bs, Wp, bp):
    """Host-side: normalization, edge sharding/sorting/packing, weight packs.
    Returns (in_maps, meta)."""
    N = NCORE * PCN
    src = np.asarray(edge_index[0], dtype=np.int64)
    dst = np.asarray(edge_index[1], dtype=np.int64)
    ew = np.asarray(edge_attr, dtype=np.float32)
    loop = np.arange(N, dtype=np.int64)
    src2 = np.concatenate([src, loop])
    dst2 = np.concatenate([dst, loop])
    ew2 = np.concatenate([ew, np.ones(N, np.float32)])

    deg = np.bincount(dst2, weights=ew2.astype(np.float64), minlength=N)
    deg = deg.astype(np.float32)
    dinv = np.where(deg > 0, 1.0 / np.sqrt(deg), 0.0).astype(np.float32)
    normf = (dinv[src2] * ew2 * dinv[dst2]).astype(np.float32)  # full norm

    score = src2 // PCN
    sslot = src2 - score * PCN
    gpid = _gpid(score, sslot)

    core_of = dst2 // PCN
    slot = dst2 - core_of * PCN
    tile_of = slot // P
    slot_in = slot % P
    region_of = (gpid >= REG0).astype(np.int64)

    NR = 2
    counts = np.zeros((NCORE, NT, NR), np.int64)
    np.add.at(counts, (core_of, tile_of, region_of), 1)
    kmax = counts.max(axis=0)                      # [NT, NR]
    K = ((kmax + P - 1) // P) * P
    K[kmax == 0] = 0

    order = np.lexsort((region_of, tile_of, core_of))
    so_gpid = gpid[order]
    so_norm = normf[order]
    so_slot = slot_in[order]
    so_core = core_of[order]
    so_tile = tile_of[order]
    so_reg = region_of[order]

    icol = np.zeros((NT, NR), np.int64)
    cbase = np.zeros((NT, NR), np.int64)
    ic = cc = 0
    for t in range(NT):
        for r in range(NR):
            icol[t, r] = ic
            cbase[t, r] = cc
            ic += K[t, r] // 16
            cc += K[t, r] // P
    idxcols, nch = ic, cc

    # full padded X in gpid layout (shared by all cores)
    x_np = np.asarray(x, dtype=np.float32)
    xf = np.zeros((NPT, FEAT), np.float16)
    nodes = np.arange(N, dtype=np.int64)
    xf[_gpid(nodes // PCN, nodes % PCN)] = x_np.astype(np.float16)

    # weight packs
    w_list = [np.asarray(w, np.float32) for w in Ws] + [np.asarray(Wp, np.float32)]
    b_list = [np.asarray(b, np.float32) for b in bs] + [np.asarray(bp, np.float32)]
    # W1 [512,2048] as lhsT blocks (kb in 4, mb in 16): col (kb*16+mb)*128
    W1 = w_list[0]
    w1p = np.zeros((P, 4 * 16 * P), np.float16)
    for kb in range(4):
        for mb in range(16):
            w1p[:, (kb * 16 + mb) * P:(kb * 16 + mb + 1) * P] = (
                W1[kb * P:(kb + 1) * P, mb * P:(mb + 1) * P].astype(np.float16))
    # W2..W5, Wp as rhs blocks (kb-major): [128, nk*Md]
    def rhs_pack(Wr, Kd, Md):
        wp_ = np.zeros((Kd, Md), np.float32)
        wp_[: Wr.shape[0], : Wr.shape[1]] = Wr
        nk = Kd // P
        out = np.zeros((P, nk * Md), np.float16)
        for kb in range(nk):
            out[:, kb * Md:(kb + 1) * Md] = wp_[kb * P:(kb + 1) * P].astype(np.float16)
        return out

    w2p = rhs_pack(w_list[1], 2048, 1024)
    w3p = rhs_pack(w_list[2], 1024, 512)
    w4p = rhs_pack(w_list[3], 512, 128)
    w5p = rhs_pack(w_list[4], 128, 128)
    wpp = rhs_pack(w_list[5], 128, 4)

    # biases as per-partition columns [128, nblk]
    def bias_cols(b, width):
        bb = np.zeros(width, np.float32)
        bb[: b.shape[0]] = b
        return bb.reshape(width // P, P).T.astype(np.float16).copy()

    b1c = bias_cols(b_list[0], 2048)   # d0 epilogue (relu)
    b2c = bias_cols(b_list[1], 1024)   # agg1 epilogue
    b3c = bias_cols(b_list[2], 512)    # agg2
    b4c = bias_cols(b_list[3], 128)    # agg3
    b5c = bias_cols(b_list[4], 128)    # agg4
    bpr = np.zeros((1, 4), np.float16)
    bpr[0, :3] = b_list[5].astype(np.float16)

    iota = np.tile(np.arange(P, dtype=np.float16), (P, 1))
    ident = np.eye(P, dtype=np.float16)
    ones1 = np.ones((1, P), np.float16)

    core_starts = np.searchsorted(so_core, np.arange(NCORE + 1))
    in_maps = []
    for c in range(NCORE):
        lo, hi = core_starts[c], core_starts[c + 1]
        ct, cr = so_tile[lo:hi], so_reg[lo:hi]
        cg, cn, cs = so_gpid[lo:hi], so_norm[lo:hi], so_slot[lo:hi]
        idx16 = np.zeros((16, idxcols), np.int16)
        slotp = np.zeros((P, nch), np.float16)
        normp = np.zeros((P, nch), np.float16)
        pos = 0
        for t in range(NT):
            for r in range(NR):
                k = int(K[t, r])
                if k == 0:
                    continue
                n_e = int(counts[c, t, r])
                seg = slice(pos, pos + n_e)
                assert np.all(ct[seg] == t) and np.all(cr[seg] == r), (c, t, r)
                reg_lo = 0 if r == 0 else REG0
                arr = np.zeros(k, np.int64)
                arr[:n_e] = cg[seg] - reg_lo
                assert 0 <= arr.min(initial=0) and arr.max(initial=0) < 32768
                idx16[:, int(icol[t, r]): int(icol[t, r]) + k // 16] = (
                    arr.reshape(k // 16, 16).T.astype(np.int16))
                sl = np.zeros(k, np.float32)
                sl[:n_e] = cs[seg]
                nm = np.zeros(k, np.float32)
                nm[:n_e] = cn[seg]
                cb = int(cbase[t, r])
                slotp[:, cb: cb + k // P] = (
                    sl.reshape(k // P, P).T.astype(np.float16))
                normp[:, cb: cb + k // P] = (
                    nm.reshape(k // P, P).T.astype(np.float16))
                pos += n_e
        assert pos == hi - lo

        m = {
            "xf": xf,
            "idx16": np.tile(idx16, (8, 1)),
            "slotp": slotp,
            "normp": normp,
            "iota": iota,
            "ident": ident,
            "ones1": ones1,
            "w1": w1p, "w2": w2p, "w3": w3p, "w4": w4p, "w5": w5p, "wp": wpp,
            "b1": b1c, "b2": b2c, "b3": b3c, "b4": b4c, "b5": b5c, "bp": bpr,
        }
        in_maps.append(m)

    meta = dict(K=K, icol=icol, cbase=cbase, idxcols=idxcols, nch=nch)
    return in_maps, meta


# ---------------------------------------------------------------- program
def _bc3(ap, ncols, inner=P, mode="col"):
    base = ap.ap
    if mode == "col":
        return bass.AP(ap.tensor, ap.offset, [base[0], [1, ncols], [0, inner]])
    else:
        return bass.AP(ap.tensor, ap.offset, [base[0], [0, ncols], base[1]])


def _3d(ap, ncols, inner=P):
    return bass.AP(ap.tensor, ap.offset, [ap.ap[0], [inner, ncols], [1, inner]])


def build_program(meta):
    import concourse.bacc as bacc
    nc = bacc.Bacc("TRN2", num_swdge_queues=4)
    K, icol, cbase = meta["K"], meta["icol"], meta["cbase"]
    idxcols, nch = meta["idxcols"], meta["nch"]
    rg = [list(range(NCORE))]

    # ---------------- params
    pr = {}
    def par(name, shape, dt):
        pr[name] = nc.declare_dram_parameter(name, shape, dt, isOutput=False)
    par("xf", [NPT, FEAT], FP16)
    par("idx16", [P, idxcols], I16)
    par("slotp", [P, nch], FP16)
    par("normp", [P, nch], FP16)
    par("iota", [P, P], FP16)
    par("ident", [P, P], FP16)
    par("ones1", [1, P], FP16)
    par("w1", [P, 4 * 16 * P], FP16)
    par("w2", [P, 16 * 1024], FP16)
    par("w3", [P, 8 * 512], FP16)
    par("w4", [P, 4 * 128], FP16)
    par("w5", [P, 128], FP16)
    par("wp", [P, 4], FP16)
    par("b1", [P, 16], FP16)
    par("b2", [P, 8], FP16)
    par("b3", [P, 4], FP16)
    par("b4", [P, 1], FP16)
    par("b5", [P, 1], FP16)
    par("bp", [1, 4], FP16)
    out_ext = nc.declare_dram_parameter("out", [PCN, 3], F32, isOutput=True)

    # ---------------- internal DRAM
    # layers l=1..4 aggregate T_{l+1}; width WG[l]
    tA = {l: nc.dram_tensor(f"ta{l}", [C0_ROWS, WG[l]], FP16) for l in range(1, 5)}
    tB = {l: nc.dram_tensor(f"tb{l}", [C1_ROWS, WG[l]], FP16) for l in range(1, 5)}
    TFA = {l: nc.dram_tensor(f"tfa{l}", [REG0, WG[l]], FP16, addr_space="Shared")
           for l in range(1, 5)}
    TFB = {l: nc.dram_tensor(f"tfb{l}", [NPT - REG0, WG[l]], FP16,
                             addr_space="Shared")
           for l in range(1, 5)}
    PART = {l: nc.dram_tensor(f"part{l}", [NT * P, WG[l]], FP16)
            for l in range(1, 5)}

    with tile.TileContext(nc) as tc:
        import contextlib
        with contextlib.ExitStack() as ctx:
            cpool = ctx.enter_context(tc.tile_pool(name="const", bufs=1))
            msgp = ctx.enter_context(tc.tile_pool(name="msg", bufs=2))
            spool = ctx.enter_context(tc.tile_pool(name="sb", bufs=3))
            pp = ctx.enter_context(tc.tile_pool(name="ps", bufs=2, space="PSUM"))
            hp = ctx.enter_context(tc.tile_pool(name="hh", bufs=2))

            # ---- resident constants
            def cload(name, shape, dt):
                t_ = cpool.tile(shape, dt, tag=name, name=name)
                nc.sync.dma_start(out=t_[:], in_=pr[name][:])
                return t_
            idx_sb = cload("idx16", [P, idxcols], I16)
            slot_sb = cload("slotp", [P, nch], FP16)
            norm_sb = cload("normp", [P, nch], FP16)
            iota_sb = cload("iota", [P, P], FP16)
            ident_sb = cload("ident", [P, P], FP16)
            ones_sb = cload("ones1", [1, P], FP16)
            w1_sb = cload("w1", [P, 4 * 16 * P], FP16)
            w2_sb = cload("w2", [P, 16 * 1024], FP16)
            w3_sb = cload("w3", [P, 8 * 512], FP16)
            w4_sb = cload("w4", [P, 4 * 128], FP16)
            w5_sb = cload("w5", [P, 128], FP16)
            wp_sb = cload("wp", [P, 4], FP16)
            b1_sb = cload("b1", [P, 16], FP16)
            b2_sb = cload("b2", [P, 8], FP16)
            b3_sb = cload("b3", [P, 4], FP16)
            b4_sb = cload("b4", [P, 1], FP16)
            b5_sb = cload("b5", [P, 1], FP16)
            bp_sb = cload("bp", [1, 4], FP16)

            qn = [0]

            def build_s(t, r_list):
                """One S tile covering the chunks of (tile t, regions r_list)
                (contiguous in cbase layout when r_list is [0,1] or single)."""
                c_lo = int(cbase[t, r_list[0]])
                ctn = sum(int(K[t, r]) // P for r in r_list)
                s_t = spool.tile([P, max(ctn, 1) * P], FP16, tag="s", name="s_t")
                for b0 in range(0, ctn, SBATCH):
                    nb = min(SBATCH, ctn - b0)
                    cb0 = c_lo + b0
                    o3 = _3d(s_t[:, b0 * P:(b0 + nb) * P], nb)
                    nc.vector.tensor_tensor(
                        out=o3,
                        in0=_bc3(slot_sb[:, cb0:cb0 + nb], nb, mode="col"),
                        in1=_bc3(iota_sb[:], nb, mode="mat"),
                        op=mybir.AluOpType.is_equal)
                    nc.vector.tensor_tensor(
                        out=o3, in0=o3,
                        in1=_bc3(norm_sb[:, cb0:cb0 + nb], nb, mode="col"),
                        op=mybir.AluOpType.mult)
                return s_t, ctn

            def gather_mm(t, r, src_dram, W, s_t, s_coff, ps, mmcnt, mmtot,
                          use_start=True):
                """Gathers + aggregation matmuls for (tile t, region r).
                s_coff: chunk offset of this region within s_t.
                mmcnt: chunks already accumulated into ps; mmtot: total
                expected (stop flags on the last). use_start=False when the
                psum was already initialized (identity re-add). Returns new
                mmcnt."""
                k = int(K[t, r])
                if k == 0:
                    return mmcnt
                nf = W // P
                pos = 0
                while pos < k:
                    ks = min(GMAX * P, k - pos)
                    ic = int(icol[t, r]) + pos // 16
                    msg = msgp.tile([P, GMAX * 1024], FP16, tag="msg", name="msg")
                    nc.gpsimd.dma_gather(
                        out_ap=_3d(msg[:, : (ks // P) * W], ks // P, inner=W),
                        in_ap=src_dram,
                        idxs_ap=idx_sb[:, ic: ic + ks // 16],
                        num_idxs=ks,
                        num_idxs_reg=ks,
                        elem_size=W,
                        elem_step=W,
                        queue_num=qn[0],
                    )
                    qn[0] = (qn[0] + 1) % 4
                    for ci in range(ks // P):
                        cglob = s_coff + pos // P + ci
                        for fb in range(nf):
                            nc.tensor.matmul(
                                out=ps[:, fb * P:(fb + 1) * P],
                                lhsT=msg[:, ci * W + fb * P: ci * W + (fb + 1) * P],
                                rhs=s_t[:, cglob * P:(cglob + 1) * P],
                                start=(use_start and mmcnt == 0),
                                stop=(mmcnt == mmtot - 1))
                        mmcnt += 1
                    pos += ks
                return mmcnt

            def store_town(l, t, t_sb, W):
                if t < T_SPLIT:
                    nc.sync.dma_start(
                        out=tA[l][t * P:(t + 1) * P, :], in_=t_sb[:, :W])
                else:
                    t2 = t - T_SPLIT
                    nc.sync.dma_start(
                        out=tB[l][t2 * P:(t2 + 1) * P, :], in_=t_sb[:, :W])

            def emit_ag(l, chunk):
                if chunk == 0:
                    nc.gpsimd.collective_compute(
                        "AllGather", mybir.AluOpType.bypass, replica_groups=rg,
                        ins=[tA[l][:]], outs=[TFA[l][:]])
                else:
                    nc.gpsimd.collective_compute(
                        "AllGather", mybir.AluOpType.bypass, replica_groups=rg,
                        ins=[tB[l][:]], outs=[TFB[l][:]])

            # ================= phase 1: agg0(X) + d0 + d1 -> T2 =================
            for t in range(NT):
                s_t, ctn = build_s(t, [0, 1])
                ps = pp.tile([P, 1024], F32, tag="agg", name="ps_agg")
                mm = 0
                mm = gather_mm(t, 0, pr["xf"][0:REG0, :], 512, s_t, 0,
                               ps, mm, ctn)
                mm = gather_mm(t, 1, pr["xf"][REG0:NPT, :], 512, s_t,
                               int(K[t, 0]) // P, ps, mm, ctn)
                assert mm == ctn and ctn > 0
                g1t = hp.tile([P, 512], FP16, tag="g1t", name="g1t")
                nc.scalar.activation(
                    out=g1t[:], in_=ps[:, :512],
                    func=mybir.ActivationFunctionType.Copy)
                # d0: H1^T = relu(W1^T-blocks @ G1^T + b1), 4 quarters
                h1t = hp.tile([P, 2048], FP16, tag="h1t", name="h1t")
                for q in range(4):
                    ps0 = pp.tile([P, 512], F32, tag="d0", name="ps_d0")
                    for mi in range(4):
                        mb = q * 4 + mi
                        for kb in range(4):
                            nc.tensor.matmul(
                                out=ps0[:, mi * P:(mi + 1) * P],
                                lhsT=w1_sb[:, (kb * 16 + mb) * P:(kb * 16 + mb + 1) * P],
                                rhs=g1t[:, kb * P:(kb + 1) * P],
                                start=(kb == 0), stop=(kb == 3))
                    for mi in range(4):
                        mb = q * 4 + mi
                        nc.scalar.activation(
                            out=h1t[:, mb * P:(mb + 1) * P],
                            in_=ps0[:, mi * P:(mi + 1) * P],
                            func=mybir.ActivationFunctionType.Relu,
                            bias=b1_sb[:, mb:mb + 1])
                # d1: T2 = H1 @ W2 (normal orientation), 2 halves of 512
                t2sb = hp.tile([P, 1024], FP16, tag="tout", name="t2sb")
                for h in range(2):
                    psd = pp.tile([P, 512], F32, tag="d", name="ps_d")
                    for kb in range(16):
                        nc.tensor.matmul(
                            out=psd[:],
                            lhsT=h1t[:, kb * P:(kb + 1) * P],
                            rhs=w2_sb[:, kb * 1024 + h * 512: kb * 1024 + h * 512 + 512],
                            start=(kb == 0), stop=(kb == 15))
                    nc.scalar.activation(
                        out=t2sb[:, h * 512:(h + 1) * 512], in_=psd[:],
                        func=mybir.ActivationFunctionType.Copy)
                store_town(1, t, t2sb, 1024)
                if t == T_SPLIT - 1:
                    emit_ag(1, 0)
            emit_ag(1, 1)

            # ============== layers l=1..4: agg_l (2-pass) + dense ==============
            # agg_l consumes TF[l] (width WG[l]), produces H^{l+1,T}; dense
            # d_{l+1} produces T_{l+2} (towns l+1) or the final output.
            for l in range(1, 5):
                W = WG[l]
                nf = W // P
                bias_sb = {1: b2_sb, 2: b3_sb, 3: b4_sb, 4: b5_sb}[l]
                # ---- pass A (region 0) -> fp16 partial in DRAM
                for t in range(NT):
                    s_t, ctn = build_s(t, [0])
                    ps = pp.tile([P, 1024], F32, tag="agg", name="ps_agg")
                    mm = gather_mm(t, 0, TFA[l][:], W, s_t, 0, ps, 0,
                                   max(ctn, 1))
                    if ctn == 0:
                        nc.vector.memset(ps[:, :W], 0.0)
                    pa = hp.tile([P, 1024], FP16, tag="pa", name="pa")
                    nc.scalar.activation(
                        out=pa[:, :W], in_=ps[:, :W],
                        func=mybir.ActivationFunctionType.Copy)
                    nc.sync.dma_start(
                        out=PART[l][t * P:(t + 1) * P, :], in_=pa[:, :W])
                # ---- pass B (region 1) + identity re-add + epilogue + dense
                for t in range(NT):
                    s_t, ctn = build_s(t, [1])
                    ps = pp.tile([P, 1024], F32, tag="agg", name="ps_agg")
                    pb = hp.tile([P, 1024], FP16, tag="pb", name="pb")
                    nc.sync.dma_start(
                        out=pb[:, :W], in_=PART[l][t * P:(t + 1) * P, :])
                    nid = (W + 511) // 512
                    for j in range(nid):
                        w_ = min(512, W - j * 512)
                        nc.tensor.matmul(
                            out=ps[:, j * 512: j * 512 + w_],
                            lhsT=ident_sb[:],
                            rhs=pb[:, j * 512: j * 512 + w_],
                            start=True, stop=(ctn == 0 and j == nid - 1))
                    if ctn > 0:
                        # identity already wrote every fb slice; chunk matmuls
                        # accumulate (start=False) and the last one stops.
                        gather_mm(t, 1, TFB[l][:], W, s_t, 0, ps, 0, ctn,
                                  use_start=False)
                    hT = hp.tile([P, 1024], FP16, tag="ht", name="hT")
                    for fb in range(nf):
                        nc.scalar.activation(
                            out=hT[:, fb * P:(fb + 1) * P],
                            in_=ps[:, fb * P:(fb + 1) * P],
                            func=mybir.ActivationFunctionType.Relu,
                            bias=bias_sb[:, fb:fb + 1])
                    # dense d_{l+1}
                    if l == 1:
                        # H2[1024] @ W3 -> T3 [512]
                        t3 = hp.tile([P, 512], FP16, tag="tout", name="t3")
                        psd = pp.tile([P, 512], F32, tag="d", name="ps_d")
                        for kb in range(8):
                            nc.tensor.matmul(
                                out=psd[:],
                                lhsT=hT[:, kb * P:(kb + 1) * P],
                                rhs=w3_sb[:, kb * 512:(kb + 1) * 512],
                                start=(kb == 0), stop=(kb == 7))
                        nc.scalar.activation(
                            out=t3[:], in_=psd[:],
                            func=mybir.ActivationFunctionType.Copy)
                        store_town(2, t, t3, 512)
                        if t == T_SPLIT - 1:
                            emit_ag(2, 0)
                    elif l == 2:
                        # H3[512] @ W4 -> T4 [128]
                        t4 = hp.tile([P, 128], FP16, tag="tout4", name="t4")
                        psd = pp.tile([P, 512], F32, tag="d", name="ps_d")
                        for kb in range(4):
                            nc.tensor.matmul(
                                out=psd[:, :128],
                                lhsT=hT[:, kb * P:(kb + 1) * P],
                                rhs=w4_sb[:, kb * 128:(kb + 1) * 128],
                                start=(kb == 0), stop=(kb == 3))
                        nc.scalar.activation(
                            out=t4[:], in_=psd[:, :128],
                            func=mybir.ActivationFunctionType.Copy)
                        store_town(3, t, t4, 128)
                        if t == T_SPLIT - 1:
                            emit_ag(3, 0)
                    elif l == 3:
                        # H4[128] @ W5 -> T5 [128]
                        t5 = hp.tile([P, 128], FP16, tag="tout4", name="t5")
                        psd = pp.tile([P, 512], F32, tag="d", name="ps_d")
                        nc.tensor.matmul(
                            out=psd[:, :128], lhsT=hT[:, :128], rhs=w5_sb[:],
                            start=True, stop=True)
                        nc.scalar.activation(
                            out=t5[:], in_=psd[:, :128],
                            func=mybir.ActivationFunctionType.Copy)
                        store_town(4, t, t5, 128)
                        if t == T_SPLIT - 1:
                            emit_ag(4, 0)
                    else:
                        # d5: out = H5 @ Wp + bp
                        psd = pp.tile([P, 512], F32, tag="d", name="ps_d")
                        nc.tensor.matmul(
                            out=psd[:, :4], lhsT=hT[:, :128], rhs=wp_sb[:],
                            start=True, stop=False)
                        nc.tensor.matmul(
                            out=psd[:, :4], lhsT=ones_sb[0:1, :],
                            rhs=bp_sb[0:1, :], start=False, stop=True)
                        osb = hp.tile([P, 4], F32, tag="fout", name="osb")
                        nc.vector.tensor_copy(out=osb[:], in_=psd[:, :4])
                        r0 = t * P
                        r1 = min((t + 1) * P, PCN)
                        if r1 > r0:
                            nc.sync.dma_start(
                                out=out_ext[r0:r1, :], in_=osb[: r1 - r0, :3])
                if l < 4:
                    emit_ag(l + 1, 1)

    nc.finalize()
    return nc


# ------------------------------------------------------------------ driver
_CACHE = {}


def kernel(x, edge_index, edge_attr, W1, b1, W2, b2, W3, b3, W4, b4, W5, b5,
           Wp, bp):
    apply_tile_patch()
    import os
    from concourse.bass_utils import run_bass_kernel_spmd

    Ws = [W1, W2, W3, W4, W5]
    bs = [b1, b2, b3, b4, b5]
    in_maps, meta = preprocess(x, edge_index, edge_attr, Ws, bs, Wp, bp)

    key = (meta["K"].tobytes(), meta["nch"], meta["idxcols"])
    nc = _CACHE.get(key)
    if nc is None:
        nc = build_program(meta)
        _CACHE[key] = nc

    res = run_bass_kernel_spmd(
        nc, in_maps, core_ids=list(range(NCORE)),
        trace=bool(int(os.environ.get("TRACE", "0"))))
    if res.exec_time_ns:
        print(f"HW exec time: {res.exec_time_ns} ns")
    out = np.concatenate(
        [res.results[c]["out"] for c in range(NCORE)], axis=0)
    return np.ascontiguousarray(out.astype(np.float32))


# revision 20
# speedup vs baseline: 1.9409x; 1.3562x over previous
"""GCN (5-layer ColorGNN) Bass kernel for 8 TRN2 NeuronCores — v2.

Pipelined design (node-sharded SPMD, 6272 padded nodes/core = 49 tiles):

  - Full padded X (fp16, gpid layout) is staged on EVERY core: the layer-1
    aggregation gathers it directly -> no AllGather for layer 1.
  - Aggregation outputs are produced TRANSPOSED (H^T tiles: features on
    partitions, 128 dst nodes on free): out^T[f,n] = sum_e msg[e,f]*S[e,n]
    with lhsT=msg (gathered rows), rhs=S (one-hot x norm). The full
    symmetric norm dinv[src]*ew*dinv[dst] is folded into S on the host, and
    the bias is a per-partition ACT bias in the relu epilogue.
  - H^T tiles feed the next dense matmul as lhsT straight from SBUF: no
    transposes, no H round-trips through DRAM. Dense outputs T (node-major)
    go to DRAM only as AllGather input. Layer-1's dense pair runs in the
    transposed orientation (lhsT = W1 blocks) so the chain stays in H^T.
  - Each T AllGather is split in 2 node-chunks (4096+2176 rows/core,
    matching gpid regions [0,32768) and [32768,50176) which also serve as
    the int16 gather-index regions). Chunk0 fires when dense tiles 0..31
    are stored; the next layer's aggregation runs in 2 passes (region A
    then region B) so its region-A gathers need only chunk0 -> chunk1's
    transfer hides under the region-A gather. Pass A's PSUM is staged to
    DRAM as an fp16 partial and re-added in pass B via an identity matmul.
  - Host preprocessing: edges bucketed per (core=dst core, dst tile, src
    region), padded to 128-multiples of the cross-core max so all 8 cores
    run one SPMD program.
"""

import numpy as np
import concourse.bass as bass
import concourse.mybir as mybir
import concourse.tile as tile

FP16 = mybir.dt.float16
F32 = mybir.dt.float32
I16 = mybir.dt.int16

P = 128
NCORE = 8
PCN = 6250            # real nodes per core
NPC = 6272            # padded nodes per core (49 tiles)
NT = 49
# AllGather chunks per core (tile-aligned): tiles 0-15 / 16-31 / 32-48
CH = [0, 2048, 4096, 6272]       # per-core row boundaries
GB = [0, 16384, 32768, 50176]    # global gpid base of each chunk
REG0 = 32768          # gpid boundary of gather region 0 (chunks 0a+0b)
NPT = NCORE * NPC     # 50176 padded total nodes
T_SPLIT = 32          # first tile of gather-region 1
GMAX = 8              # chunks per gather call (1024 idx ucode cap)
SBATCH = 8            # chunks per batched S-build DVE op

FEAT = 512
WG = [512, 1024, 512, 128, 128]   # gather/agg width per layer l = 0..4
# dense widths: d0: 512->2048(T out), d1: 2048->1024, d2: 1024->512,
#               d3: 512->128, d4: 128->128, d5: 128->4


# ---------------------------------------------------------------- tile patch
def apply_tile_patch():
    """This walrus build allows only 1 sync-wait per Drain; split the tail
    drain's waits across a chain of drains."""
    import bass_rust

    def _drain_and_barrier_split(self, tick_clock, wait_clock):
        from bass_rust import ScopedClock
        drain_inst = self.nc.sync.drain()
        wait_clock.add_sem_waits(
            drain_inst.ins, ScopedClock({None: tick_clock.global_clock})
        )
        si = drain_inst.ins.sync_info
        waits = list(si.on_wait) if si is not None else []
        if len(waits) > 1:
            si.on_wait = [waits[0]]
            for w in waits[1:]:
                extra = self.nc.sync.drain()
                if extra.ins.sync_info is None:
                    extra.ins.sync_info = bass_rust.SyncInfo(
                        on_wait=[w], on_update=[])
                else:
                    extra.ins.sync_info.on_wait = [w]
        self.nc.all_engine_barrier()
        popped = self.nc._tile_sem_poison_stack.pop()
        assert popped is self._sem_poison
        self.nc.clear_and_free_semaphores(list(self.sems.allocated().values()))
        self.nc.all_engine_barrier()

    tile.TileContext._drain_and_barrier = _drain_and_barrier_split


def _gpid(core, slot):
    """Chunk-major padded global id (vectorized over 3 AG chunks)."""
    out = np.empty_like(np.broadcast_arrays(core, slot)[1])
    core = np.asarray(core)
    slot = np.asarray(slot)
    for j in range(3):
        m = (slot >= CH[j]) & (slot < CH[j + 1])
        w = CH[j + 1] - CH[j]
        out[m] = GB[j] + core[m] * w + (slot[m] - CH[j])
    return out


# ------------------------------------------------------------- preprocess
def preprocess(x, edge_index, edge_attr, Ws, bs, Wp, bp):
    """Host-side: normalization, edge sharding/sorting/packing, weight packs.
    Returns (in_maps, meta)."""
    N = NCORE * PCN
    src = np.asarray(edge_index[0], dtype=np.int64)
    dst = np.asarray(edge_index[1], dtype=np.int64)
    ew = np.asarray(edge_attr, dtype=np.float32)
    loop = np.arange(N, dtype=np.int64)
    src2 = np.concatenate([src, loop])
    dst2 = np.concatenate([dst, loop])
    ew2 = np.concatenate([ew, np.ones(N, np.float32)])

    deg = np.bincount(dst2, weights=ew2.astype(np.float64), minlength=N)
    deg = deg.astype(np.float32)
    dinv = np.where(deg > 0, 1.0 / np.sqrt(deg), 0.0).astype(np.float32)
    normf = (dinv[src2] * ew2 * dinv[dst2]).astype(np.float32)  # full norm

    score = src2 // PCN
    sslot = src2 - score * PCN
    gpid = _gpid(score, sslot)

    core_of = dst2 // PCN
    slot = dst2 - core_of * PCN
    tile_of = slot // P
    slot_in = slot % P
    region_of = (gpid >= REG0).astype(np.int64)

    NR = 2
    counts = np.zeros((NCORE, NT, NR), np.int64)
    np.add.at(counts, (core_of, tile_of, region_of), 1)
    kmax = counts.max(axis=0)                      # [NT, NR]
    K = ((kmax + P - 1) // P) * P
    K[kmax == 0] = 0

    order = np.lexsort((region_of, tile_of, core_of))
    so_gpid = gpid[order]
    so_norm = normf[order]
    so_slot = slot_in[order]
    so_core = core_of[order]
    so_tile = tile_of[order]
    so_reg = region_of[order]

    icol = np.zeros((NT, NR), np.int64)
    cbase = np.zeros((NT, NR), np.int64)
    ic = cc = 0
    for t in range(NT):
        for r in range(NR):
            icol[t, r] = ic
            cbase[t, r] = cc
            ic += K[t, r] // 16
            cc += K[t, r] // P
    idxcols, nch = ic, cc

    # full padded X in gpid layout (shared by all cores)
    x_np = np.asarray(x, dtype=np.float32)
    xf = np.zeros((NPT, FEAT), np.float16)
    nodes = np.arange(N, dtype=np.int64)
    xf[_gpid(nodes // PCN, nodes % PCN)] = x_np.astype(np.float16)

    # weight packs
    w_list = [np.asarray(w, np.float32) for w in Ws] + [np.asarray(Wp, np.float32)]
    b_list = [np.asarray(b, np.float32) for b in bs] + [np.asarray(bp, np.float32)]
    # W1 [512,2048] as lhsT blocks (kb in 4, mb in 16): col (kb*16+mb)*128
    W1 = w_list[0]
    w1p = np.zeros((P, 4 * 16 * P), np.float16)
    for kb in range(4):
        for mb in range(16):
            w1p[:, (kb * 16 + mb) * P:(kb * 16 + mb + 1) * P] = (
                W1[kb * P:(kb + 1) * P, mb * P:(mb + 1) * P].astype(np.float16))
    # W2..W5, Wp as rhs blocks (kb-major): [128, nk*Md]
    def rhs_pack(Wr, Kd, Md):
        wp_ = np.zeros((Kd, Md), np.float32)
        wp_[: Wr.shape[0], : Wr.shape[1]] = Wr
        nk = Kd // P
        out = np.zeros((P, nk * Md), np.float16)
        for kb in range(nk):
            out[:, kb * Md:(kb + 1) * Md] = wp_[kb * P:(kb + 1) * P].astype(np.float16)
        return out

    w2p = rhs_pack(w_list[1], 2048, 1024)
    w3p = rhs_pack(w_list[2], 1024, 512)
    w4p = rhs_pack(w_list[3], 512, 128)
    w5p = rhs_pack(w_list[4], 128, 128)
    wpp = rhs_pack(w_list[5], 128, 4)

    # biases as per-partition columns [128, nblk]
    def bias_cols(b, width):
        bb = np.zeros(width, np.float32)
        bb[: b.shape[0]] = b
        return bb.reshape(width // P, P).T.astype(np.float16).copy()

    b1c = bias_cols(b_list[0], 2048)   # d0 epilogue (relu)
    b2c = bias_cols(b_list[1], 1024)   # agg1 epilogue
    b3c = bias_cols(b_list[2], 512)    # agg2
    b4c = bias_cols(b_list[3], 128)    # agg3
    b5c = bias_cols(b_list[4], 128)    # agg4
    bpr = np.zeros((1, 4), np.float16)
    bpr[0, :3] = b_list[5].astype(np.float16)

    iota = np.tile(np.arange(P, dtype=np.float16), (P, 1))
    ident = np.eye(P, dtype=np.float16)
    ones1 = np.ones((1, P), np.float16)

    core_starts = np.searchsorted(so_core, np.arange(NCORE + 1))
    in_maps = []
    for c in range(NCORE):
        lo, hi = core_starts[c], core_starts[c + 1]
        ct, cr = so_tile[lo:hi], so_reg[lo:hi]
        cg, cn, cs = so_gpid[lo:hi], so_norm[lo:hi], so_slot[lo:hi]
        idx16 = np.zeros((16, idxcols), np.int16)
        slotp = np.zeros((P, nch), np.float16)
        normp = np.zeros((P, nch), np.float16)
        pos = 0
        for t in range(NT):
            for r in range(NR):
                k = int(K[t, r])
                if k == 0:
                    continue
                n_e = int(counts[c, t, r])
                seg = slice(pos, pos + n_e)
                assert np.all(ct[seg] == t) and np.all(cr[seg] == r), (c, t, r)
                reg_lo = 0 if r == 0 else REG0
                arr = np.zeros(k, np.int64)
                arr[:n_e] = cg[seg] - reg_lo
                assert 0 <= arr.min(initial=0) and arr.max(initial=0) < 32768
                idx16[:, int(icol[t, r]): int(icol[t, r]) + k // 16] = (
                    arr.reshape(k // 16, 16).T.astype(np.int16))
                sl = np.zeros(k, np.float32)
                sl[:n_e] = cs[seg]
                nm = np.zeros(k, np.float32)
                nm[:n_e] = cn[seg]
                cb = int(cbase[t, r])
                slotp[:, cb: cb + k // P] = (
                    sl.reshape(k // P, P).T.astype(np.float16))
                normp[:, cb: cb + k // P] = (
                    nm.reshape(k // P, P).T.astype(np.float16))
                pos += n_e
        assert pos == hi - lo

        m = {
            "xf": xf,
            "idx16": np.tile(idx16, (8, 1)),
            "slotp": slotp,
            "normp": normp,
            "iota": iota,
            "ident": ident,
            "ones1": ones1,
            "w1": w1p, "w2": w2p, "w3": w3p, "w4": w4p, "w5": w5p, "wp": wpp,
            "b1": b1c, "b2": b2c, "b3": b3c, "b4": b4c, "b5": b5c, "bp": bpr,
        }
        in_maps.append(m)

    meta = dict(K=K, icol=icol, cbase=cbase, idxcols=idxcols, nch=nch)
    return in_maps, meta


# ---------------------------------------------------------------- program
def _bc3(ap, ncols, inner=P, mode="col"):
    base = ap.ap
    if mode == "col":
        return bass.AP(ap.tensor, ap.offset, [base[0], [1, ncols], [0, inner]])
    else:
        return bass.AP(ap.tensor, ap.offset, [base[0], [0, ncols], base[1]])


def _3d(ap, ncols, inner=P):
    return bass.AP(ap.tensor, ap.offset, [ap.ap[0], [inner, ncols], [1, inner]])


def build_program(meta):
    import concourse.bacc as bacc
    nc = bacc.Bacc("TRN2", num_swdge_queues=4)
    K, icol, cbase = meta["K"], meta["icol"], meta["cbase"]
    idxcols, nch = meta["idxcols"], meta["nch"]
    rg = [list(range(NCORE))]

    # ---------------- params
    pr = {}
    def par(name, shape, dt):
        pr[name] = nc.declare_dram_parameter(name, shape, dt, isOutput=False)
    par("xf", [NPT, FEAT], FP16)
    par("idx16", [P, idxcols], I16)
    par("slotp", [P, nch], FP16)
    par("normp", [P, nch], FP16)
    par("iota", [P, P], FP16)
    par("ident", [P, P], FP16)
    par("ones1", [1, P], FP16)
    par("w1", [P, 4 * 16 * P], FP16)
    par("w2", [P, 16 * 1024], FP16)
    par("w3", [P, 8 * 512], FP16)
    par("w4", [P, 4 * 128], FP16)
    par("w5", [P, 128], FP16)
    par("wp", [P, 4], FP16)
    par("b1", [P, 16], FP16)
    par("b2", [P, 8], FP16)
    par("b3", [P, 4], FP16)
    par("b4", [P, 1], FP16)
    par("b5", [P, 1], FP16)
    par("bp", [1, 4], FP16)
    out_ext = nc.declare_dram_parameter("out", [PCN, 3], F32, isOutput=True)

    # ---------------- internal DRAM
    # layers l=1..4 aggregate T_{l+1}; width WG[l]
    town = {}   # (l, chunk j) -> per-core town tensor
    for l in range(1, 5):
        for j in range(3):
            town[l, j] = nc.dram_tensor(
                f"tn{l}_{j}", [CH[j + 1] - CH[j], WG[l]], FP16)
    TFA = {l: nc.dram_tensor(f"tfa{l}", [REG0, WG[l]], FP16, addr_space="Shared")
           for l in range(1, 5)}
    TFB = {l: nc.dram_tensor(f"tfb{l}", [NPT - REG0, WG[l]], FP16,
                             addr_space="Shared")
           for l in range(1, 5)}
    PART = {l: nc.dram_tensor(f"part{l}", [NT * P, WG[l]], FP16)
            for l in range(1, 5)}

    with tile.TileContext(nc) as tc:
        import contextlib
        with contextlib.ExitStack() as ctx:
            cpool = ctx.enter_context(tc.tile_pool(name="const", bufs=1))
            msgp = ctx.enter_context(tc.tile_pool(name="msg", bufs=4))
            spool = ctx.enter_context(tc.tile_pool(name="sb", bufs=3))
            pp = ctx.enter_context(tc.tile_pool(name="ps", bufs=2, space="PSUM"))
            hp = ctx.enter_context(tc.tile_pool(name="hh", bufs=2))

            # ---- resident constants
            def cload(name, shape, dt):
                t_ = cpool.tile(shape, dt, tag=name, name=name)
                nc.sync.dma_start(out=t_[:], in_=pr[name][:])
                return t_
            idx_sb = cload("idx16", [P, idxcols], I16)
            slot_sb = cload("slotp", [P, nch], FP16)
            norm_sb = cload("normp", [P, nch], FP16)
            iota_sb = cload("iota", [P, P], FP16)
            ident_sb = cload("ident", [P, P], FP16)
            ones_sb = cload("ones1", [1, P], FP16)
            w1_sb = cload("w1", [P, 4 * 16 * P], FP16)
            w2_sb = cload("w2", [P, 16 * 1024], FP16)
            w3_sb = cload("w3", [P, 8 * 512], FP16)
            w4_sb = cload("w4", [P, 4 * 128], FP16)
            w5_sb = cload("w5", [P, 128], FP16)
            wp_sb = cload("wp", [P, 4], FP16)
            b1_sb = cload("b1", [P, 16], FP16)
            b2_sb = cload("b2", [P, 8], FP16)
            b3_sb = cload("b3", [P, 4], FP16)
            b4_sb = cload("b4", [P, 1], FP16)
            b5_sb = cload("b5", [P, 1], FP16)
            bp_sb = cload("bp", [1, 4], FP16)

            qn = [0]

            def build_s(t, r_list):
                """One S tile covering the chunks of (tile t, regions r_list)
                (contiguous in cbase layout when r_list is [0,1] or single)."""
                c_lo = int(cbase[t, r_list[0]])
                ctn = sum(int(K[t, r]) // P for r in r_list)
                s_t = spool.tile([P, max(ctn, 1) * P], FP16, tag="s", name="s_t")
                for b0 in range(0, ctn, SBATCH):
                    nb = min(SBATCH, ctn - b0)
                    cb0 = c_lo + b0
                    o3 = _3d(s_t[:, b0 * P:(b0 + nb) * P], nb)
                    nc.vector.tensor_tensor(
                        out=o3,
                        in0=_bc3(slot_sb[:, cb0:cb0 + nb], nb, mode="col"),
                        in1=_bc3(iota_sb[:], nb, mode="mat"),
                        op=mybir.AluOpType.is_equal)
                    nc.vector.tensor_tensor(
                        out=o3, in0=o3,
                        in1=_bc3(norm_sb[:, cb0:cb0 + nb], nb, mode="col"),
                        op=mybir.AluOpType.mult)
                return s_t, ctn

            def gather_mm(t, r, src_dram, W, s_t, s_coff, ps, mmcnt, mmtot,
                          use_start=True):
                """Gathers + aggregation matmuls for (tile t, region r).
                s_coff: chunk offset of this region within s_t.
                mmcnt: chunks already accumulated into ps; mmtot: total
                expected (stop flags on the last). use_start=False when the
                psum was already initialized (identity re-add). Returns new
                mmcnt."""
                k = int(K[t, r])
                if k == 0:
                    return mmcnt
                nf = W // P
                gmax = GMAX if W <= 512 else (4096 // W)  # cap msg at 8KB/part
                pos = 0
                while pos < k:
                    ks = min(gmax * P, k - pos)
                    ic = int(icol[t, r]) + pos // 16
                    msg = msgp.tile([P, 4096], FP16, tag="msg", name="msg")
                    nc.gpsimd.dma_gather(
                        out_ap=_3d(msg[:, : (ks // P) * W], ks // P, inner=W),
                        in_ap=src_dram,
                        idxs_ap=idx_sb[:, ic: ic + ks // 16],
                        num_idxs=ks,
                        num_idxs_reg=ks,
                        elem_size=W,
                        elem_step=W,
                        queue_num=qn[0],
                    )
                    qn[0] = (qn[0] + 1) % 4
                    for ci in range(ks // P):
                        cglob = s_coff + pos // P + ci
                        for fb in range(nf):
                            nc.tensor.matmul(
                                out=ps[:, fb * P:(fb + 1) * P],
                                lhsT=msg[:, ci * W + fb * P: ci * W + (fb + 1) * P],
                                rhs=s_t[:, cglob * P:(cglob + 1) * P],
                                start=(use_start and mmcnt == 0),
                                stop=(mmcnt == mmtot - 1))
                        mmcnt += 1
                    pos += ks
                return mmcnt

            def store_town(l, t, t_sb, W):
                j = 0 if t < 16 else (1 if t < 32 else 2)
                r0 = t * P - CH[j]
                nc.sync.dma_start(
                    out=town[l, j][r0:r0 + P, :], in_=t_sb[:, :W])

            def emit_ag(l, j):
                if j < 2:
                    out_ap = TFA[l][GB[j]:GB[j + 1], :]
                else:
                    out_ap = TFB[l][:]
                nc.gpsimd.collective_compute(
                    "AllGather", mybir.AluOpType.bypass, replica_groups=rg,
                    ins=[town[l, j][:]], outs=[out_ap])

            # ================= phase 1: agg0(X) + d0 + d1 -> T2 =================
            for t in range(NT):
                s_t, ctn = build_s(t, [0, 1])
                ps = pp.tile([P, 1024], F32, tag="agg", name="ps_agg")
                mm = 0
                mm = gather_mm(t, 0, pr["xf"][0:REG0, :], 512, s_t, 0,
                               ps, mm, ctn)
                mm = gather_mm(t, 1, pr["xf"][REG0:NPT, :], 512, s_t,
                               int(K[t, 0]) // P, ps, mm, ctn)
                assert mm == ctn and ctn > 0
                g1t = hp.tile([P, 512], FP16, tag="g1t", name="g1t")
                nc.scalar.activation(
                    out=g1t[:], in_=ps[:, :512],
                    func=mybir.ActivationFunctionType.Copy)
                # d0: H1^T = relu(W1^T-blocks @ G1^T + b1), 4 quarters
                h1t = hp.tile([P, 2048], FP16, tag="h1t", name="h1t")
                for q in range(4):
                    ps0 = pp.tile([P, 512], F32, tag="d0", name="ps_d0")
                    for mi in range(4):
                        mb = q * 4 + mi
                        for kb in range(4):
                            nc.tensor.matmul(
                                out=ps0[:, mi * P:(mi + 1) * P],
                                lhsT=w1_sb[:, (kb * 16 + mb) * P:(kb * 16 + mb + 1) * P],
                                rhs=g1t[:, kb * P:(kb + 1) * P],
                                start=(kb == 0), stop=(kb == 3))
                    for mi in range(4):
                        mb = q * 4 + mi
                        nc.scalar.activation(
                            out=h1t[:, mb * P:(mb + 1) * P],
                            in_=ps0[:, mi * P:(mi + 1) * P],
                            func=mybir.ActivationFunctionType.Relu,
                            bias=b1_sb[:, mb:mb + 1])
                # d1: T2 = H1 @ W2 (normal orientation), 2 halves of 512
                t2sb = hp.tile([P, 1024], FP16, tag="tout", name="t2sb")
                for h in range(2):
                    psd = pp.tile([P, 512], F32, tag="d", name="ps_d")
                    for kb in range(16):
                        nc.tensor.matmul(
                            out=psd[:],
                            lhsT=h1t[:, kb * P:(kb + 1) * P],
                            rhs=w2_sb[:, kb * 1024 + h * 512: kb * 1024 + h * 512 + 512],
                            start=(kb == 0), stop=(kb == 15))
                    nc.scalar.activation(
                        out=t2sb[:, h * 512:(h + 1) * 512], in_=psd[:],
                        func=mybir.ActivationFunctionType.Copy)
                store_town(1, t, t2sb, 1024)
                if t == 15:
                    emit_ag(1, 0)
                elif t == 31:
                    emit_ag(1, 1)
            emit_ag(1, 2)

            # ============== layers l=1..4: agg_l (2-pass) + dense ==============
            # agg_l consumes TF[l] (width WG[l]), produces H^{l+1,T}; dense
            # d_{l+1} produces T_{l+2} (towns l+1) or the final output.
            for l in range(1, 5):
                W = WG[l]
                nf = W // P
                bias_sb = {1: b2_sb, 2: b3_sb, 3: b4_sb, 4: b5_sb}[l]
                # ---- pass A (region 0) -> fp16 partial in DRAM
                for t in range(NT):
                    s_t, ctn = build_s(t, [0])
                    ps = pp.tile([P, 1024], F32, tag="agg", name="ps_agg")
                    mm = gather_mm(t, 0, TFA[l][:], W, s_t, 0, ps, 0,
                                   max(ctn, 1))
                    if ctn == 0:
                        nc.vector.memset(ps[:, :W], 0.0)
                    pa = hp.tile([P, 1024], FP16, tag="pa", name="pa")
                    nc.scalar.activation(
                        out=pa[:, :W], in_=ps[:, :W],
                        func=mybir.ActivationFunctionType.Copy)
                    nc.sync.dma_start(
                        out=PART[l][t * P:(t + 1) * P, :], in_=pa[:, :W])
                # ---- pass B (region 1) + identity re-add + epilogue + dense
                for t in range(NT):
                    s_t, ctn = build_s(t, [1])
                    ps = pp.tile([P, 1024], F32, tag="agg", name="ps_agg")
                    pb = hp.tile([P, 1024], FP16, tag="pb", name="pb")
                    nc.sync.dma_start(
                        out=pb[:, :W], in_=PART[l][t * P:(t + 1) * P, :])
                    nid = (W + 511) // 512
                    for j in range(nid):
                        w_ = min(512, W - j * 512)
                        nc.tensor.matmul(
                            out=ps[:, j * 512: j * 512 + w_],
                            lhsT=ident_sb[:],
                            rhs=pb[:, j * 512: j * 512 + w_],
                            start=True, stop=(ctn == 0 and j == nid - 1))
                    if ctn > 0:
                        # identity already wrote every fb slice; chunk matmuls
                        # accumulate (start=False) and the last one stops.
                        gather_mm(t, 1, TFB[l][:], W, s_t, 0, ps, 0, ctn,
                                  use_start=False)
                    hT = hp.tile([P, 1024], FP16, tag="ht", name="hT")
                    for fb in range(nf):
                        nc.scalar.activation(
                            out=hT[:, fb * P:(fb + 1) * P],
                            in_=ps[:, fb * P:(fb + 1) * P],
                            func=mybir.ActivationFunctionType.Relu,
                            bias=bias_sb[:, fb:fb + 1])
                    # dense d_{l+1}
                    if l == 1:
                        # H2[1024] @ W3 -> T3 [512]
                        t3 = hp.tile([P, 512], FP16, tag="tout", name="t3")
                        psd = pp.tile([P, 512], F32, tag="d", name="ps_d")
                        for kb in range(8):
                            nc.tensor.matmul(
                                out=psd[:],
                                lhsT=hT[:, kb * P:(kb + 1) * P],
                                rhs=w3_sb[:, kb * 512:(kb + 1) * 512],
                                start=(kb == 0), stop=(kb == 7))
                        nc.scalar.activation(
                            out=t3[:], in_=psd[:],
                            func=mybir.ActivationFunctionType.Copy)
                        store_town(2, t, t3, 512)
                        if t == 15:
                            emit_ag(2, 0)
                        elif t == 31:
                            emit_ag(2, 1)
                    elif l == 2:
                        # H3[512] @ W4 -> T4 [128]
                        t4 = hp.tile([P, 128], FP16, tag="tout4", name="t4")
                        psd = pp.tile([P, 512], F32, tag="d", name="ps_d")
                        for kb in range(4):
                            nc.tensor.matmul(
                                out=psd[:, :128],
                                lhsT=hT[:, kb * P:(kb + 1) * P],
                                rhs=w4_sb[:, kb * 128:(kb + 1) * 128],
                                start=(kb == 0), stop=(kb == 3))
                        nc.scalar.activation(
                            out=t4[:], in_=psd[:, :128],
                            func=mybir.ActivationFunctionType.Copy)
                        store_town(3, t, t4, 128)
                        if t == 15:
                            emit_ag(3, 0)
                        elif t == 31:
                            emit_ag(3, 1)
                    elif l == 3:
                        # H4[128] @ W5 -> T5 [128]
                        t5 = hp.tile([P, 128], FP16, tag="tout4", name="t5")
                        psd = pp.tile([P, 512], F32, tag="d", name="ps_d")
                        nc.tensor.matmul(
                            out=psd[:, :128], lhsT=hT[:, :128], rhs=w5_sb[:],
                            start=True, stop=True)
                        nc.scalar.activation(
                            out=t5[:], in_=psd[:, :128],
                            func=mybir.ActivationFunctionType.Copy)
                        store_town(4, t, t5, 128)
                        if t == 15:
                            emit_ag(4, 0)
                        elif t == 31:
                            emit_ag(4, 1)
                    else:
                        # d5: out = H5 @ Wp + bp
                        psd = pp.tile([P, 512], F32, tag="d", name="ps_d")
                        nc.tensor.matmul(
                            out=psd[:, :4], lhsT=hT[:, :128], rhs=wp_sb[:],
                            start=True, stop=False)
                        nc.tensor.matmul(
                            out=psd[:, :4], lhsT=ones_sb[0:1, :],
                            rhs=bp_sb[0:1, :], start=False, stop=True)
                        osb = hp.tile([P, 4], F32, tag="fout", name="osb")
                        nc.vector.tensor_copy(out=osb[:], in_=psd[:, :4])
                        r0 = t * P
                        r1 = min((t + 1) * P, PCN)
                        if r1 > r0:
                            nc.sync.dma_start(
                                out=out_ext[r0:r1, :], in_=osb[: r1 - r0, :3])
                if l < 4:
                    emit_ag(l + 1, 2)

    nc.finalize()
    return nc


# ------------------------------------------------------------------ driver
_CACHE = {}


def kernel(x, edge_index, edge_attr, W1, b1, W2, b2, W3, b3, W4, b4, W5, b5,
           Wp, bp):
    apply_tile_patch()
    import os
    from concourse.bass_utils import run_bass_kernel_spmd

    Ws = [W1, W2, W3, W4, W5]
    bs = [b1, b2, b3, b4, b5]
    in_maps, meta = preprocess(x, edge_index, edge_attr, Ws, bs, Wp, bp)

    key = (meta["K"].tobytes(), meta["nch"], meta["idxcols"])
    nc = _CACHE.get(key)
    if nc is None:
        nc = build_program(meta)
        _CACHE[key] = nc

    res = run_bass_kernel_spmd(
        nc, in_maps, core_ids=list(range(NCORE)),
        trace=bool(int(os.environ.get("TRACE", "0"))))
    if res.exec_time_ns:
        print(f"HW exec time: {res.exec_time_ns} ns")
    out = np.concatenate(
        [res.results[c]["out"] for c in range(NCORE)], axis=0)
    return np.ascontiguousarray(out.astype(np.float32))


# revision 23
# speedup vs baseline: 2.0018x; 1.0314x over previous
"""GCN (5-layer ColorGNN) Bass kernel for 8 TRN2 NeuronCores — v2.

Pipelined design (node-sharded SPMD, 6272 padded nodes/core = 49 tiles):

  - Full padded X (fp16, gpid layout) is staged on EVERY core: the layer-1
    aggregation gathers it directly -> no AllGather for layer 1.
  - Aggregation outputs are produced TRANSPOSED (H^T tiles: features on
    partitions, 128 dst nodes on free): out^T[f,n] = sum_e msg[e,f]*S[e,n]
    with lhsT=msg (gathered rows), rhs=S (one-hot x norm). The full
    symmetric norm dinv[src]*ew*dinv[dst] is folded into S on the host, and
    the bias is a per-partition ACT bias in the relu epilogue.
  - H^T tiles feed the next dense matmul as lhsT straight from SBUF: no
    transposes, no H round-trips through DRAM. Dense outputs T (node-major)
    go to DRAM only as AllGather input. Layer-1's dense pair runs in the
    transposed orientation (lhsT = W1 blocks) so the chain stays in H^T.
  - Each T AllGather is split in 2 node-chunks (4096+2176 rows/core,
    matching gpid regions [0,32768) and [32768,50176) which also serve as
    the int16 gather-index regions). Chunk0 fires when dense tiles 0..31
    are stored; the next layer's aggregation runs in 2 passes (region A
    then region B) so its region-A gathers need only chunk0 -> chunk1's
    transfer hides under the region-A gather. Pass A's PSUM is staged to
    DRAM as an fp16 partial and re-added in pass B via an identity matmul.
  - Host preprocessing: edges bucketed per (core=dst core, dst tile, src
    region), padded to 128-multiples of the cross-core max so all 8 cores
    run one SPMD program.
"""

import numpy as np
import concourse.bass as bass
import concourse.mybir as mybir
import concourse.tile as tile

FP16 = mybir.dt.float16
F32 = mybir.dt.float32
I16 = mybir.dt.int16

P = 128
NCORE = 8
PCN = 6250            # real nodes per core
NPC = 6272            # padded nodes per core (49 tiles)
NT = 49
# AllGather chunks per core (tile-aligned): tiles 0-15 / 16-31 / 32-48
CH = [0, 2048, 4096, 6272]       # per-core row boundaries
GB = [0, 16384, 32768, 50176]    # global gpid base of each chunk
REG0 = 32768          # gpid boundary of gather region 0 (chunks 0a+0b)
NPT = NCORE * NPC     # 50176 padded total nodes
T_SPLIT = 32          # first tile of gather-region 1
GMAX = 8              # chunks per gather call (1024 idx ucode cap)
SBATCH = 8            # chunks per batched S-build DVE op

FEAT = 512
WG = [512, 1024, 512, 128, 128]   # gather/agg width per layer l = 0..4
# dense widths: d0: 512->2048(T out), d1: 2048->1024, d2: 1024->512,
#               d3: 512->128, d4: 128->128, d5: 128->4


# ---------------------------------------------------------------- tile patch
def apply_tile_patch():
    """This walrus build allows only 1 sync-wait per Drain; split the tail
    drain's waits across a chain of drains."""
    import bass_rust

    def _drain_and_barrier_split(self, tick_clock, wait_clock):
        from bass_rust import ScopedClock
        drain_inst = self.nc.sync.drain()
        wait_clock.add_sem_waits(
            drain_inst.ins, ScopedClock({None: tick_clock.global_clock})
        )
        si = drain_inst.ins.sync_info
        waits = list(si.on_wait) if si is not None else []
        if len(waits) > 1:
            si.on_wait = [waits[0]]
            for w in waits[1:]:
                extra = self.nc.sync.drain()
                if extra.ins.sync_info is None:
                    extra.ins.sync_info = bass_rust.SyncInfo(
                        on_wait=[w], on_update=[])
                else:
                    extra.ins.sync_info.on_wait = [w]
        self.nc.all_engine_barrier()
        popped = self.nc._tile_sem_poison_stack.pop()
        assert popped is self._sem_poison
        self.nc.clear_and_free_semaphores(list(self.sems.allocated().values()))
        self.nc.all_engine_barrier()

    tile.TileContext._drain_and_barrier = _drain_and_barrier_split


def _gpid(core, slot):
    """Chunk-major padded global id (vectorized over 3 AG chunks)."""
    out = np.empty_like(np.broadcast_arrays(core, slot)[1])
    core = np.asarray(core)
    slot = np.asarray(slot)
    for j in range(3):
        m = (slot >= CH[j]) & (slot < CH[j + 1])
        w = CH[j + 1] - CH[j]
        out[m] = GB[j] + core[m] * w + (slot[m] - CH[j])
    return out


# ------------------------------------------------------------- preprocess
def preprocess(x, edge_index, edge_attr, Ws, bs, Wp, bp):
    """Host-side: normalization, edge sharding/sorting/packing, weight packs.
    Returns (in_maps, meta)."""
    N = NCORE * PCN
    src = np.asarray(edge_index[0], dtype=np.int64)
    dst = np.asarray(edge_index[1], dtype=np.int64)
    ew = np.asarray(edge_attr, dtype=np.float32)
    loop = np.arange(N, dtype=np.int64)
    src2 = np.concatenate([src, loop])
    dst2 = np.concatenate([dst, loop])
    ew2 = np.concatenate([ew, np.ones(N, np.float32)])

    deg = np.bincount(dst2, weights=ew2.astype(np.float64), minlength=N)
    deg = deg.astype(np.float32)
    dinv = np.where(deg > 0, 1.0 / np.sqrt(deg), 0.0).astype(np.float32)
    normf = (dinv[src2] * ew2 * dinv[dst2]).astype(np.float32)  # full norm

    score = src2 // PCN
    sslot = src2 - score * PCN
    gpid = _gpid(score, sslot)

    core_of = dst2 // PCN
    slot = dst2 - core_of * PCN
    tile_of = slot // P
    slot_in = slot % P
    region_of = np.digitize(gpid, GB[1:3]).astype(np.int64)

    NR = 3
    counts = np.zeros((NCORE, NT, NR), np.int64)
    np.add.at(counts, (core_of, tile_of, region_of), 1)
    kmax = counts.max(axis=0)                      # [NT, NR]
    K = ((kmax + P - 1) // P) * P
    K[kmax == 0] = 0

    order = np.lexsort((region_of, tile_of, core_of))
    so_gpid = gpid[order]
    so_norm = normf[order]
    so_slot = slot_in[order]
    so_core = core_of[order]
    so_tile = tile_of[order]
    so_reg = region_of[order]

    icol = np.zeros((NT, NR), np.int64)
    cbase = np.zeros((NT, NR), np.int64)
    ic = cc = 0
    for t in range(NT):
        for r in range(NR):
            icol[t, r] = ic
            cbase[t, r] = cc
            ic += K[t, r] // 16
            cc += K[t, r] // P
    idxcols, nch = ic, cc

    # full padded X in gpid layout (shared by all cores)
    x_np = np.asarray(x, dtype=np.float32)
    xf = np.zeros((NPT, FEAT), np.float16)
    nodes = np.arange(N, dtype=np.int64)
    xf[_gpid(nodes // PCN, nodes % PCN)] = x_np.astype(np.float16)

    # weight packs
    w_list = [np.asarray(w, np.float32) for w in Ws] + [np.asarray(Wp, np.float32)]
    b_list = [np.asarray(b, np.float32) for b in bs] + [np.asarray(bp, np.float32)]
    # W1 [512,2048] as lhsT blocks (kb in 4, mb in 16): col (kb*16+mb)*128
    W1 = w_list[0]
    w1p = np.zeros((P, 4 * 16 * P), np.float16)
    for kb in range(4):
        for mb in range(16):
            w1p[:, (kb * 16 + mb) * P:(kb * 16 + mb + 1) * P] = (
                W1[kb * P:(kb + 1) * P, mb * P:(mb + 1) * P].astype(np.float16))
    # W2..W5, Wp as rhs blocks (kb-major): [128, nk*Md]
    def rhs_pack(Wr, Kd, Md):
        wp_ = np.zeros((Kd, Md), np.float32)
        wp_[: Wr.shape[0], : Wr.shape[1]] = Wr
        nk = Kd // P
        out = np.zeros((P, nk * Md), np.float16)
        for kb in range(nk):
            out[:, kb * Md:(kb + 1) * Md] = wp_[kb * P:(kb + 1) * P].astype(np.float16)
        return out

    w2p = rhs_pack(w_list[1], 2048, 1024)
    w3p = rhs_pack(w_list[2], 1024, 512)
    w4p = rhs_pack(w_list[3], 512, 128)
    w5p = rhs_pack(w_list[4], 128, 128)
    wpp = rhs_pack(w_list[5], 128, 4)

    # biases as per-partition columns [128, nblk]
    def bias_cols(b, width):
        bb = np.zeros(width, np.float32)
        bb[: b.shape[0]] = b
        return bb.reshape(width // P, P).T.astype(np.float16).copy()

    b1c = bias_cols(b_list[0], 2048)   # d0 epilogue (relu)
    b2c = bias_cols(b_list[1], 1024)   # agg1 epilogue
    b3c = bias_cols(b_list[2], 512)    # agg2
    b4c = bias_cols(b_list[3], 128)    # agg3
    b5c = bias_cols(b_list[4], 128)    # agg4
    bpr = np.zeros((1, 4), np.float16)
    bpr[0, :3] = b_list[5].astype(np.float16)

    iota = np.tile(np.arange(P, dtype=np.float16), (P, 1))
    ident = np.eye(P, dtype=np.float16)
    ones1 = np.ones((1, P), np.float16)

    core_starts = np.searchsorted(so_core, np.arange(NCORE + 1))
    in_maps = []
    for c in range(NCORE):
        lo, hi = core_starts[c], core_starts[c + 1]
        ct, cr = so_tile[lo:hi], so_reg[lo:hi]
        cg, cn, cs = so_gpid[lo:hi], so_norm[lo:hi], so_slot[lo:hi]
        idx16 = np.zeros((16, idxcols), np.int16)
        slotp = np.zeros((P, nch), np.float16)
        normp = np.zeros((P, nch), np.float16)
        pos = 0
        for t in range(NT):
            for r in range(NR):
                k = int(K[t, r])
                if k == 0:
                    continue
                n_e = int(counts[c, t, r])
                seg = slice(pos, pos + n_e)
                assert np.all(ct[seg] == t) and np.all(cr[seg] == r), (c, t, r)
                reg_lo = GB[r]
                arr = np.zeros(k, np.int64)
                arr[:n_e] = cg[seg] - reg_lo
                assert 0 <= arr.min(initial=0) and arr.max(initial=0) < 32768
                idx16[:, int(icol[t, r]): int(icol[t, r]) + k // 16] = (
                    arr.reshape(k // 16, 16).T.astype(np.int16))
                sl = np.zeros(k, np.float32)
                sl[:n_e] = cs[seg]
                nm = np.zeros(k, np.float32)
                nm[:n_e] = cn[seg]
                cb = int(cbase[t, r])
                slotp[:, cb: cb + k // P] = (
                    sl.reshape(k // P, P).T.astype(np.float16))
                normp[:, cb: cb + k // P] = (
                    nm.reshape(k // P, P).T.astype(np.float16))
                pos += n_e
        assert pos == hi - lo

        m = {
            "xf": xf,
            "idx16": np.tile(idx16, (8, 1)),
            "slotp": slotp,
            "normp": normp,
            "iota": iota,
            "ident": ident,
            "ones1": ones1,
            "w1": w1p, "w2": w2p, "w3": w3p, "w4": w4p, "w5": w5p, "wp": wpp,
            "b1": b1c, "b2": b2c, "b3": b3c, "b4": b4c, "b5": b5c, "bp": bpr,
        }
        in_maps.append(m)

    meta = dict(K=K, icol=icol, cbase=cbase, idxcols=idxcols, nch=nch)
    return in_maps, meta


# ---------------------------------------------------------------- program
def _bc3(ap, ncols, inner=P, mode="col"):
    base = ap.ap
    if mode == "col":
        return bass.AP(ap.tensor, ap.offset, [base[0], [1, ncols], [0, inner]])
    else:
        return bass.AP(ap.tensor, ap.offset, [base[0], [0, ncols], base[1]])


def _3d(ap, ncols, inner=P):
    return bass.AP(ap.tensor, ap.offset, [ap.ap[0], [inner, ncols], [1, inner]])


def build_program(meta):
    import concourse.bacc as bacc
    nc = bacc.Bacc("TRN2", num_swdge_queues=4)
    K, icol, cbase = meta["K"], meta["icol"], meta["cbase"]
    idxcols, nch = meta["idxcols"], meta["nch"]
    rg = [list(range(NCORE))]

    # ---------------- params
    pr = {}
    def par(name, shape, dt):
        pr[name] = nc.declare_dram_parameter(name, shape, dt, isOutput=False)
    par("xf", [NPT, FEAT], FP16)
    par("idx16", [P, idxcols], I16)
    par("slotp", [P, nch], FP16)
    par("normp", [P, nch], FP16)
    par("iota", [P, P], FP16)
    par("ident", [P, P], FP16)
    par("ones1", [1, P], FP16)
    par("w1", [P, 4 * 16 * P], FP16)
    par("w2", [P, 16 * 1024], FP16)
    par("w3", [P, 8 * 512], FP16)
    par("w4", [P, 4 * 128], FP16)
    par("w5", [P, 128], FP16)
    par("wp", [P, 4], FP16)
    par("b1", [P, 16], FP16)
    par("b2", [P, 8], FP16)
    par("b3", [P, 4], FP16)
    par("b4", [P, 1], FP16)
    par("b5", [P, 1], FP16)
    par("bp", [1, 4], FP16)
    out_ext = nc.declare_dram_parameter("out", [PCN, 3], F32, isOutput=True)

    # ---------------- internal DRAM
    # layers l=1..4 aggregate T_{l+1}; width WG[l]
    town = {}   # (l, chunk j) -> per-core town tensor
    for l in range(1, 5):
        for j in range(3):
            town[l, j] = nc.dram_tensor(
                f"tn{l}_{j}", [CH[j + 1] - CH[j], WG[l]], FP16)
    TF = {}
    for l in range(1, 5):
        for j in range(3):
            TF[l, j] = nc.dram_tensor(
                f"tf{l}_{j}", [(GB[j + 1] - GB[j]), WG[l]], FP16,
                addr_space="Shared")
    PART = {(l, p): nc.dram_tensor(f"part{l}_{p}", [NT * P, WG[l]], FP16)
            for l in range(1, 5) for p in range(2)}

    with tile.TileContext(nc) as tc:
        import contextlib
        with contextlib.ExitStack() as ctx:
            cpool = ctx.enter_context(tc.tile_pool(name="const", bufs=1))
            msgp = ctx.enter_context(tc.tile_pool(name="msg", bufs=4))
            spool = ctx.enter_context(tc.tile_pool(name="sb", bufs=3))
            pp = ctx.enter_context(tc.tile_pool(name="ps", bufs=2, space="PSUM"))
            hp = ctx.enter_context(tc.tile_pool(name="hh", bufs=2))

            # ---- resident constants
            def cload(name, shape, dt):
                t_ = cpool.tile(shape, dt, tag=name, name=name)
                nc.sync.dma_start(out=t_[:], in_=pr[name][:])
                return t_
            idx_sb = cload("idx16", [P, idxcols], I16)
            slot_sb = cload("slotp", [P, nch], FP16)
            norm_sb = cload("normp", [P, nch], FP16)
            iota_sb = cload("iota", [P, P], FP16)
            ident_sb = cload("ident", [P, P], FP16)
            ones_sb = cload("ones1", [1, P], FP16)
            w1_sb = cload("w1", [P, 4 * 16 * P], FP16)
            w2_sb = cload("w2", [P, 16 * 1024], FP16)
            w3_sb = cload("w3", [P, 8 * 512], FP16)
            w4_sb = cload("w4", [P, 4 * 128], FP16)
            w5_sb = cload("w5", [P, 128], FP16)
            wp_sb = cload("wp", [P, 4], FP16)
            b1_sb = cload("b1", [P, 16], FP16)
            b2_sb = cload("b2", [P, 8], FP16)
            b3_sb = cload("b3", [P, 4], FP16)
            b4_sb = cload("b4", [P, 1], FP16)
            b5_sb = cload("b5", [P, 1], FP16)
            bp_sb = cload("bp", [1, 4], FP16)

            qn = [0]

            def build_s(t, r_list):
                """One S tile covering the chunks of (tile t, regions r_list)
                (contiguous in cbase layout when r_list is [0,1] or single)."""
                c_lo = int(cbase[t, r_list[0]])
                ctn = sum(int(K[t, r]) // P for r in r_list)
                s_t = spool.tile([P, max(ctn, 1) * P], FP16, tag="s", name="s_t")
                for b0 in range(0, ctn, SBATCH):
                    nb = min(SBATCH, ctn - b0)
                    cb0 = c_lo + b0
                    o3 = _3d(s_t[:, b0 * P:(b0 + nb) * P], nb)
                    nc.vector.tensor_tensor(
                        out=o3,
                        in0=_bc3(slot_sb[:, cb0:cb0 + nb], nb, mode="col"),
                        in1=_bc3(iota_sb[:], nb, mode="mat"),
                        op=mybir.AluOpType.is_equal)
                    nc.vector.tensor_tensor(
                        out=o3, in0=o3,
                        in1=_bc3(norm_sb[:, cb0:cb0 + nb], nb, mode="col"),
                        op=mybir.AluOpType.mult)
                return s_t, ctn

            def gather_mm(t, r, src_dram, W, s_t, s_coff, ps, mmcnt, mmtot,
                          use_start=True):
                """Gathers + aggregation matmuls for (tile t, region r).
                s_coff: chunk offset of this region within s_t.
                mmcnt: chunks already accumulated into ps; mmtot: total
                expected (stop flags on the last). use_start=False when the
                psum was already initialized (identity re-add). Returns new
                mmcnt."""
                k = int(K[t, r])
                if k == 0:
                    return mmcnt
                nf = W // P
                gmax = GMAX if W <= 512 else (4096 // W)  # cap msg at 8KB/part
                pos = 0
                while pos < k:
                    ks = min(gmax * P, k - pos)
                    ic = int(icol[t, r]) + pos // 16
                    msg = msgp.tile([P, 4096], FP16, tag="msg", name="msg")
                    nc.gpsimd.dma_gather(
                        out_ap=_3d(msg[:, : (ks // P) * W], ks // P, inner=W),
                        in_ap=src_dram,
                        idxs_ap=idx_sb[:, ic: ic + ks // 16],
                        num_idxs=ks,
                        num_idxs_reg=ks,
                        elem_size=W,
                        elem_step=W,
                        queue_num=qn[0],
                    )
                    qn[0] = (qn[0] + 1) % 4
                    for ci in range(ks // P):
                        cglob = s_coff + pos // P + ci
                        for fb in range(nf):
                            nc.tensor.matmul(
                                out=ps[:, fb * P:(fb + 1) * P],
                                lhsT=msg[:, ci * W + fb * P: ci * W + (fb + 1) * P],
                                rhs=s_t[:, cglob * P:(cglob + 1) * P],
                                start=(use_start and mmcnt == 0),
                                stop=(mmcnt == mmtot - 1))
                        mmcnt += 1
                    pos += ks
                return mmcnt

            def store_town(l, t, t_sb, W):
                j = 0 if t < 16 else (1 if t < 32 else 2)
                r0 = t * P - CH[j]
                nc.sync.dma_start(
                    out=town[l, j][r0:r0 + P, :], in_=t_sb[:, :W])

            def emit_ag(l, j):
                nc.gpsimd.collective_compute(
                    "AllGather", mybir.AluOpType.bypass, replica_groups=rg,
                    ins=[town[l, j][:]], outs=[TF[l, j][:]])

            # ================= phase 1: agg0(X) + d0 + d1 -> T2 =================
            for t in range(NT):
                s_t, ctn = build_s(t, [0, 1, 2])
                ps = pp.tile([P, 1024], F32, tag="agg", name="ps_agg")
                mm = 0
                coff = 0
                for r in range(3):
                    mm = gather_mm(t, r, pr["xf"][GB[r]:GB[r + 1], :], 512,
                                   s_t, coff, ps, mm, ctn)
                    coff += int(K[t, r]) // P
                assert mm == ctn and ctn > 0
                g1t = hp.tile([P, 512], FP16, tag="g1t", name="g1t")
                nc.scalar.activation(
                    out=g1t[:], in_=ps[:, :512],
                    func=mybir.ActivationFunctionType.Copy)
                # d0: H1^T = relu(W1^T-blocks @ G1^T + b1), 4 quarters
                h1t = hp.tile([P, 2048], FP16, tag="h1t", name="h1t")
                for q in range(4):
                    ps0 = pp.tile([P, 512], F32, tag="d0", name="ps_d0")
                    for mi in range(4):
                        mb = q * 4 + mi
                        for kb in range(4):
                            nc.tensor.matmul(
                                out=ps0[:, mi * P:(mi + 1) * P],
                                lhsT=w1_sb[:, (kb * 16 + mb) * P:(kb * 16 + mb + 1) * P],
                                rhs=g1t[:, kb * P:(kb + 1) * P],
                                start=(kb == 0), stop=(kb == 3))
                    for mi in range(4):
                        mb = q * 4 + mi
                        nc.scalar.activation(
                            out=h1t[:, mb * P:(mb + 1) * P],
                            in_=ps0[:, mi * P:(mi + 1) * P],
                            func=mybir.ActivationFunctionType.Relu,
                            bias=b1_sb[:, mb:mb + 1])
                # d1: T2 = H1 @ W2 (normal orientation), 2 halves of 512
                t2sb = hp.tile([P, 1024], FP16, tag="tout", name="t2sb")
                for h in range(2):
                    psd = pp.tile([P, 512], F32, tag="d", name="ps_d")
                    for kb in range(16):
                        nc.tensor.matmul(
                            out=psd[:],
                            lhsT=h1t[:, kb * P:(kb + 1) * P],
                            rhs=w2_sb[:, kb * 1024 + h * 512: kb * 1024 + h * 512 + 512],
                            start=(kb == 0), stop=(kb == 15))
                    nc.scalar.activation(
                        out=t2sb[:, h * 512:(h + 1) * 512], in_=psd[:],
                        func=mybir.ActivationFunctionType.Copy)
                store_town(1, t, t2sb, 1024)
                if t == 15:
                    emit_ag(1, 0)
                elif t == 31:
                    emit_ag(1, 1)
            emit_ag(1, 2)

            # ========= layers l=1..4: agg_l (3 passes, 1 per region) + dense ====
            # agg_l consumes TF[l,*] (width WG[l]), produces H^{l+1,T}; dense
            # d_{l+1} produces T_{l+2} (towns l+1) or the final output. Passes
            # 0/1 stage the PSUM to DRAM as fp16 partials; passes 1/2 re-add
            # them via an identity matmul.
            for l in range(1, 5):
                W = WG[l]
                nf = W // P
                bias_sb = {1: b2_sb, 2: b3_sb, 3: b4_sb, 4: b5_sb}[l]
                for p in range(3):
                    last = p == 2
                    for t in range(NT):
                        s_t, ctn = build_s(t, [p])
                        ps = pp.tile([P, 1024], F32, tag="agg", name="ps_agg")
                        if p > 0:
                            pb = hp.tile([P, 1024], FP16, tag="pb", name="pb")
                            nc.sync.dma_start(
                                out=pb[:, :W],
                                in_=PART[l, p - 1][t * P:(t + 1) * P, :])
                            nid = (W + 511) // 512
                            for j in range(nid):
                                w_ = min(512, W - j * 512)
                                nc.tensor.matmul(
                                    out=ps[:, j * 512: j * 512 + w_],
                                    lhsT=ident_sb[:],
                                    rhs=pb[:, j * 512: j * 512 + w_],
                                    start=True,
                                    stop=(ctn == 0 and j == nid - 1))
                            if ctn > 0:
                                gather_mm(t, p, TF[l, p][:], W, s_t, 0, ps,
                                          0, ctn, use_start=False)
                        else:
                            gather_mm(t, 0, TF[l, 0][:], W, s_t, 0, ps, 0,
                                      max(ctn, 1))
                            if ctn == 0:
                                nc.vector.memset(ps[:, :W], 0.0)
                        if not last:
                            pa = hp.tile([P, 1024], FP16, tag="pa", name="pa")
                            nc.scalar.activation(
                                out=pa[:, :W], in_=ps[:, :W],
                                func=mybir.ActivationFunctionType.Copy)
                            nc.sync.dma_start(
                                out=PART[l, p][t * P:(t + 1) * P, :],
                                in_=pa[:, :W])
                            continue
                        hT = hp.tile([P, 1024], FP16, tag="ht", name="hT")
                        for fb in range(nf):
                            nc.scalar.activation(
                                out=hT[:, fb * P:(fb + 1) * P],
                                in_=ps[:, fb * P:(fb + 1) * P],
                                func=mybir.ActivationFunctionType.Relu,
                                bias=bias_sb[:, fb:fb + 1])
                        # dense d_{l+1}
                        if l == 1:
                            # H2[1024] @ W3 -> T3 [512]
                            t3 = hp.tile([P, 512], FP16, tag="tout", name="t3")
                            psd = pp.tile([P, 512], F32, tag="d", name="ps_d")
                            for kb in range(8):
                                nc.tensor.matmul(
                                    out=psd[:],
                                    lhsT=hT[:, kb * P:(kb + 1) * P],
                                    rhs=w3_sb[:, kb * 512:(kb + 1) * 512],
                                    start=(kb == 0), stop=(kb == 7))
                            nc.scalar.activation(
                                out=t3[:], in_=psd[:],
                                func=mybir.ActivationFunctionType.Copy)
                            store_town(2, t, t3, 512)
                            if t == 15:
                                emit_ag(2, 0)
                            elif t == 31:
                                emit_ag(2, 1)
                        elif l == 2:
                            # H3[512] @ W4 -> T4 [128]
                            t4 = hp.tile([P, 128], FP16, tag="tout4", name="t4")
                            psd = pp.tile([P, 512], F32, tag="d", name="ps_d")
                            for kb in range(4):
                                nc.tensor.matmul(
                                    out=psd[:, :128],
                                    lhsT=hT[:, kb * P:(kb + 1) * P],
                                    rhs=w4_sb[:, kb * 128:(kb + 1) * 128],
                                    start=(kb == 0), stop=(kb == 3))
                            nc.scalar.activation(
                                out=t4[:], in_=psd[:, :128],
                                func=mybir.ActivationFunctionType.Copy)
                            store_town(3, t, t4, 128)
                            if t == 15:
                                emit_ag(3, 0)
                            elif t == 31:
                                emit_ag(3, 1)
                        elif l == 3:
                            # H4[128] @ W5 -> T5 [128]
                            t5 = hp.tile([P, 128], FP16, tag="tout4", name="t5")
                            psd = pp.tile([P, 512], F32, tag="d", name="ps_d")
                            nc.tensor.matmul(
                                out=psd[:, :128], lhsT=hT[:, :128],
                                rhs=w5_sb[:], start=True, stop=True)
                            nc.scalar.activation(
                                out=t5[:], in_=psd[:, :128],
                                func=mybir.ActivationFunctionType.Copy)
                            store_town(4, t, t5, 128)
                            if t == 15:
                                emit_ag(4, 0)
                            elif t == 31:
                                emit_ag(4, 1)
                        else:
                            # d5: out = H5 @ Wp + bp
                            psd = pp.tile([P, 512], F32, tag="d", name="ps_d")
                            nc.tensor.matmul(
                                out=psd[:, :4], lhsT=hT[:, :128], rhs=wp_sb[:],
                                start=True, stop=False)
                            nc.tensor.matmul(
                                out=psd[:, :4], lhsT=ones_sb[0:1, :],
                                rhs=bp_sb[0:1, :], start=False, stop=True)
                            osb = hp.tile([P, 4], F32, tag="fout", name="osb")
                            nc.vector.tensor_copy(out=osb[:], in_=psd[:, :4])
                            r0 = t * P
                            r1 = min((t + 1) * P, PCN)
                            if r1 > r0:
                                nc.sync.dma_start(
                                    out=out_ext[r0:r1, :],
                                    in_=osb[: r1 - r0, :3])
                if l < 4:
                    emit_ag(l + 1, 2)

    nc.finalize()
    return nc


# ------------------------------------------------------------------ driver
_CACHE = {}


def kernel(x, edge_index, edge_attr, W1, b1, W2, b2, W3, b3, W4, b4, W5, b5,
           Wp, bp):
    apply_tile_patch()
    import os
    from concourse.bass_utils import run_bass_kernel_spmd

    Ws = [W1, W2, W3, W4, W5]
    bs = [b1, b2, b3, b4, b5]
    in_maps, meta = preprocess(x, edge_index, edge_attr, Ws, bs, Wp, bp)

    key = (meta["K"].tobytes(), meta["nch"], meta["idxcols"])
    nc = _CACHE.get(key)
    if nc is None:
        nc = build_program(meta)
        _CACHE[key] = nc

    res = run_bass_kernel_spmd(
        nc, in_maps, core_ids=list(range(NCORE)),
        trace=bool(int(os.environ.get("TRACE", "0"))))
    if res.exec_time_ns:
        print(f"HW exec time: {res.exec_time_ns} ns")
    out = np.concatenate(
        [res.results[c]["out"] for c in range(NCORE)], axis=0)
    return np.ascontiguousarray(out.astype(np.float32))


# revision 27
# speedup vs baseline: 2.0141x; 1.0061x over previous
"""GCN (5-layer ColorGNN) Bass kernel for 8 TRN2 NeuronCores — v2.

Pipelined design (node-sharded SPMD, 6272 padded nodes/core = 49 tiles):

  - Full padded X (fp16, gpid layout) is staged on EVERY core: the layer-1
    aggregation gathers it directly -> no AllGather for layer 1.
  - Aggregation outputs are produced TRANSPOSED (H^T tiles: features on
    partitions, 128 dst nodes on free): out^T[f,n] = sum_e msg[e,f]*S[e,n]
    with lhsT=msg (gathered rows), rhs=S (one-hot x norm). The full
    symmetric norm dinv[src]*ew*dinv[dst] is folded into S on the host, and
    the bias is a per-partition ACT bias in the relu epilogue.
  - H^T tiles feed the next dense matmul as lhsT straight from SBUF: no
    transposes, no H round-trips through DRAM. Dense outputs T (node-major)
    go to DRAM only as AllGather input. Layer-1's dense pair runs in the
    transposed orientation (lhsT = W1 blocks) so the chain stays in H^T.
  - Each T AllGather is split in 2 node-chunks (4096+2176 rows/core,
    matching gpid regions [0,32768) and [32768,50176) which also serve as
    the int16 gather-index regions). Chunk0 fires when dense tiles 0..31
    are stored; the next layer's aggregation runs in 2 passes (region A
    then region B) so its region-A gathers need only chunk0 -> chunk1's
    transfer hides under the region-A gather. Pass A's PSUM is staged to
    DRAM as an fp16 partial and re-added in pass B via an identity matmul.
  - Host preprocessing: edges bucketed per (core=dst core, dst tile, src
    region), padded to 128-multiples of the cross-core max so all 8 cores
    run one SPMD program.
"""

import numpy as np
import concourse.bass as bass
import concourse.mybir as mybir
import concourse.tile as tile

FP16 = mybir.dt.float16
F32 = mybir.dt.float32
I16 = mybir.dt.int16

P = 128
NCORE = 8
PCN = 6250            # real nodes per core
NPC = 6272            # padded nodes per core (49 tiles)
NT = 49
# AllGather chunks per core (tile-aligned): tiles 0-15 / 16-31 / 32-48
CH = [0, 2048, 4096, 6272]       # per-core row boundaries
GB = [0, 16384, 32768, 50176]    # global gpid base of each chunk
REG0 = 32768          # gpid boundary of gather region 0 (chunks 0a+0b)
NPT = NCORE * NPC     # 50176 padded total nodes
T_SPLIT = 32          # first tile of gather-region 1
GMAX = 8              # chunks per gather call (1024 idx ucode cap)
SBATCH = 8            # chunks per batched S-build DVE op

FEAT = 512
WG = [512, 1024, 512, 128, 128]   # gather/agg width per layer l = 0..4
# dense widths: d0: 512->2048(T out), d1: 2048->1024, d2: 1024->512,
#               d3: 512->128, d4: 128->128, d5: 128->4


# ---------------------------------------------------------------- tile patch
def apply_tile_patch():
    """This walrus build allows only 1 sync-wait per Drain; split the tail
    drain's waits across a chain of drains."""
    import bass_rust

    def _drain_and_barrier_split(self, tick_clock, wait_clock):
        from bass_rust import ScopedClock
        drain_inst = self.nc.sync.drain()
        wait_clock.add_sem_waits(
            drain_inst.ins, ScopedClock({None: tick_clock.global_clock})
        )
        si = drain_inst.ins.sync_info
        waits = list(si.on_wait) if si is not None else []
        if len(waits) > 1:
            si.on_wait = [waits[0]]
            for w in waits[1:]:
                extra = self.nc.sync.drain()
                if extra.ins.sync_info is None:
                    extra.ins.sync_info = bass_rust.SyncInfo(
                        on_wait=[w], on_update=[])
                else:
                    extra.ins.sync_info.on_wait = [w]
        self.nc.all_engine_barrier()
        popped = self.nc._tile_sem_poison_stack.pop()
        assert popped is self._sem_poison
        self.nc.clear_and_free_semaphores(list(self.sems.allocated().values()))
        self.nc.all_engine_barrier()

    tile.TileContext._drain_and_barrier = _drain_and_barrier_split


def _gpid(core, slot):
    """Chunk-major padded global id (vectorized over 3 AG chunks)."""
    out = np.empty_like(np.broadcast_arrays(core, slot)[1])
    core = np.asarray(core)
    slot = np.asarray(slot)
    for j in range(3):
        m = (slot >= CH[j]) & (slot < CH[j + 1])
        w = CH[j + 1] - CH[j]
        out[m] = GB[j] + core[m] * w + (slot[m] - CH[j])
    return out


# ------------------------------------------------------------- preprocess
def preprocess(x, edge_index, edge_attr, Ws, bs, Wp, bp):
    """Host-side: normalization, edge sharding/sorting/packing, weight packs.
    Returns (in_maps, meta)."""
    N = NCORE * PCN
    src = np.asarray(edge_index[0], dtype=np.int64)
    dst = np.asarray(edge_index[1], dtype=np.int64)
    ew = np.asarray(edge_attr, dtype=np.float32)
    loop = np.arange(N, dtype=np.int64)
    src2 = np.concatenate([src, loop])
    dst2 = np.concatenate([dst, loop])
    ew2 = np.concatenate([ew, np.ones(N, np.float32)])

    deg = np.bincount(dst2, weights=ew2.astype(np.float64), minlength=N)
    deg = deg.astype(np.float32)
    dinv = np.where(deg > 0, 1.0 / np.sqrt(deg), 0.0).astype(np.float32)
    normf = (dinv[src2] * ew2 * dinv[dst2]).astype(np.float32)  # full norm

    score = src2 // PCN
    sslot = src2 - score * PCN
    gpid = _gpid(score, sslot)

    core_of = dst2 // PCN
    slot = dst2 - core_of * PCN
    tile_of = slot // P
    slot_in = slot % P
    region_of = np.digitize(gpid, GB[1:3]).astype(np.int64)

    NR = 3
    counts = np.zeros((NCORE, NT, NR), np.int64)
    np.add.at(counts, (core_of, tile_of, region_of), 1)
    kmax = counts.max(axis=0)                      # [NT, NR]
    K = ((kmax + P - 1) // P) * P
    K[kmax == 0] = 0

    order = np.lexsort((region_of, tile_of, core_of))
    so_gpid = gpid[order]
    so_norm = normf[order]
    so_slot = slot_in[order]
    so_core = core_of[order]
    so_tile = tile_of[order]
    so_reg = region_of[order]

    icol = np.zeros((NT, NR), np.int64)
    cbase = np.zeros((NT, NR), np.int64)
    ic = cc = 0
    for t in range(NT):
        for r in range(NR):
            icol[t, r] = ic
            cbase[t, r] = cc
            ic += K[t, r] // 16
            cc += K[t, r] // P
    idxcols, nch = ic, cc

    # full padded X in gpid layout (shared by all cores)
    x_np = np.asarray(x, dtype=np.float32)
    xf = np.zeros((NPT, FEAT), np.float16)
    nodes = np.arange(N, dtype=np.int64)
    xf[_gpid(nodes // PCN, nodes % PCN)] = x_np.astype(np.float16)

    # weight packs
    w_list = [np.asarray(w, np.float32) for w in Ws] + [np.asarray(Wp, np.float32)]
    b_list = [np.asarray(b, np.float32) for b in bs] + [np.asarray(bp, np.float32)]
    # W1 [512,2048] as lhsT blocks (kb in 4, mb in 16): col (kb*16+mb)*128
    W1 = w_list[0]
    w1p = np.zeros((P, 4 * 16 * P), np.float16)
    for kb in range(4):
        for mb in range(16):
            w1p[:, (kb * 16 + mb) * P:(kb * 16 + mb + 1) * P] = (
                W1[kb * P:(kb + 1) * P, mb * P:(mb + 1) * P].astype(np.float16))
    # W2..W5, Wp as rhs blocks (kb-major): [128, nk*Md]
    def rhs_pack(Wr, Kd, Md):
        wp_ = np.zeros((Kd, Md), np.float32)
        wp_[: Wr.shape[0], : Wr.shape[1]] = Wr
        nk = Kd // P
        out = np.zeros((P, nk * Md), np.float16)
        for kb in range(nk):
            out[:, kb * Md:(kb + 1) * Md] = wp_[kb * P:(kb + 1) * P].astype(np.float16)
        return out

    w2p = rhs_pack(w_list[1], 2048, 1024)
    w3p = rhs_pack(w_list[2], 1024, 512)
    w4p = rhs_pack(w_list[3], 512, 128)
    w5p = rhs_pack(w_list[4], 128, 128)
    wpp = rhs_pack(w_list[5], 128, 4)

    # biases as per-partition columns [128, nblk]
    def bias_cols(b, width):
        bb = np.zeros(width, np.float32)
        bb[: b.shape[0]] = b
        return bb.reshape(width // P, P).T.astype(np.float16).copy()

    b1c = bias_cols(b_list[0], 2048)   # d0 epilogue (relu)
    b2c = bias_cols(b_list[1], 1024)   # agg1 epilogue
    b3c = bias_cols(b_list[2], 512)    # agg2
    b4c = bias_cols(b_list[3], 128)    # agg3
    b5c = bias_cols(b_list[4], 128)    # agg4
    bpr = np.zeros((1, 4), np.float16)
    bpr[0, :3] = b_list[5].astype(np.float16)

    iota = np.tile(np.arange(P, dtype=np.float16), (P, 1))
    ident = np.eye(P, dtype=np.float16)
    ones1 = np.ones((1, P), np.float16)

    core_starts = np.searchsorted(so_core, np.arange(NCORE + 1))
    in_maps = []
    for c in range(NCORE):
        lo, hi = core_starts[c], core_starts[c + 1]
        ct, cr = so_tile[lo:hi], so_reg[lo:hi]
        cg, cn, cs = so_gpid[lo:hi], so_norm[lo:hi], so_slot[lo:hi]
        idx16 = np.zeros((16, idxcols), np.int16)
        slotp = np.zeros((P, nch), np.float16)
        normp = np.zeros((P, nch), np.float16)
        pos = 0
        for t in range(NT):
            for r in range(NR):
                k = int(K[t, r])
                if k == 0:
                    continue
                n_e = int(counts[c, t, r])
                seg = slice(pos, pos + n_e)
                assert np.all(ct[seg] == t) and np.all(cr[seg] == r), (c, t, r)
                reg_lo = GB[r]
                arr = np.zeros(k, np.int64)
                arr[:n_e] = cg[seg] - reg_lo
                assert 0 <= arr.min(initial=0) and arr.max(initial=0) < 32768
                idx16[:, int(icol[t, r]): int(icol[t, r]) + k // 16] = (
                    arr.reshape(k // 16, 16).T.astype(np.int16))
                sl = np.zeros(k, np.float32)
                sl[:n_e] = cs[seg]
                nm = np.zeros(k, np.float32)
                nm[:n_e] = cn[seg]
                cb = int(cbase[t, r])
                slotp[:, cb: cb + k // P] = (
                    sl.reshape(k // P, P).T.astype(np.float16))
                normp[:, cb: cb + k // P] = (
                    nm.reshape(k // P, P).T.astype(np.float16))
                pos += n_e
        assert pos == hi - lo

        # full S (one-hot x norm), for layers that load S instead of
        # building it on DVE
        oh = (slotp[:, :, None] == np.arange(P, dtype=np.float16)[None, None, :])
        sfull = (oh * normp[:, :, None]).astype(np.float16).reshape(P, nch * P)
        m = {
            "xf": xf,
            "idx16": np.tile(idx16, (8, 1)),
            "slotp": slotp,
            "normp": normp,
            "sfull": sfull,
            "iota": iota,
            "ident": ident,
            "ones1": ones1,
            "w1": w1p, "w2": w2p, "w3": w3p, "w4": w4p, "w5": w5p, "wp": wpp,
            "b1": b1c, "b2": b2c, "b3": b3c, "b4": b4c, "b5": b5c, "bp": bpr,
        }
        in_maps.append(m)

    meta = dict(K=K, icol=icol, cbase=cbase, idxcols=idxcols, nch=nch)
    return in_maps, meta


# ---------------------------------------------------------------- program
def _bc3(ap, ncols, inner=P, mode="col"):
    base = ap.ap
    if mode == "col":
        return bass.AP(ap.tensor, ap.offset, [base[0], [1, ncols], [0, inner]])
    else:
        return bass.AP(ap.tensor, ap.offset, [base[0], [0, ncols], base[1]])


def _3d(ap, ncols, inner=P):
    return bass.AP(ap.tensor, ap.offset, [ap.ap[0], [inner, ncols], [1, inner]])


def build_program(meta):
    import concourse.bacc as bacc
    nc = bacc.Bacc("TRN2", num_swdge_queues=4)
    K, icol, cbase = meta["K"], meta["icol"], meta["cbase"]
    idxcols, nch = meta["idxcols"], meta["nch"]
    rg = [list(range(NCORE))]

    # ---------------- params
    pr = {}
    def par(name, shape, dt):
        pr[name] = nc.declare_dram_parameter(name, shape, dt, isOutput=False)
    par("xf", [NPT, FEAT], FP16)
    par("idx16", [P, idxcols], I16)
    par("slotp", [P, nch], FP16)
    par("normp", [P, nch], FP16)
    par("sfull", [P, nch * P], FP16)
    par("iota", [P, P], FP16)
    par("ident", [P, P], FP16)
    par("ones1", [1, P], FP16)
    par("w1", [P, 4 * 16 * P], FP16)
    par("w2", [P, 16 * 1024], FP16)
    par("w3", [P, 8 * 512], FP16)
    par("w4", [P, 4 * 128], FP16)
    par("w5", [P, 128], FP16)
    par("wp", [P, 4], FP16)
    par("b1", [P, 16], FP16)
    par("b2", [P, 8], FP16)
    par("b3", [P, 4], FP16)
    par("b4", [P, 1], FP16)
    par("b5", [P, 1], FP16)
    par("bp", [1, 4], FP16)
    out_ext = nc.declare_dram_parameter("out", [PCN, 3], F32, isOutput=True)

    # ---------------- internal DRAM
    # layers l=1..4 aggregate T_{l+1}; width WG[l]
    town = {}   # (l, chunk j) -> per-core town tensor
    for l in range(1, 5):
        for j in range(3):
            town[l, j] = nc.dram_tensor(
                f"tn{l}_{j}", [CH[j + 1] - CH[j], WG[l]], FP16)
    TF = {}
    for l in range(1, 5):
        for j in range(3):
            TF[l, j] = nc.dram_tensor(
                f"tf{l}_{j}", [(GB[j + 1] - GB[j]), WG[l]], FP16,
                addr_space="Shared")
    PART = {(l, p): nc.dram_tensor(f"part{l}_{p}", [NT * P, WG[l]], FP16)
            for l in range(1, 5) for p in range(2)}

    with tile.TileContext(nc) as tc:
        import contextlib
        with contextlib.ExitStack() as ctx:
            cpool = ctx.enter_context(tc.tile_pool(name="const", bufs=1))
            msgp = ctx.enter_context(tc.tile_pool(name="msg", bufs=4))
            spool = ctx.enter_context(tc.tile_pool(name="sb", bufs=3))
            pp = ctx.enter_context(tc.tile_pool(name="ps", bufs=2, space="PSUM"))
            hp = ctx.enter_context(tc.tile_pool(name="hh", bufs=2))

            # ---- resident constants
            def cload(name, shape, dt):
                t_ = cpool.tile(shape, dt, tag=name, name=name)
                nc.sync.dma_start(out=t_[:], in_=pr[name][:])
                return t_
            idx_sb = cload("idx16", [P, idxcols], I16)
            slot_sb = cload("slotp", [P, nch], FP16)
            norm_sb = cload("normp", [P, nch], FP16)
            iota_sb = cload("iota", [P, P], FP16)
            ident_sb = cload("ident", [P, P], FP16)
            ones_sb = cload("ones1", [1, P], FP16)
            w1_sb = cload("w1", [P, 4 * 16 * P], FP16)
            w2_sb = cload("w2", [P, 16 * 1024], FP16)
            w3_sb = cload("w3", [P, 8 * 512], FP16)
            w4_sb = cload("w4", [P, 4 * 128], FP16)
            w5_sb = cload("w5", [P, 128], FP16)
            wp_sb = cload("wp", [P, 4], FP16)
            b1_sb = cload("b1", [P, 16], FP16)
            b2_sb = cload("b2", [P, 8], FP16)
            b3_sb = cload("b3", [P, 4], FP16)
            b4_sb = cload("b4", [P, 1], FP16)
            b5_sb = cload("b5", [P, 1], FP16)
            bp_sb = cload("bp", [1, 4], FP16)

            qn = [0]

            def build_s(t, r_list, load=False):
                """One S tile covering the chunks of (tile t, regions r_list)
                (contiguous in cbase layout). load=True DMAs the precomputed
                S from DRAM instead of building it on DVE."""
                c_lo = int(cbase[t, r_list[0]])
                ctn = sum(int(K[t, r]) // P for r in r_list)
                s_t = spool.tile([P, max(ctn, 1) * P], FP16, tag="s", name="s_t")
                if load:
                    if ctn > 0:
                        nc.sync.dma_start(
                            out=s_t[:, : ctn * P],
                            in_=pr["sfull"][:, c_lo * P:(c_lo + ctn) * P])
                    return s_t, ctn
                for b0 in range(0, ctn, SBATCH):
                    nb = min(SBATCH, ctn - b0)
                    cb0 = c_lo + b0
                    o3 = _3d(s_t[:, b0 * P:(b0 + nb) * P], nb)
                    nc.vector.tensor_tensor(
                        out=o3,
                        in0=_bc3(slot_sb[:, cb0:cb0 + nb], nb, mode="col"),
                        in1=_bc3(iota_sb[:], nb, mode="mat"),
                        op=mybir.AluOpType.is_equal)
                    nc.vector.tensor_tensor(
                        out=o3, in0=o3,
                        in1=_bc3(norm_sb[:, cb0:cb0 + nb], nb, mode="col"),
                        op=mybir.AluOpType.mult)
                return s_t, ctn

            def gather_mm(t, r, src_dram, W, s_t, s_coff, ps, mmcnt, mmtot,
                          use_start=True):
                """Gathers + aggregation matmuls for (tile t, region r).
                s_coff: chunk offset of this region within s_t.
                mmcnt: chunks already accumulated into ps; mmtot: total
                expected (stop flags on the last). use_start=False when the
                psum was already initialized (identity re-add). Returns new
                mmcnt."""
                k = int(K[t, r])
                if k == 0:
                    return mmcnt
                nf = W // P
                gmax = GMAX if W <= 512 else (4096 // W)  # cap msg at 8KB/part
                pos = 0
                while pos < k:
                    ks = min(gmax * P, k - pos)
                    ic = int(icol[t, r]) + pos // 16
                    msg = msgp.tile([P, 4096], FP16, tag="msg", name="msg")
                    nc.gpsimd.dma_gather(
                        out_ap=_3d(msg[:, : (ks // P) * W], ks // P, inner=W),
                        in_ap=src_dram,
                        idxs_ap=idx_sb[:, ic: ic + ks // 16],
                        num_idxs=ks,
                        num_idxs_reg=ks,
                        elem_size=W,
                        elem_step=W,
                        queue_num=qn[0],
                    )
                    qn[0] = (qn[0] + 1) % 4
                    for ci in range(ks // P):
                        cglob = s_coff + pos // P + ci
                        for fb in range(nf):
                            nc.tensor.matmul(
                                out=ps[:, fb * P:(fb + 1) * P],
                                lhsT=msg[:, ci * W + fb * P: ci * W + (fb + 1) * P],
                                rhs=s_t[:, cglob * P:(cglob + 1) * P],
                                start=(use_start and mmcnt == 0),
                                stop=(mmcnt == mmtot - 1))
                        mmcnt += 1
                    pos += ks
                return mmcnt

            def store_town(l, t, t_sb, W):
                j = 0 if t < 16 else (1 if t < 32 else 2)
                r0 = t * P - CH[j]
                nc.sync.dma_start(
                    out=town[l, j][r0:r0 + P, :], in_=t_sb[:, :W])

            def emit_ag(l, j):
                nc.gpsimd.collective_compute(
                    "AllGather", mybir.AluOpType.bypass, replica_groups=rg,
                    ins=[town[l, j][:]], outs=[TF[l, j][:]])

            # ================= phase 1: agg0(X) + d0 + d1 -> T2 =================
            def phase1_tile(t):
                s_t, ctn = build_s(t, [0, 1, 2])
                ps = pp.tile([P, 1024], F32, tag="agg", name="ps_agg")
                mm = 0
                coff = 0
                for r in range(3):
                    mm = gather_mm(t, r, pr["xf"][GB[r]:GB[r + 1], :], 512,
                                   s_t, coff, ps, mm, ctn)
                    coff += int(K[t, r]) // P
                assert mm == ctn and ctn > 0
                g1t = hp.tile([P, 512], FP16, tag="g1t", name="g1t")
                nc.scalar.activation(
                    out=g1t[:], in_=ps[:, :512],
                    func=mybir.ActivationFunctionType.Copy)
                # d0: H1^T = relu(W1^T-blocks @ G1^T + b1), 4 quarters
                h1t = hp.tile([P, 2048], FP16, tag="h1t", name="h1t")
                for q in range(4):
                    ps0 = pp.tile([P, 512], F32, tag="d0", name="ps_d0")
                    for mi in range(4):
                        mb = q * 4 + mi
                        for kb in range(4):
                            nc.tensor.matmul(
                                out=ps0[:, mi * P:(mi + 1) * P],
                                lhsT=w1_sb[:, (kb * 16 + mb) * P:(kb * 16 + mb + 1) * P],
                                rhs=g1t[:, kb * P:(kb + 1) * P],
                                start=(kb == 0), stop=(kb == 3))
                    for mi in range(4):
                        mb = q * 4 + mi
                        nc.scalar.activation(
                            out=h1t[:, mb * P:(mb + 1) * P],
                            in_=ps0[:, mi * P:(mi + 1) * P],
                            func=mybir.ActivationFunctionType.Relu,
                            bias=b1_sb[:, mb:mb + 1])
                # d1: T2 = H1 @ W2 (normal orientation), 2 halves of 512
                t2sb = hp.tile([P, 1024], FP16, tag="tout", name="t2sb")
                for h in range(2):
                    psd = pp.tile([P, 512], F32, tag="d", name="ps_d")
                    for kb in range(16):
                        nc.tensor.matmul(
                            out=psd[:],
                            lhsT=h1t[:, kb * P:(kb + 1) * P],
                            rhs=w2_sb[:, kb * 1024 + h * 512: kb * 1024 + h * 512 + 512],
                            start=(kb == 0), stop=(kb == 15))
                    nc.scalar.activation(
                        out=t2sb[:, h * 512:(h + 1) * 512], in_=psd[:],
                        func=mybir.ActivationFunctionType.Copy)
                store_town(1, t, t2sb, 1024)
                if t == 15:
                    emit_ag(1, 0)
                elif t == 31:
                    emit_ag(1, 1)

            # ========= layers l=1..4: agg_l (3 passes, 1 per region) + dense ====
            # agg_l consumes TF[l,*] (width WG[l]), produces H^{l+1,T}; dense
            # d_{l+1} produces T_{l+2} (towns l+1) or the final output. Passes
            # 0/1 stage the PSUM to DRAM as fp16 partials; passes 1/2 re-add
            # them via an identity matmul.
            def agg_tile(l, p, t):
                W = WG[l]
                nf = W // P
                bias_sb = {1: b2_sb, 2: b3_sb, 3: b4_sb, 4: b5_sb}[l]
                last = p == 2
                if True:
                    if True:
                        s_t, ctn = build_s(t, [p], load=(l >= 3))
                        ps = pp.tile([P, 1024], F32, tag="agg", name="ps_agg")
                        if p > 0:
                            pb = hp.tile([P, 1024], FP16, tag="pb", name="pb")
                            nc.sync.dma_start(
                                out=pb[:, :W],
                                in_=PART[l, p - 1][t * P:(t + 1) * P, :])
                            nid = (W + 511) // 512
                            for j in range(nid):
                                w_ = min(512, W - j * 512)
                                nc.tensor.matmul(
                                    out=ps[:, j * 512: j * 512 + w_],
                                    lhsT=ident_sb[:],
                                    rhs=pb[:, j * 512: j * 512 + w_],
                                    start=True,
                                    stop=(ctn == 0 and j == nid - 1))
                            if ctn > 0:
                                gather_mm(t, p, TF[l, p][:], W, s_t, 0, ps,
                                          0, ctn, use_start=False)
                        else:
                            gather_mm(t, 0, TF[l, 0][:], W, s_t, 0, ps, 0,
                                      max(ctn, 1))
                            if ctn == 0:
                                nc.vector.memset(ps[:, :W], 0.0)
                        if not last:
                            pa = hp.tile([P, 1024], FP16, tag="pa", name="pa")
                            nc.scalar.activation(
                                out=pa[:, :W], in_=ps[:, :W],
                                func=mybir.ActivationFunctionType.Copy)
                            nc.sync.dma_start(
                                out=PART[l, p][t * P:(t + 1) * P, :],
                                in_=pa[:, :W])
                            return
                        hT = hp.tile([P, 1024], FP16, tag="ht", name="hT")
                        for fb in range(nf):
                            nc.scalar.activation(
                                out=hT[:, fb * P:(fb + 1) * P],
                                in_=ps[:, fb * P:(fb + 1) * P],
                                func=mybir.ActivationFunctionType.Relu,
                                bias=bias_sb[:, fb:fb + 1])
                        # dense d_{l+1}
                        if l == 1:
                            # H2[1024] @ W3 -> T3 [512]
                            t3 = hp.tile([P, 512], FP16, tag="tout", name="t3")
                            psd = pp.tile([P, 512], F32, tag="d", name="ps_d")
                            for kb in range(8):
                                nc.tensor.matmul(
                                    out=psd[:],
                                    lhsT=hT[:, kb * P:(kb + 1) * P],
                                    rhs=w3_sb[:, kb * 512:(kb + 1) * 512],
                                    start=(kb == 0), stop=(kb == 7))
                            nc.scalar.activation(
                                out=t3[:], in_=psd[:],
                                func=mybir.ActivationFunctionType.Copy)
                            store_town(2, t, t3, 512)
                            if t == 15:
                                emit_ag(2, 0)
                            elif t == 31:
                                emit_ag(2, 1)
                        elif l == 2:
                            # H3[512] @ W4 -> T4 [128]
                            t4 = hp.tile([P, 128], FP16, tag="tout4", name="t4")
                            psd = pp.tile([P, 512], F32, tag="d", name="ps_d")
                            for kb in range(4):
                                nc.tensor.matmul(
                                    out=psd[:, :128],
                                    lhsT=hT[:, kb * P:(kb + 1) * P],
                                    rhs=w4_sb[:, kb * 128:(kb + 1) * 128],
                                    start=(kb == 0), stop=(kb == 3))
                            nc.scalar.activation(
                                out=t4[:], in_=psd[:, :128],
                                func=mybir.ActivationFunctionType.Copy)
                            store_town(3, t, t4, 128)
                            if t == 15:
                                emit_ag(3, 0)
                            elif t == 31:
                                emit_ag(3, 1)
                        elif l == 3:
                            # H4[128] @ W5 -> T5 [128]
                            t5 = hp.tile([P, 128], FP16, tag="tout4", name="t5")
                            psd = pp.tile([P, 512], F32, tag="d", name="ps_d")
                            nc.tensor.matmul(
                                out=psd[:, :128], lhsT=hT[:, :128],
                                rhs=w5_sb[:], start=True, stop=True)
                            nc.scalar.activation(
                                out=t5[:], in_=psd[:, :128],
                                func=mybir.ActivationFunctionType.Copy)
                            store_town(4, t, t5, 128)
                            if t == 15:
                                emit_ag(4, 0)
                            elif t == 31:
                                emit_ag(4, 1)
                        else:
                            # d5: out = H5 @ Wp + bp
                            psd = pp.tile([P, 512], F32, tag="d", name="ps_d")
                            nc.tensor.matmul(
                                out=psd[:, :4], lhsT=hT[:, :128], rhs=wp_sb[:],
                                start=True, stop=False)
                            nc.tensor.matmul(
                                out=psd[:, :4], lhsT=ones_sb[0:1, :],
                                rhs=bp_sb[0:1, :], start=False, stop=True)
                            osb = hp.tile([P, 4], F32, tag="fout", name="osb")
                            nc.vector.tensor_copy(out=osb[:], in_=psd[:, :4])
                            r0 = t * P
                            r1 = min((t + 1) * P, PCN)
                            if r1 > r0:
                                nc.sync.dma_start(
                                    out=out_ext[r0:r1, :],
                                    in_=osb[: r1 - r0, :3])

            # ---------------- emission driver (software pipelining) ----------
            # phase 1 tiles 0..33, then interleave its tail with agg1-pass0
            # (whose gathers wait on the first T2 AllGather chunk).
            for t in range(34):
                phase1_tile(t)
            j = 0
            for t in range(34, NT):
                phase1_tile(t)
                while j < 2 * (t - 33) and j < NT:
                    agg_tile(1, 0, j)
                    j += 1
            emit_ag(1, 2)
            while j < NT:
                agg_tile(1, 0, j)
                j += 1
            for p in (1, 2):
                for t in range(NT):
                    agg_tile(1, p, t)
            emit_ag(2, 2)
            for l in range(2, 5):
                for p in range(3):
                    for t in range(NT):
                        agg_tile(l, p, t)
                if l < 4:
                    emit_ag(l + 1, 2)

    nc.finalize()
    return nc


# ------------------------------------------------------------------ driver
_CACHE = {}


def kernel(x, edge_index, edge_attr, W1, b1, W2, b2, W3, b3, W4, b4, W5, b5,
           Wp, bp):
    apply_tile_patch()
    import os
    from concourse.bass_utils import run_bass_kernel_spmd

    Ws = [W1, W2, W3, W4, W5]
    bs = [b1, b2, b3, b4, b5]
    in_maps, meta = preprocess(x, edge_index, edge_attr, Ws, bs, Wp, bp)

    key = (meta["K"].tobytes(), meta["nch"], meta["idxcols"])
    nc = _CACHE.get(key)
    if nc is None:
        nc = build_program(meta)
        _CACHE[key] = nc

    res = run_bass_kernel_spmd(
        nc, in_maps, core_ids=list(range(NCORE)),
        trace=bool(int(os.environ.get("TRACE", "0"))))
    if res.exec_time_ns:
        print(f"HW exec time: {res.exec_time_ns} ns")
    out = np.concatenate(
        [res.results[c]["out"] for c in range(NCORE)], axis=0)
    return np.ascontiguousarray(out.astype(np.float32))


# revision 31
# speedup vs baseline: 2.0261x; 1.0060x over previous
"""GCN (5-layer ColorGNN) Bass kernel for 8 TRN2 NeuronCores — v2.

Pipelined design (node-sharded SPMD, 6272 padded nodes/core = 49 tiles):

  - Full padded X (fp16, gpid layout) is staged on EVERY core: the layer-1
    aggregation gathers it directly -> no AllGather for layer 1.
  - Aggregation outputs are produced TRANSPOSED (H^T tiles: features on
    partitions, 128 dst nodes on free): out^T[f,n] = sum_e msg[e,f]*S[e,n]
    with lhsT=msg (gathered rows), rhs=S (one-hot x norm). The full
    symmetric norm dinv[src]*ew*dinv[dst] is folded into S on the host, and
    the bias is a per-partition ACT bias in the relu epilogue.
  - H^T tiles feed the next dense matmul as lhsT straight from SBUF: no
    transposes, no H round-trips through DRAM. Dense outputs T (node-major)
    go to DRAM only as AllGather input. Layer-1's dense pair runs in the
    transposed orientation (lhsT = W1 blocks) so the chain stays in H^T.
  - Each T AllGather is split in 2 node-chunks (4096+2176 rows/core,
    matching gpid regions [0,32768) and [32768,50176) which also serve as
    the int16 gather-index regions). Chunk0 fires when dense tiles 0..31
    are stored; the next layer's aggregation runs in 2 passes (region A
    then region B) so its region-A gathers need only chunk0 -> chunk1's
    transfer hides under the region-A gather. Pass A's PSUM is staged to
    DRAM as an fp16 partial and re-added in pass B via an identity matmul.
  - Host preprocessing: edges bucketed per (core=dst core, dst tile, src
    region), padded to 128-multiples of the cross-core max so all 8 cores
    run one SPMD program.
"""

import numpy as np
import concourse.bass as bass
import concourse.mybir as mybir
import concourse.tile as tile

FP16 = mybir.dt.float16
F32 = mybir.dt.float32
I16 = mybir.dt.int16

P = 128
NCORE = 8
PCN = 6250            # real nodes per core
NPC = 6272            # padded nodes per core (49 tiles)
NT = 49
# AllGather chunks per core (tile-aligned): tiles 0-15 / 16-31 / 32-48
CH = [0, 2048, 4096, 6272]       # per-core row boundaries
GB = [0, 16384, 32768, 50176]    # global gpid base of each chunk
REG0 = 32768          # gpid boundary of gather region 0 (chunks 0a+0b)
NPT = NCORE * NPC     # 50176 padded total nodes
T_SPLIT = 32          # first tile of gather-region 1
GMAX = 8              # chunks per gather call (1024 idx ucode cap)
SBATCH = 8            # chunks per batched S-build DVE op

FEAT = 512
WG = [512, 1024, 512, 128, 128]   # gather/agg width per layer l = 0..4
# dense widths: d0: 512->2048(T out), d1: 2048->1024, d2: 1024->512,
#               d3: 512->128, d4: 128->128, d5: 128->4


# ---------------------------------------------------------------- tile patch
def apply_tile_patch():
    """This walrus build allows only 1 sync-wait per Drain; split the tail
    drain's waits across a chain of drains."""
    import bass_rust

    def _drain_and_barrier_split(self, tick_clock, wait_clock):
        from bass_rust import ScopedClock
        drain_inst = self.nc.sync.drain()
        wait_clock.add_sem_waits(
            drain_inst.ins, ScopedClock({None: tick_clock.global_clock})
        )
        si = drain_inst.ins.sync_info
        waits = list(si.on_wait) if si is not None else []
        if len(waits) > 1:
            si.on_wait = [waits[0]]
            for w in waits[1:]:
                extra = self.nc.sync.drain()
                if extra.ins.sync_info is None:
                    extra.ins.sync_info = bass_rust.SyncInfo(
                        on_wait=[w], on_update=[])
                else:
                    extra.ins.sync_info.on_wait = [w]
        self.nc.all_engine_barrier()
        popped = self.nc._tile_sem_poison_stack.pop()
        assert popped is self._sem_poison
        self.nc.clear_and_free_semaphores(list(self.sems.allocated().values()))
        self.nc.all_engine_barrier()

    tile.TileContext._drain_and_barrier = _drain_and_barrier_split


def _gpid(core, slot):
    """Chunk-major padded global id (vectorized over 3 AG chunks)."""
    out = np.empty_like(np.broadcast_arrays(core, slot)[1])
    core = np.asarray(core)
    slot = np.asarray(slot)
    for j in range(3):
        m = (slot >= CH[j]) & (slot < CH[j + 1])
        w = CH[j + 1] - CH[j]
        out[m] = GB[j] + core[m] * w + (slot[m] - CH[j])
    return out


# ------------------------------------------------------------- preprocess
def preprocess(x, edge_index, edge_attr, Ws, bs, Wp, bp):
    """Host-side: normalization, edge sharding/sorting/packing, weight packs.
    Returns (in_maps, meta)."""
    N = NCORE * PCN
    src = np.asarray(edge_index[0], dtype=np.int64)
    dst = np.asarray(edge_index[1], dtype=np.int64)
    ew = np.asarray(edge_attr, dtype=np.float32)
    loop = np.arange(N, dtype=np.int64)
    src2 = np.concatenate([src, loop])
    dst2 = np.concatenate([dst, loop])
    ew2 = np.concatenate([ew, np.ones(N, np.float32)])

    deg = np.bincount(dst2, weights=ew2.astype(np.float64), minlength=N)
    deg = deg.astype(np.float32)
    dinv = np.where(deg > 0, 1.0 / np.sqrt(deg), 0.0).astype(np.float32)
    normf = (dinv[src2] * ew2 * dinv[dst2]).astype(np.float32)  # full norm

    score = src2 // PCN
    sslot = src2 - score * PCN
    gpid = _gpid(score, sslot)

    core_of = dst2 // PCN
    slot = dst2 - core_of * PCN
    tile_of = slot // P
    slot_in = slot % P
    region_of = np.digitize(gpid, GB[1:3]).astype(np.int64)

    NR = 3
    counts = np.zeros((NCORE, NT, NR), np.int64)
    np.add.at(counts, (core_of, tile_of, region_of), 1)
    kmax = counts.max(axis=0)                      # [NT, NR]
    K = ((kmax + P - 1) // P) * P
    K[kmax == 0] = 0

    order = np.lexsort((region_of, tile_of, core_of))
    so_gpid = gpid[order]
    so_norm = normf[order]
    so_slot = slot_in[order]
    so_core = core_of[order]
    so_tile = tile_of[order]
    so_reg = region_of[order]

    icol = np.zeros((NT, NR), np.int64)
    cbase = np.zeros((NT, NR), np.int64)
    ic = cc = 0
    for t in range(NT):
        for r in range(NR):
            icol[t, r] = ic
            cbase[t, r] = cc
            ic += K[t, r] // 16
            cc += K[t, r] // P
    idxcols, nch = ic, cc

    # full padded X in gpid layout (shared by all cores)
    x_np = np.asarray(x, dtype=np.float32)
    xf = np.zeros((NPT, FEAT), np.float16)
    nodes = np.arange(N, dtype=np.int64)
    xf[_gpid(nodes // PCN, nodes % PCN)] = x_np.astype(np.float16)

    # weight packs
    w_list = [np.asarray(w, np.float32) for w in Ws] + [np.asarray(Wp, np.float32)]
    b_list = [np.asarray(b, np.float32) for b in bs] + [np.asarray(bp, np.float32)]
    # W1 [512,2048] as lhsT blocks (kb in 4, mb in 16): col (kb*16+mb)*128
    W1 = w_list[0]
    w1p = np.zeros((P, 4 * 16 * P), np.float16)
    for kb in range(4):
        for mb in range(16):
            w1p[:, (kb * 16 + mb) * P:(kb * 16 + mb + 1) * P] = (
                W1[kb * P:(kb + 1) * P, mb * P:(mb + 1) * P].astype(np.float16))
    # W2..W5, Wp as rhs blocks (kb-major): [128, nk*Md]
    def rhs_pack(Wr, Kd, Md):
        wp_ = np.zeros((Kd, Md), np.float32)
        wp_[: Wr.shape[0], : Wr.shape[1]] = Wr
        nk = Kd // P
        out = np.zeros((P, nk * Md), np.float16)
        for kb in range(nk):
            out[:, kb * Md:(kb + 1) * Md] = wp_[kb * P:(kb + 1) * P].astype(np.float16)
        return out

    w2p = rhs_pack(w_list[1], 2048, 1024)
    w3p = rhs_pack(w_list[2], 1024, 512)
    w4p = rhs_pack(w_list[3], 512, 128)
    w5p = rhs_pack(w_list[4], 128, 128)
    wpp = rhs_pack(w_list[5], 128, 4)

    # biases as per-partition columns [128, nblk]
    def bias_cols(b, width):
        bb = np.zeros(width, np.float32)
        bb[: b.shape[0]] = b
        return bb.reshape(width // P, P).T.astype(np.float16).copy()

    b1c = bias_cols(b_list[0], 2048)   # d0 epilogue (relu)
    b2c = bias_cols(b_list[1], 1024)   # agg1 epilogue
    b3c = bias_cols(b_list[2], 512)    # agg2
    b4c = bias_cols(b_list[3], 128)    # agg3
    b5c = bias_cols(b_list[4], 128)    # agg4
    bpr = np.zeros((1, 4), np.float16)
    bpr[0, :3] = b_list[5].astype(np.float16)

    iota = np.tile(np.arange(P, dtype=np.float16), (P, 1))
    ident = np.eye(P, dtype=np.float16)
    ones1 = np.ones((1, P), np.float16)

    core_starts = np.searchsorted(so_core, np.arange(NCORE + 1))
    in_maps = []
    for c in range(NCORE):
        lo, hi = core_starts[c], core_starts[c + 1]
        ct, cr = so_tile[lo:hi], so_reg[lo:hi]
        cg, cn, cs = so_gpid[lo:hi], so_norm[lo:hi], so_slot[lo:hi]
        idx16 = np.zeros((16, idxcols), np.int16)
        slotp = np.zeros((P, nch), np.float16)
        normp = np.zeros((P, nch), np.float16)
        pos = 0
        for t in range(NT):
            for r in range(NR):
                k = int(K[t, r])
                if k == 0:
                    continue
                n_e = int(counts[c, t, r])
                seg = slice(pos, pos + n_e)
                assert np.all(ct[seg] == t) and np.all(cr[seg] == r), (c, t, r)
                reg_lo = GB[r]
                arr = np.zeros(k, np.int64)
                arr[:n_e] = cg[seg] - reg_lo
                assert 0 <= arr.min(initial=0) and arr.max(initial=0) < 32768
                idx16[:, int(icol[t, r]): int(icol[t, r]) + k // 16] = (
                    arr.reshape(k // 16, 16).T.astype(np.int16))
                sl = np.zeros(k, np.float32)
                sl[:n_e] = cs[seg]
                nm = np.zeros(k, np.float32)
                nm[:n_e] = cn[seg]
                cb = int(cbase[t, r])
                slotp[:, cb: cb + k // P] = (
                    sl.reshape(k // P, P).T.astype(np.float16))
                normp[:, cb: cb + k // P] = (
                    nm.reshape(k // P, P).T.astype(np.float16))
                pos += n_e
        assert pos == hi - lo

        # full S (one-hot x norm), for layers that load S instead of
        # building it on DVE
        oh = (slotp[:, :, None] == np.arange(P, dtype=np.float16)[None, None, :])
        sfull = (oh * normp[:, :, None]).astype(np.float16).reshape(P, nch * P)
        m = {
            "xf": xf,
            "idx16": np.tile(idx16, (8, 1)),
            "slotp": slotp,
            "normp": normp,
            "sfull": sfull,
            "iota": iota,
            "ident": ident,
            "ones1": ones1,
            "w1": w1p, "w2": w2p, "w3": w3p, "w4": w4p, "w5": w5p, "wp": wpp,
            "b1": b1c, "b2": b2c, "b3": b3c, "b4": b4c, "b5": b5c, "bp": bpr,
        }
        in_maps.append(m)

    meta = dict(K=K, icol=icol, cbase=cbase, idxcols=idxcols, nch=nch)
    return in_maps, meta


# ---------------------------------------------------------------- program
def _bc3(ap, ncols, inner=P, mode="col"):
    base = ap.ap
    if mode == "col":
        return bass.AP(ap.tensor, ap.offset, [base[0], [1, ncols], [0, inner]])
    else:
        return bass.AP(ap.tensor, ap.offset, [base[0], [0, ncols], base[1]])


def _3d(ap, ncols, inner=P):
    return bass.AP(ap.tensor, ap.offset, [ap.ap[0], [inner, ncols], [1, inner]])


def build_program(meta):
    import concourse.bacc as bacc
    nc = bacc.Bacc("TRN2", num_swdge_queues=4)
    K, icol, cbase = meta["K"], meta["icol"], meta["cbase"]
    idxcols, nch = meta["idxcols"], meta["nch"]
    rg = [list(range(NCORE))]

    # ---------------- params
    pr = {}
    def par(name, shape, dt):
        pr[name] = nc.declare_dram_parameter(name, shape, dt, isOutput=False)
    par("xf", [NPT, FEAT], FP16)
    par("idx16", [P, idxcols], I16)
    par("slotp", [P, nch], FP16)
    par("normp", [P, nch], FP16)
    par("sfull", [P, nch * P], FP16)
    par("iota", [P, P], FP16)
    par("ident", [P, P], FP16)
    par("ones1", [1, P], FP16)
    par("w1", [P, 4 * 16 * P], FP16)
    par("w2", [P, 16 * 1024], FP16)
    par("w3", [P, 8 * 512], FP16)
    par("w4", [P, 4 * 128], FP16)
    par("w5", [P, 128], FP16)
    par("wp", [P, 4], FP16)
    par("b1", [P, 16], FP16)
    par("b2", [P, 8], FP16)
    par("b3", [P, 4], FP16)
    par("b4", [P, 1], FP16)
    par("b5", [P, 1], FP16)
    par("bp", [1, 4], FP16)
    out_ext = nc.declare_dram_parameter("out", [PCN, 3], F32, isOutput=True)
    import os
    DBG = bool(int(os.environ.get("KDBG", "0")))
    dbg = {}
    if DBG:
        for nm, shape in [("dbg_g1", [NT * P, 512]), ("dbg_h1", [NT * P, 2048]),
                          ("dbg_t2", [NPC, 1024]), ("dbg_h2", [NT * P, 1024]),
                          ("dbg_t3", [NPC, 512]), ("dbg_t4", [NPC, 128]),
                          ("dbg_t5", [NPC, 128])]:
            dbg[nm] = nc.declare_dram_parameter(nm, shape, FP16, isOutput=True)

    # ---------------- internal DRAM
    # layers l=1..4 aggregate T_{l+1}; width WG[l]
    town = {}   # (l, chunk j) -> per-core town tensor
    for l in range(1, 5):
        for j in range(3):
            town[l, j] = nc.dram_tensor(
                f"tn{l}_{j}", [CH[j + 1] - CH[j], WG[l]], FP16)
    TF = {}
    for l in range(1, 5):
        for j in range(3):
            TF[l, j] = nc.dram_tensor(
                f"tf{l}_{j}", [(GB[j + 1] - GB[j]), WG[l]], FP16,
                addr_space="Shared")
    PART = {(l, p): nc.dram_tensor(f"part{l}_{p}", [NT * P, WG[l]], FP16)
            for l in range(1, 5) for p in range(2)}

    with tile.TileContext(nc) as tc:
        import contextlib
        with contextlib.ExitStack() as ctx:
            cpool = ctx.enter_context(tc.tile_pool(name="const", bufs=1))
            msgp = ctx.enter_context(tc.tile_pool(name="msg", bufs=4))
            spool = ctx.enter_context(tc.tile_pool(name="sb", bufs=3))
            pp = ctx.enter_context(tc.tile_pool(name="ps", bufs=2, space="PSUM"))
            hp = ctx.enter_context(tc.tile_pool(name="hh", bufs=2))

            # ---- resident constants
            def cload(name, shape, dt):
                t_ = cpool.tile(shape, dt, tag=name, name=name)
                nc.sync.dma_start(out=t_[:], in_=pr[name][:])
                return t_
            idx_sb = cload("idx16", [P, idxcols], I16)
            slot_sb = cload("slotp", [P, nch], FP16)
            norm_sb = cload("normp", [P, nch], FP16)
            iota_sb = cload("iota", [P, P], FP16)
            ident_sb = cload("ident", [P, P], FP16)
            ones_sb = cload("ones1", [1, P], FP16)
            w1_sb = cload("w1", [P, 4 * 16 * P], FP16)
            w2_sb = cload("w2", [P, 16 * 1024], FP16)
            w3_sb = cload("w3", [P, 8 * 512], FP16)
            w4_sb = cload("w4", [P, 4 * 128], FP16)
            w5_sb = cload("w5", [P, 128], FP16)
            wp_sb = cload("wp", [P, 4], FP16)
            b1_sb = cload("b1", [P, 16], FP16)
            b2_sb = cload("b2", [P, 8], FP16)
            b3_sb = cload("b3", [P, 4], FP16)
            b4_sb = cload("b4", [P, 1], FP16)
            b5_sb = cload("b5", [P, 1], FP16)
            bp_sb = cload("bp", [1, 4], FP16)

            qn = [0]

            def build_s(t, r_list, load=False):
                """One S tile covering the chunks of (tile t, regions r_list)
                (contiguous in cbase layout). load=True DMAs the precomputed
                S from DRAM instead of building it on DVE."""
                c_lo = int(cbase[t, r_list[0]])
                ctn = sum(int(K[t, r]) // P for r in r_list)
                s_t = spool.tile([P, max(ctn, 1) * P], FP16, tag="s", name="s_t")
                if load:
                    if ctn > 0:
                        nc.sync.dma_start(
                            out=s_t[:, : ctn * P],
                            in_=pr["sfull"][:, c_lo * P:(c_lo + ctn) * P])
                    return s_t, ctn
                for b0 in range(0, ctn, SBATCH):
                    nb = min(SBATCH, ctn - b0)
                    cb0 = c_lo + b0
                    o3 = _3d(s_t[:, b0 * P:(b0 + nb) * P], nb)
                    nc.vector.tensor_tensor(
                        out=o3,
                        in0=_bc3(slot_sb[:, cb0:cb0 + nb], nb, mode="col"),
                        in1=_bc3(iota_sb[:], nb, mode="mat"),
                        op=mybir.AluOpType.is_equal)
                    nc.vector.tensor_tensor(
                        out=o3, in0=o3,
                        in1=_bc3(norm_sb[:, cb0:cb0 + nb], nb, mode="col"),
                        op=mybir.AluOpType.mult)
                return s_t, ctn

            def gather_mm(t, r, src_dram, W, s_t, s_coff, ps, mmcnt, mmtot,
                          use_start=True):
                """Gathers + aggregation matmuls for (tile t, region r).
                s_coff: chunk offset of this region within s_t.
                mmcnt: chunks already accumulated into ps; mmtot: total
                expected (stop flags on the last). use_start=False when the
                psum was already initialized (identity re-add). Returns new
                mmcnt."""
                k = int(K[t, r])
                if k == 0:
                    return mmcnt
                nf = W // P
                gmax = GMAX if W <= 512 else (4096 // W)  # cap msg at 8KB/part
                pos = 0
                while pos < k:
                    ks = min(gmax * P, k - pos)
                    ic = int(icol[t, r]) + pos // 16
                    msg = msgp.tile([P, 4096], FP16, tag="msg", name="msg")
                    nc.gpsimd.dma_gather(
                        out_ap=_3d(msg[:, : (ks // P) * W], ks // P, inner=W),
                        in_ap=src_dram,
                        idxs_ap=idx_sb[:, ic: ic + ks // 16],
                        num_idxs=ks,
                        num_idxs_reg=ks,
                        elem_size=W,
                        elem_step=W,
                        queue_num=qn[0],
                    )
                    qn[0] = (qn[0] + 1) % 4
                    for ci in range(ks // P):
                        cglob = s_coff + pos // P + ci
                        for fb in range(nf):
                            # start only on the first write to each 2KB PSUM
                            # zero region (512 f32 cols = 4 fb slices)
                            nc.tensor.matmul(
                                out=ps[:, fb * P:(fb + 1) * P],
                                lhsT=msg[:, ci * W + fb * P: ci * W + (fb + 1) * P],
                                rhs=s_t[:, cglob * P:(cglob + 1) * P],
                                start=(use_start and mmcnt == 0
                                       and fb % 4 == 0),
                                stop=(mmcnt == mmtot - 1))
                        mmcnt += 1
                    pos += ks
                return mmcnt

            def store_town(l, t, t_sb, W):
                j = 0 if t < 16 else (1 if t < 32 else 2)
                r0 = t * P - CH[j]
                nc.sync.dma_start(
                    out=town[l, j][r0:r0 + P, :], in_=t_sb[:, :W])

            def emit_ag(l, j):
                nc.gpsimd.collective_compute(
                    "AllGather", mybir.AluOpType.bypass, replica_groups=rg,
                    ins=[town[l, j][:]], outs=[TF[l, j][:]])

            # ================= phase 1: agg0(X) + d0 + d1 -> T2 =================
            def phase1_tile(t):
                s_t, ctn = build_s(t, [0, 1, 2])
                ps = pp.tile([P, 1024], F32, tag="agg", name="ps_agg")
                mm = 0
                coff = 0
                for r in range(3):
                    mm = gather_mm(t, r, pr["xf"][GB[r]:GB[r + 1], :], 512,
                                   s_t, coff, ps, mm, ctn)
                    coff += int(K[t, r]) // P
                assert mm == ctn and ctn > 0
                g1t = hp.tile([P, 512], FP16, tag="g1t", name="g1t")
                nc.scalar.activation(
                    out=g1t[:], in_=ps[:, :512],
                    func=mybir.ActivationFunctionType.Copy)
                if DBG:
                    nc.sync.dma_start(out=dbg["dbg_g1"][t * P:(t + 1) * P, :],
                                      in_=g1t[:])
                # d0: H1^T = relu(W1^T-blocks @ G1^T + b1), 4 quarters
                h1t = hp.tile([P, 2048], FP16, tag="h1t", name="h1t")
                for q in range(4):
                    ps0 = pp.tile([P, 512], F32, tag="d0", name="ps_d0")
                    for mi in range(4):
                        mb = q * 4 + mi
                        for kb in range(4):
                            nc.tensor.matmul(
                                out=ps0[:, mi * P:(mi + 1) * P],
                                lhsT=w1_sb[:, (kb * 16 + mb) * P:(kb * 16 + mb + 1) * P],
                                rhs=g1t[:, kb * P:(kb + 1) * P],
                                start=(kb == 0 and mi == 0),
                                stop=(kb == 3))
                    for mi in range(4):
                        mb = q * 4 + mi
                        nc.scalar.activation(
                            out=h1t[:, mb * P:(mb + 1) * P],
                            in_=ps0[:, mi * P:(mi + 1) * P],
                            func=mybir.ActivationFunctionType.Relu,
                            bias=b1_sb[:, mb:mb + 1])
                # d1: T2 = H1 @ W2 (normal orientation), 2 halves of 512
                t2sb = hp.tile([P, 1024], FP16, tag="tout", name="t2sb")
                for h in range(2):
                    psd = pp.tile([P, 512], F32, tag="d", name="ps_d")
                    for kb in range(16):
                        nc.tensor.matmul(
                            out=psd[:],
                            lhsT=h1t[:, kb * P:(kb + 1) * P],
                            rhs=w2_sb[:, kb * 1024 + h * 512: kb * 1024 + h * 512 + 512],
                            start=(kb == 0), stop=(kb == 15))
                    nc.scalar.activation(
                        out=t2sb[:, h * 512:(h + 1) * 512], in_=psd[:],
                        func=mybir.ActivationFunctionType.Copy)
                if DBG:
                    nc.sync.dma_start(out=dbg["dbg_h1"][t * P:(t + 1) * P, :],
                                      in_=h1t[:])
                    nc.sync.dma_start(out=dbg["dbg_t2"][t * P:(t + 1) * P, :],
                                      in_=t2sb[:])
                store_town(1, t, t2sb, 1024)
                if t == 15:
                    emit_ag(1, 0)
                elif t == 31:
                    emit_ag(1, 1)

            # ========= layers l=1..4: agg_l (3 passes, 1 per region) + dense ====
            # agg_l consumes TF[l,*] (width WG[l]), produces H^{l+1,T}; dense
            # d_{l+1} produces T_{l+2} (towns l+1) or the final output. Passes
            # 0/1 stage the PSUM to DRAM as fp16 partials; passes 1/2 re-add
            # them via an identity matmul.
            def agg_tile(l, p, t):
                W = WG[l]
                nf = W // P
                bias_sb = {1: b2_sb, 2: b3_sb, 3: b4_sb, 4: b5_sb}[l]
                last = p == 2
                if True:
                    if True:
                        s_t, ctn = build_s(t, [p],
                                           load=False)
                        ps = pp.tile([P, 1024], F32, tag="agg", name="ps_agg")
                        if p > 0:
                            pb = hp.tile([P, 1024], FP16, tag="pb", name="pb")
                            nc.sync.dma_start(
                                out=pb[:, :W],
                                in_=PART[l, p - 1][t * P:(t + 1) * P, :])
                            nid = (W + 511) // 512
                            for j in range(nid):
                                w_ = min(512, W - j * 512)
                                nc.tensor.matmul(
                                    out=ps[:, j * 512: j * 512 + w_],
                                    lhsT=ident_sb[:],
                                    rhs=pb[:, j * 512: j * 512 + w_],
                                    start=True,
                                    stop=(ctn == 0 and j == nid - 1))
                            if ctn > 0:
                                gather_mm(t, p, TF[l, p][:], W, s_t, 0, ps,
                                          0, ctn, use_start=False)
                        else:
                            gather_mm(t, 0, TF[l, 0][:], W, s_t, 0, ps, 0,
                                      max(ctn, 1))
                            if ctn == 0:
                                nc.vector.memset(ps[:, :W], 0.0)
                        if not last:
                            pa = hp.tile([P, 1024], FP16, tag="pa", name="pa")
                            nc.scalar.activation(
                                out=pa[:, :W], in_=ps[:, :W],
                                func=mybir.ActivationFunctionType.Copy)
                            nc.sync.dma_start(
                                out=PART[l, p][t * P:(t + 1) * P, :],
                                in_=pa[:, :W])
                            return
                        hT = hp.tile([P, 1024], FP16, tag="ht", name="hT")
                        for fb in range(nf):
                            nc.scalar.activation(
                                out=hT[:, fb * P:(fb + 1) * P],
                                in_=ps[:, fb * P:(fb + 1) * P],
                                func=mybir.ActivationFunctionType.Relu,
                                bias=bias_sb[:, fb:fb + 1])
                        if DBG and l == 1:
                            nc.sync.dma_start(
                                out=dbg["dbg_h2"][t * P:(t + 1) * P, :],
                                in_=hT[:, :1024])
                        # dense d_{l+1}
                        if l == 1:
                            # H2[1024] @ W3 -> T3 [512]
                            t3 = hp.tile([P, 512], FP16, tag="tout", name="t3")
                            psd = pp.tile([P, 512], F32, tag="d", name="ps_d")
                            for kb in range(8):
                                nc.tensor.matmul(
                                    out=psd[:],
                                    lhsT=hT[:, kb * P:(kb + 1) * P],
                                    rhs=w3_sb[:, kb * 512:(kb + 1) * 512],
                                    start=(kb == 0), stop=(kb == 7))
                            nc.scalar.activation(
                                out=t3[:], in_=psd[:],
                                func=mybir.ActivationFunctionType.Copy)
                            if DBG:
                                nc.sync.dma_start(
                                    out=dbg["dbg_t3"][t * P:(t + 1) * P, :],
                                    in_=t3[:])
                            store_town(2, t, t3, 512)
                            if t == 15:
                                emit_ag(2, 0)
                            elif t == 31:
                                emit_ag(2, 1)
                        elif l == 2:
                            # H3[512] @ W4 -> T4 [128]
                            t4 = hp.tile([P, 128], FP16, tag="tout4", name="t4")
                            psd = pp.tile([P, 512], F32, tag="d", name="ps_d")
                            for kb in range(4):
                                nc.tensor.matmul(
                                    out=psd[:, :128],
                                    lhsT=hT[:, kb * P:(kb + 1) * P],
                                    rhs=w4_sb[:, kb * 128:(kb + 1) * 128],
                                    start=(kb == 0), stop=(kb == 3))
                            nc.scalar.activation(
                                out=t4[:], in_=psd[:, :128],
                                func=mybir.ActivationFunctionType.Copy)
                            if DBG:
                                nc.sync.dma_start(
                                    out=dbg["dbg_t4"][t * P:(t + 1) * P, :],
                                    in_=t4[:])
                            store_town(3, t, t4, 128)
                            if t == 15:
                                emit_ag(3, 0)
                            elif t == 31:
                                emit_ag(3, 1)
                        elif l == 3:
                            # H4[128] @ W5 -> T5 [128]
                            t5 = hp.tile([P, 128], FP16, tag="tout4", name="t5")
                            psd = pp.tile([P, 512], F32, tag="d", name="ps_d")
                            nc.tensor.matmul(
                                out=psd[:, :128], lhsT=hT[:, :128],
                                rhs=w5_sb[:], start=True, stop=True)
                            nc.scalar.activation(
                                out=t5[:], in_=psd[:, :128],
                                func=mybir.ActivationFunctionType.Copy)
                            if DBG:
                                nc.sync.dma_start(
                                    out=dbg["dbg_t5"][t * P:(t + 1) * P, :],
                                    in_=t5[:])
                            store_town(4, t, t5, 128)
                            if t == 15:
                                emit_ag(4, 0)
                            elif t == 31:
                                emit_ag(4, 1)
                        else:
                            # d5: out = H5 @ Wp + bp
                            psd = pp.tile([P, 512], F32, tag="d", name="ps_d")
                            nc.tensor.matmul(
                                out=psd[:, :4], lhsT=hT[:, :128], rhs=wp_sb[:],
                                start=True, stop=False)
                            nc.tensor.matmul(
                                out=psd[:, :4], lhsT=ones_sb[0:1, :],
                                rhs=bp_sb[0:1, :], start=False, stop=True)
                            osb = hp.tile([P, 4], F32, tag="fout", name="osb")
                            nc.vector.tensor_copy(out=osb[:], in_=psd[:, :4])
                            r0 = t * P
                            r1 = min((t + 1) * P, PCN)
                            if r1 > r0:
                                nc.sync.dma_start(
                                    out=out_ext[r0:r1, :],
                                    in_=osb[: r1 - r0, :3])

            # ---------------- emission driver (software pipelining) ----------
            # phase 1 tiles 0..33, then interleave its tail with agg1-pass0
            # (whose gathers wait on the first T2 AllGather chunk).
            for t in range(34):
                phase1_tile(t)
            j = 0
            for t in range(34, NT):
                phase1_tile(t)
                while j < 2 * (t - 33) and j < NT:
                    agg_tile(1, 0, j)
                    j += 1
            emit_ag(1, 2)
            while j < NT:
                agg_tile(1, 0, j)
                j += 1
            for p in (1, 2):
                for t in range(NT):
                    agg_tile(1, p, t)
            emit_ag(2, 2)
            for l in range(2, 5):
                for p in range(3):
                    for t in range(NT):
                        agg_tile(l, p, t)
                if l < 4:
                    emit_ag(l + 1, 2)

    nc.finalize()
    return nc


# ------------------------------------------------------------------ driver
_CACHE = {}


def kernel(x, edge_index, edge_attr, W1, b1, W2, b2, W3, b3, W4, b4, W5, b5,
           Wp, bp):
    apply_tile_patch()
    import os
    from concourse.bass_utils import run_bass_kernel_spmd

    Ws = [W1, W2, W3, W4, W5]
    bs = [b1, b2, b3, b4, b5]
    in_maps, meta = preprocess(x, edge_index, edge_attr, Ws, bs, Wp, bp)

    key = (meta["K"].tobytes(), meta["nch"], meta["idxcols"])
    nc = _CACHE.get(key)
    if nc is None:
        nc = build_program(meta)
        _CACHE[key] = nc

    res = run_bass_kernel_spmd(
        nc, in_maps, core_ids=list(range(NCORE)),
        trace=bool(int(os.environ.get("TRACE", "0"))))
    if res.exec_time_ns:
        print(f"HW exec time: {res.exec_time_ns} ns")
    out = np.concatenate(
        [res.results[c]["out"] for c in range(NCORE)], axis=0)
    return np.ascontiguousarray(out.astype(np.float32))


# revision 33
# speedup vs baseline: 2.0345x; 1.0041x over previous
"""GCN (5-layer ColorGNN) Bass kernel for 8 TRN2 NeuronCores — v2.

Pipelined design (node-sharded SPMD, 6272 padded nodes/core = 49 tiles):

  - Full padded X (fp16, gpid layout) is staged on EVERY core: the layer-1
    aggregation gathers it directly -> no AllGather for layer 1.
  - Aggregation outputs are produced TRANSPOSED (H^T tiles: features on
    partitions, 128 dst nodes on free): out^T[f,n] = sum_e msg[e,f]*S[e,n]
    with lhsT=msg (gathered rows), rhs=S (one-hot x norm). The full
    symmetric norm dinv[src]*ew*dinv[dst] is folded into S on the host, and
    the bias is a per-partition ACT bias in the relu epilogue.
  - H^T tiles feed the next dense matmul as lhsT straight from SBUF: no
    transposes, no H round-trips through DRAM. Dense outputs T (node-major)
    go to DRAM only as AllGather input. Layer-1's dense pair runs in the
    transposed orientation (lhsT = W1 blocks) so the chain stays in H^T.
  - Each T AllGather is split in 2 node-chunks (4096+2176 rows/core,
    matching gpid regions [0,32768) and [32768,50176) which also serve as
    the int16 gather-index regions). Chunk0 fires when dense tiles 0..31
    are stored; the next layer's aggregation runs in 2 passes (region A
    then region B) so its region-A gathers need only chunk0 -> chunk1's
    transfer hides under the region-A gather. Pass A's PSUM is staged to
    DRAM as an fp16 partial and re-added in pass B via an identity matmul.
  - Host preprocessing: edges bucketed per (core=dst core, dst tile, src
    region), padded to 128-multiples of the cross-core max so all 8 cores
    run one SPMD program.
"""

import numpy as np
import concourse.bass as bass
import concourse.mybir as mybir
import concourse.tile as tile

FP16 = mybir.dt.float16
F32 = mybir.dt.float32
I16 = mybir.dt.int16

P = 128
NCORE = 8
PCN = 6250            # real nodes per core
NPC = 6272            # padded nodes per core (49 tiles)
NT = 49
# AllGather chunks per core (tile-aligned): tiles 0-15 / 16-31 / 32-48
CH = [0, 2048, 4096, 6272]       # per-core row boundaries
GB = [0, 16384, 32768, 50176]    # global gpid base of each chunk
REG0 = 32768          # gpid boundary of gather region 0 (chunks 0a+0b)
NPT = NCORE * NPC     # 50176 padded total nodes
T_SPLIT = 32          # first tile of gather-region 1
GMAX = 8              # chunks per gather call (1024 idx ucode cap)
SBATCH = 8            # chunks per batched S-build DVE op

FEAT = 512
WG = [512, 1024, 512, 128, 128]   # gather/agg width per layer l = 0..4
# dense widths: d0: 512->2048(T out), d1: 2048->1024, d2: 1024->512,
#               d3: 512->128, d4: 128->128, d5: 128->4


# ---------------------------------------------------------------- tile patch
def apply_tile_patch():
    """This walrus build allows only 1 sync-wait per Drain; split the tail
    drain's waits across a chain of drains."""
    import bass_rust

    def _drain_and_barrier_split(self, tick_clock, wait_clock):
        from bass_rust import ScopedClock
        drain_inst = self.nc.sync.drain()
        wait_clock.add_sem_waits(
            drain_inst.ins, ScopedClock({None: tick_clock.global_clock})
        )
        si = drain_inst.ins.sync_info
        waits = list(si.on_wait) if si is not None else []
        if len(waits) > 1:
            si.on_wait = [waits[0]]
            for w in waits[1:]:
                extra = self.nc.sync.drain()
                if extra.ins.sync_info is None:
                    extra.ins.sync_info = bass_rust.SyncInfo(
                        on_wait=[w], on_update=[])
                else:
                    extra.ins.sync_info.on_wait = [w]
        self.nc.all_engine_barrier()
        popped = self.nc._tile_sem_poison_stack.pop()
        assert popped is self._sem_poison
        self.nc.clear_and_free_semaphores(list(self.sems.allocated().values()))
        self.nc.all_engine_barrier()

    tile.TileContext._drain_and_barrier = _drain_and_barrier_split


def _gpid(core, slot):
    """Chunk-major padded global id (vectorized over 3 AG chunks)."""
    out = np.empty_like(np.broadcast_arrays(core, slot)[1])
    core = np.asarray(core)
    slot = np.asarray(slot)
    for j in range(3):
        m = (slot >= CH[j]) & (slot < CH[j + 1])
        w = CH[j + 1] - CH[j]
        out[m] = GB[j] + core[m] * w + (slot[m] - CH[j])
    return out


# ------------------------------------------------------------- preprocess
def preprocess(x, edge_index, edge_attr, Ws, bs, Wp, bp):
    """Host-side: normalization, edge sharding/sorting/packing, weight packs.
    Returns (in_maps, meta)."""
    N = NCORE * PCN
    src = np.asarray(edge_index[0], dtype=np.int64)
    dst = np.asarray(edge_index[1], dtype=np.int64)
    ew = np.asarray(edge_attr, dtype=np.float32)
    loop = np.arange(N, dtype=np.int64)
    src2 = np.concatenate([src, loop])
    dst2 = np.concatenate([dst, loop])
    ew2 = np.concatenate([ew, np.ones(N, np.float32)])

    deg = np.bincount(dst2, weights=ew2.astype(np.float64), minlength=N)
    deg = deg.astype(np.float32)
    dinv = np.where(deg > 0, 1.0 / np.sqrt(deg), 0.0).astype(np.float32)
    normf = (dinv[src2] * ew2 * dinv[dst2]).astype(np.float32)  # full norm

    score = src2 // PCN
    sslot = src2 - score * PCN
    gpid = _gpid(score, sslot)

    core_of = dst2 // PCN
    slot = dst2 - core_of * PCN
    tile_of = slot // P
    slot_in = slot % P
    region_of = np.digitize(gpid, GB[1:3]).astype(np.int64)

    NR = 3
    counts = np.zeros((NCORE, NT, NR), np.int64)
    np.add.at(counts, (core_of, tile_of, region_of), 1)
    kmax = counts.max(axis=0)                      # [NT, NR]
    K = ((kmax + P - 1) // P) * P
    K[kmax == 0] = 0

    order = np.lexsort((region_of, tile_of, core_of))
    so_gpid = gpid[order]
    so_norm = normf[order]
    so_slot = slot_in[order]
    so_core = core_of[order]
    so_tile = tile_of[order]
    so_reg = region_of[order]

    icol = np.zeros((NT, NR), np.int64)
    cbase = np.zeros((NT, NR), np.int64)
    ic = cc = 0
    for t in range(NT):
        for r in range(NR):
            icol[t, r] = ic
            cbase[t, r] = cc
            ic += K[t, r] // 16
            cc += K[t, r] // P
    idxcols, nch = ic, cc

    # full padded X in gpid layout (shared by all cores)
    x_np = np.asarray(x, dtype=np.float32)
    xf = np.zeros((NPT, FEAT), np.float16)
    nodes = np.arange(N, dtype=np.int64)
    xf[_gpid(nodes // PCN, nodes % PCN)] = x_np.astype(np.float16)

    # weight packs
    w_list = [np.asarray(w, np.float32) for w in Ws] + [np.asarray(Wp, np.float32)]
    b_list = [np.asarray(b, np.float32) for b in bs] + [np.asarray(bp, np.float32)]
    # W1 [512,2048] as lhsT blocks (kb in 4, mb in 16): col (kb*16+mb)*128
    W1 = w_list[0]
    w1p = np.zeros((P, 4 * 16 * P), np.float16)
    for kb in range(4):
        for mb in range(16):
            w1p[:, (kb * 16 + mb) * P:(kb * 16 + mb + 1) * P] = (
                W1[kb * P:(kb + 1) * P, mb * P:(mb + 1) * P].astype(np.float16))
    # W2..W5, Wp as rhs blocks (kb-major): [128, nk*Md]
    def rhs_pack(Wr, Kd, Md):
        wp_ = np.zeros((Kd, Md), np.float32)
        wp_[: Wr.shape[0], : Wr.shape[1]] = Wr
        nk = Kd // P
        out = np.zeros((P, nk * Md), np.float16)
        for kb in range(nk):
            out[:, kb * Md:(kb + 1) * Md] = wp_[kb * P:(kb + 1) * P].astype(np.float16)
        return out

    w2p = rhs_pack(w_list[1], 2048, 1024)
    w3p = rhs_pack(w_list[2], 1024, 512)
    w4p = rhs_pack(w_list[3], 512, 128)
    w5p = rhs_pack(w_list[4], 128, 128)
    wpp = rhs_pack(w_list[5], 128, 4)

    # biases as per-partition columns [128, nblk]
    def bias_cols(b, width):
        bb = np.zeros(width, np.float32)
        bb[: b.shape[0]] = b
        return bb.reshape(width // P, P).T.astype(np.float16).copy()

    b1c = bias_cols(b_list[0], 2048)   # d0 epilogue (relu)
    b2c = bias_cols(b_list[1], 1024)   # agg1 epilogue
    b3c = bias_cols(b_list[2], 512)    # agg2
    b4c = bias_cols(b_list[3], 128)    # agg3
    b5c = bias_cols(b_list[4], 128)    # agg4
    bpr = np.zeros((1, 4), np.float16)
    bpr[0, :3] = b_list[5].astype(np.float16)

    iota = np.tile(np.arange(P, dtype=np.float16), (P, 1))
    ident = np.eye(P, dtype=np.float16)
    ones1 = np.ones((1, P), np.float16)

    core_starts = np.searchsorted(so_core, np.arange(NCORE + 1))
    in_maps = []
    for c in range(NCORE):
        lo, hi = core_starts[c], core_starts[c + 1]
        ct, cr = so_tile[lo:hi], so_reg[lo:hi]
        cg, cn, cs = so_gpid[lo:hi], so_norm[lo:hi], so_slot[lo:hi]
        idx16 = np.zeros((16, idxcols), np.int16)
        slotp = np.zeros((P, nch), np.float16)
        normp = np.zeros((P, nch), np.float16)
        pos = 0
        for t in range(NT):
            for r in range(NR):
                k = int(K[t, r])
                if k == 0:
                    continue
                n_e = int(counts[c, t, r])
                seg = slice(pos, pos + n_e)
                assert np.all(ct[seg] == t) and np.all(cr[seg] == r), (c, t, r)
                reg_lo = GB[r]
                arr = np.zeros(k, np.int64)
                arr[:n_e] = cg[seg] - reg_lo
                assert 0 <= arr.min(initial=0) and arr.max(initial=0) < 32768
                idx16[:, int(icol[t, r]): int(icol[t, r]) + k // 16] = (
                    arr.reshape(k // 16, 16).T.astype(np.int16))
                sl = np.zeros(k, np.float32)
                sl[:n_e] = cs[seg]
                nm = np.zeros(k, np.float32)
                nm[:n_e] = cn[seg]
                cb = int(cbase[t, r])
                slotp[:, cb: cb + k // P] = (
                    sl.reshape(k // P, P).T.astype(np.float16))
                normp[:, cb: cb + k // P] = (
                    nm.reshape(k // P, P).T.astype(np.float16))
                pos += n_e
        assert pos == hi - lo

        # full S (one-hot x norm), for layers that load S instead of
        # building it on DVE
        oh = (slotp[:, :, None] == np.arange(P, dtype=np.float16)[None, None, :])
        sfull = (oh * normp[:, :, None]).astype(np.float16).reshape(P, nch * P)
        m = {
            "xf": xf,
            "idx16": np.tile(idx16, (8, 1)),
            "slotp": slotp,
            "normp": normp,
            "sfull": sfull,
            "iota": iota,
            "ident": ident,
            "ones1": ones1,
            "w1": w1p, "w2": w2p, "w3": w3p, "w4": w4p, "w5": w5p, "wp": wpp,
            "b1": b1c, "b2": b2c, "b3": b3c, "b4": b4c, "b5": b5c, "bp": bpr,
        }
        in_maps.append(m)

    meta = dict(K=K, icol=icol, cbase=cbase, idxcols=idxcols, nch=nch)
    return in_maps, meta


# ---------------------------------------------------------------- program
def _bc3(ap, ncols, inner=P, mode="col"):
    base = ap.ap
    if mode == "col":
        return bass.AP(ap.tensor, ap.offset, [base[0], [1, ncols], [0, inner]])
    else:
        return bass.AP(ap.tensor, ap.offset, [base[0], [0, ncols], base[1]])


def _3d(ap, ncols, inner=P):
    return bass.AP(ap.tensor, ap.offset, [ap.ap[0], [inner, ncols], [1, inner]])


def build_program(meta):
    import concourse.bacc as bacc
    nc = bacc.Bacc("TRN2", num_swdge_queues=4)
    K, icol, cbase = meta["K"], meta["icol"], meta["cbase"]
    idxcols, nch = meta["idxcols"], meta["nch"]
    rg = [list(range(NCORE))]

    # ---------------- params
    pr = {}
    def par(name, shape, dt):
        pr[name] = nc.declare_dram_parameter(name, shape, dt, isOutput=False)
    par("xf", [NPT, FEAT], FP16)
    par("idx16", [P, idxcols], I16)
    par("slotp", [P, nch], FP16)
    par("normp", [P, nch], FP16)
    par("sfull", [P, nch * P], FP16)
    par("iota", [P, P], FP16)
    par("ident", [P, P], FP16)
    par("ones1", [1, P], FP16)
    par("w1", [P, 4 * 16 * P], FP16)
    par("w2", [P, 16 * 1024], FP16)
    par("w3", [P, 8 * 512], FP16)
    par("w4", [P, 4 * 128], FP16)
    par("w5", [P, 128], FP16)
    par("wp", [P, 4], FP16)
    par("b1", [P, 16], FP16)
    par("b2", [P, 8], FP16)
    par("b3", [P, 4], FP16)
    par("b4", [P, 1], FP16)
    par("b5", [P, 1], FP16)
    par("bp", [1, 4], FP16)
    out_ext = nc.declare_dram_parameter("out", [PCN, 3], F32, isOutput=True)
    import os
    DBG = bool(int(os.environ.get("KDBG", "0")))
    dbg = {}
    if DBG:
        for nm, shape in [("dbg_g1", [NT * P, 512]), ("dbg_h1", [NT * P, 2048]),
                          ("dbg_t2", [NPC, 1024]), ("dbg_h2", [NT * P, 1024]),
                          ("dbg_t3", [NPC, 512]), ("dbg_t4", [NPC, 128]),
                          ("dbg_t5", [NPC, 128])]:
            dbg[nm] = nc.declare_dram_parameter(nm, shape, FP16, isOutput=True)

    # ---------------- internal DRAM
    # layers l=1..4 aggregate T_{l+1}; width WG[l]
    town = {}   # (l, chunk j) -> per-core town tensor
    for l in range(1, 5):
        for j in range(3):
            town[l, j] = nc.dram_tensor(
                f"tn{l}_{j}", [CH[j + 1] - CH[j], WG[l]], FP16)
    TF = {}
    for l in range(1, 5):
        for j in range(3):
            TF[l, j] = nc.dram_tensor(
                f"tf{l}_{j}", [(GB[j + 1] - GB[j]), WG[l]], FP16,
                addr_space="Shared")
    PART = {(l, p): nc.dram_tensor(f"part{l}_{p}", [NT * P, WG[l]], FP16)
            for l in range(1, 5) for p in range(2)}

    with tile.TileContext(nc) as tc:
        import contextlib
        with contextlib.ExitStack() as ctx:
            cpool = ctx.enter_context(tc.tile_pool(name="const", bufs=1))
            msgp = ctx.enter_context(tc.tile_pool(name="msg", bufs=4))
            spool = ctx.enter_context(tc.tile_pool(name="sb", bufs=3))
            pp = ctx.enter_context(tc.tile_pool(name="ps", bufs=2, space="PSUM"))
            hp = ctx.enter_context(tc.tile_pool(name="hh", bufs=2))

            # ---- resident constants
            def cload(name, shape, dt):
                t_ = cpool.tile(shape, dt, tag=name, name=name)
                nc.sync.dma_start(out=t_[:], in_=pr[name][:])
                return t_
            idx_sb = cload("idx16", [P, idxcols], I16)
            slot_sb = cload("slotp", [P, nch], FP16)
            norm_sb = cload("normp", [P, nch], FP16)
            iota_sb = cload("iota", [P, P], FP16)
            ident_sb = cload("ident", [P, P], FP16)
            ones_sb = cload("ones1", [1, P], FP16)
            w1_sb = cload("w1", [P, 4 * 16 * P], FP16)
            w2_sb = cload("w2", [P, 16 * 1024], FP16)
            w3_sb = cload("w3", [P, 8 * 512], FP16)
            w4_sb = cload("w4", [P, 4 * 128], FP16)
            w5_sb = cload("w5", [P, 128], FP16)
            wp_sb = cload("wp", [P, 4], FP16)
            b1_sb = cload("b1", [P, 16], FP16)
            b2_sb = cload("b2", [P, 8], FP16)
            b3_sb = cload("b3", [P, 4], FP16)
            b4_sb = cload("b4", [P, 1], FP16)
            b5_sb = cload("b5", [P, 1], FP16)
            bp_sb = cload("bp", [1, 4], FP16)

            qn = [0]

            def build_s(t, r_list, load=False):
                """One S tile covering the chunks of (tile t, regions r_list)
                (contiguous in cbase layout). load=True DMAs the precomputed
                S from DRAM instead of building it on DVE."""
                c_lo = int(cbase[t, r_list[0]])
                ctn = sum(int(K[t, r]) // P for r in r_list)
                s_t = spool.tile([P, max(ctn, 1) * P], FP16, tag="s", name="s_t")
                if load:
                    if ctn > 0:
                        nc.sync.dma_start(
                            out=s_t[:, : ctn * P],
                            in_=pr["sfull"][:, c_lo * P:(c_lo + ctn) * P])
                    return s_t, ctn
                for b0 in range(0, ctn, SBATCH):
                    nb = min(SBATCH, ctn - b0)
                    cb0 = c_lo + b0
                    o3 = _3d(s_t[:, b0 * P:(b0 + nb) * P], nb)
                    nc.vector.tensor_tensor(
                        out=o3,
                        in0=_bc3(slot_sb[:, cb0:cb0 + nb], nb, mode="col"),
                        in1=_bc3(iota_sb[:], nb, mode="mat"),
                        op=mybir.AluOpType.is_equal)
                    nc.vector.tensor_tensor(
                        out=o3, in0=o3,
                        in1=_bc3(norm_sb[:, cb0:cb0 + nb], nb, mode="col"),
                        op=mybir.AluOpType.mult)
                return s_t, ctn

            def gather_mm(t, r, src_dram, W, s_t, s_coff, ps, mmcnt, mmtot,
                          use_start=True):
                """Gathers + aggregation matmuls for (tile t, region r).
                s_coff: chunk offset of this region within s_t.
                mmcnt: chunks already accumulated into ps; mmtot: total
                expected (stop flags on the last). use_start=False when the
                psum was already initialized (identity re-add). Returns new
                mmcnt."""
                k = int(K[t, r])
                if k == 0:
                    return mmcnt
                nf = W // P
                gmax = GMAX if W <= 512 else (4096 // W)  # cap msg at 8KB/part
                pos = 0
                while pos < k:
                    ks = min(gmax * P, k - pos)
                    ic = int(icol[t, r]) + pos // 16
                    msg = msgp.tile([P, 4096], FP16, tag="msg", name="msg")
                    nc.gpsimd.dma_gather(
                        out_ap=_3d(msg[:, : (ks // P) * W], ks // P, inner=W),
                        in_ap=src_dram,
                        idxs_ap=idx_sb[:, ic: ic + ks // 16],
                        num_idxs=ks,
                        num_idxs_reg=ks,
                        elem_size=W,
                        elem_step=W,
                        queue_num=qn[0],
                    )
                    qn[0] = (qn[0] + 1) % 4
                    for ci in range(ks // P):
                        cglob = s_coff + pos // P + ci
                        for fb in range(nf):
                            # start only on the first write to each 2KB PSUM
                            # zero region (512 f32 cols = 4 fb slices)
                            nc.tensor.matmul(
                                out=ps[:, fb * P:(fb + 1) * P],
                                lhsT=msg[:, ci * W + fb * P: ci * W + (fb + 1) * P],
                                rhs=s_t[:, cglob * P:(cglob + 1) * P],
                                start=(use_start and mmcnt == 0
                                       and fb % 4 == 0),
                                stop=(mmcnt == mmtot - 1))
                        mmcnt += 1
                    pos += ks
                return mmcnt

            def store_town(l, t, t_sb, W):
                j = 0 if t < 16 else (1 if t < 32 else 2)
                r0 = t * P - CH[j]
                nc.sync.dma_start(
                    out=town[l, j][r0:r0 + P, :], in_=t_sb[:, :W])

            def emit_ag(l, j):
                nc.gpsimd.collective_compute(
                    "AllGather", mybir.AluOpType.bypass, replica_groups=rg,
                    ins=[town[l, j][:]], outs=[TF[l, j][:]])

            # ================= phase 1: agg0(X) + d0 + d1 -> T2 =================
            def phase1_tile(t):
                s_t, ctn = build_s(t, [0, 1, 2])
                ps = pp.tile([P, 1024], F32, tag="agg", name="ps_agg")
                mm = 0
                coff = 0
                for r in range(3):
                    mm = gather_mm(t, r, pr["xf"][GB[r]:GB[r + 1], :], 512,
                                   s_t, coff, ps, mm, ctn)
                    coff += int(K[t, r]) // P
                assert mm == ctn and ctn > 0
                g1t = hp.tile([P, 512], FP16, tag="g1t", name="g1t")
                nc.scalar.activation(
                    out=g1t[:], in_=ps[:, :512],
                    func=mybir.ActivationFunctionType.Copy)
                if DBG:
                    nc.sync.dma_start(out=dbg["dbg_g1"][t * P:(t + 1) * P, :],
                                      in_=g1t[:])
                # d0: H1^T = relu(W1^T-blocks @ G1^T + b1), 4 quarters
                h1t = hp.tile([P, 2048], FP16, tag="h1t", name="h1t")
                for q in range(4):
                    ps0 = pp.tile([P, 512], F32, tag="d0", name="ps_d0")
                    for mi in range(4):
                        mb = q * 4 + mi
                        for kb in range(4):
                            nc.tensor.matmul(
                                out=ps0[:, mi * P:(mi + 1) * P],
                                lhsT=w1_sb[:, (kb * 16 + mb) * P:(kb * 16 + mb + 1) * P],
                                rhs=g1t[:, kb * P:(kb + 1) * P],
                                start=(kb == 0 and mi == 0),
                                stop=(kb == 3))
                    for mi in range(4):
                        mb = q * 4 + mi
                        nc.scalar.activation(
                            out=h1t[:, mb * P:(mb + 1) * P],
                            in_=ps0[:, mi * P:(mi + 1) * P],
                            func=mybir.ActivationFunctionType.Relu,
                            bias=b1_sb[:, mb:mb + 1])
                # d1: T2 = H1 @ W2 (normal orientation), 2 halves of 512
                t2sb = hp.tile([P, 1024], FP16, tag="tout", name="t2sb")
                for h in range(2):
                    psd = pp.tile([P, 512], F32, tag="d", name="ps_d")
                    for kb in range(16):
                        nc.tensor.matmul(
                            out=psd[:],
                            lhsT=h1t[:, kb * P:(kb + 1) * P],
                            rhs=w2_sb[:, kb * 1024 + h * 512: kb * 1024 + h * 512 + 512],
                            start=(kb == 0), stop=(kb == 15))
                    nc.scalar.activation(
                        out=t2sb[:, h * 512:(h + 1) * 512], in_=psd[:],
                        func=mybir.ActivationFunctionType.Copy)
                if DBG:
                    nc.sync.dma_start(out=dbg["dbg_h1"][t * P:(t + 1) * P, :],
                                      in_=h1t[:])
                    nc.sync.dma_start(out=dbg["dbg_t2"][t * P:(t + 1) * P, :],
                                      in_=t2sb[:])
                store_town(1, t, t2sb, 1024)
                if t == 15:
                    emit_ag(1, 0)
                elif t == 31:
                    emit_ag(1, 1)

            # ========= layers l=1..4: agg_l (3 passes, 1 per region) + dense ====
            # agg_l consumes TF[l,*] (width WG[l]), produces H^{l+1,T}; dense
            # d_{l+1} produces T_{l+2} (towns l+1) or the final output. Passes
            # 0/1 stage the PSUM to DRAM as fp16 partials; passes 1/2 re-add
            # them via an identity matmul.
            def agg_tile(l, p, t):
                W = WG[l]
                nf = W // P
                bias_sb = {1: b2_sb, 2: b3_sb, 3: b4_sb, 4: b5_sb}[l]
                last = p == 2
                if True:
                    if True:
                        s_t, ctn = build_s(t, [p],
                                           load=(l >= 3 and t % 4 == 1))
                        ps = pp.tile([P, 1024], F32, tag="agg", name="ps_agg")
                        if p > 0:
                            pb = hp.tile([P, 1024], FP16, tag="pb", name="pb")
                            nc.sync.dma_start(
                                out=pb[:, :W],
                                in_=PART[l, p - 1][t * P:(t + 1) * P, :])
                            nid = (W + 511) // 512
                            for j in range(nid):
                                w_ = min(512, W - j * 512)
                                nc.tensor.matmul(
                                    out=ps[:, j * 512: j * 512 + w_],
                                    lhsT=ident_sb[:],
                                    rhs=pb[:, j * 512: j * 512 + w_],
                                    start=True,
                                    stop=(ctn == 0 and j == nid - 1))
                            if ctn > 0:
                                gather_mm(t, p, TF[l, p][:], W, s_t, 0, ps,
                                          0, ctn, use_start=False)
                        else:
                            gather_mm(t, 0, TF[l, 0][:], W, s_t, 0, ps, 0,
                                      max(ctn, 1))
                            if ctn == 0:
                                nc.vector.memset(ps[:, :W], 0.0)
                        if not last:
                            pa = hp.tile([P, 1024], FP16, tag="pa", name="pa")
                            nc.scalar.activation(
                                out=pa[:, :W], in_=ps[:, :W],
                                func=mybir.ActivationFunctionType.Copy)
                            nc.sync.dma_start(
                                out=PART[l, p][t * P:(t + 1) * P, :],
                                in_=pa[:, :W])
                            return
                        hT = hp.tile([P, 1024], FP16, tag="ht", name="hT")
                        for fb in range(nf):
                            nc.scalar.activation(
                                out=hT[:, fb * P:(fb + 1) * P],
                                in_=ps[:, fb * P:(fb + 1) * P],
                                func=mybir.ActivationFunctionType.Relu,
                                bias=bias_sb[:, fb:fb + 1])
                        if DBG and l == 1:
                            nc.sync.dma_start(
                                out=dbg["dbg_h2"][t * P:(t + 1) * P, :],
                                in_=hT[:, :1024])
                        # dense d_{l+1}
                        if l == 1:
                            # H2[1024] @ W3 -> T3 [512]
                            t3 = hp.tile([P, 512], FP16, tag="tout", name="t3")
                            psd = pp.tile([P, 512], F32, tag="d", name="ps_d")
                            for kb in range(8):
                                nc.tensor.matmul(
                                    out=psd[:],
                                    lhsT=hT[:, kb * P:(kb + 1) * P],
                                    rhs=w3_sb[:, kb * 512:(kb + 1) * 512],
                                    start=(kb == 0), stop=(kb == 7))
                            nc.scalar.activation(
                                out=t3[:], in_=psd[:],
                                func=mybir.ActivationFunctionType.Copy)
                            if DBG:
                                nc.sync.dma_start(
                                    out=dbg["dbg_t3"][t * P:(t + 1) * P, :],
                                    in_=t3[:])
                            store_town(2, t, t3, 512)
                            if t == 15:
                                emit_ag(2, 0)
                            elif t == 31:
                                emit_ag(2, 1)
                        elif l == 2:
                            # H3[512] @ W4 -> T4 [128]
                            t4 = hp.tile([P, 128], FP16, tag="tout4", name="t4")
                            psd = pp.tile([P, 512], F32, tag="d", name="ps_d")
                            for kb in range(4):
                                nc.tensor.matmul(
                                    out=psd[:, :128],
                                    lhsT=hT[:, kb * P:(kb + 1) * P],
                                    rhs=w4_sb[:, kb * 128:(kb + 1) * 128],
                                    start=(kb == 0), stop=(kb == 3))
                            nc.scalar.activation(
                                out=t4[:], in_=psd[:, :128],
                                func=mybir.ActivationFunctionType.Copy)
                            if DBG:
                                nc.sync.dma_start(
                                    out=dbg["dbg_t4"][t * P:(t + 1) * P, :],
                                    in_=t4[:])
                            store_town(3, t, t4, 128)
                            if t == 15:
                                emit_ag(3, 0)
                            elif t == 31:
                                emit_ag(3, 1)
                        elif l == 3:
                            # H4[128] @ W5 -> T5 [128]
                            t5 = hp.tile([P, 128], FP16, tag="tout4", name="t5")
                            psd = pp.tile([P, 512], F32, tag="d", name="ps_d")
                            nc.tensor.matmul(
                                out=psd[:, :128], lhsT=hT[:, :128],
                                rhs=w5_sb[:], start=True, stop=True)
                            nc.scalar.activation(
                                out=t5[:], in_=psd[:, :128],
                                func=mybir.ActivationFunctionType.Copy)
                            if DBG:
                                nc.sync.dma_start(
                                    out=dbg["dbg_t5"][t * P:(t + 1) * P, :],
                                    in_=t5[:])
                            store_town(4, t, t5, 128)
                            if t == 15:
                                emit_ag(4, 0)
                            elif t == 31:
                                emit_ag(4, 1)
                        else:
                            # d5: out = H5 @ Wp + bp
                            psd = pp.tile([P, 512], F32, tag="d", name="ps_d")
                            nc.tensor.matmul(
                                out=psd[:, :4], lhsT=hT[:, :128], rhs=wp_sb[:],
                                start=True, stop=False)
                            nc.tensor.matmul(
                                out=psd[:, :4], lhsT=ones_sb[0:1, :],
                                rhs=bp_sb[0:1, :], start=False, stop=True)
                            osb = hp.tile([P, 4], F32, tag="fout", name="osb")
                            nc.vector.tensor_copy(out=osb[:], in_=psd[:, :4])
                            r0 = t * P
                            r1 = min((t + 1) * P, PCN)
                            if r1 > r0:
                                nc.sync.dma_start(
                                    out=out_ext[r0:r1, :],
                                    in_=osb[: r1 - r0, :3])

            # ---------------- emission driver (software pipelining) ----------
            # phase 1 tiles 0..33, then interleave its tail with agg1-pass0
            # (whose gathers wait on the first T2 AllGather chunk).
            for t in range(34):
                phase1_tile(t)
            j = 0
            for t in range(34, NT):
                phase1_tile(t)
                while j < 2 * (t - 33) and j < NT:
                    agg_tile(1, 0, j)
                    j += 1
            emit_ag(1, 2)
            while j < NT:
                agg_tile(1, 0, j)
                j += 1
            for p in (1, 2):
                for t in range(NT):
                    agg_tile(1, p, t)
            emit_ag(2, 2)
            for l in range(2, 5):
                for p in range(3):
                    for t in range(NT):
                        agg_tile(l, p, t)
                if l < 4:
                    emit_ag(l + 1, 2)

    nc.finalize()
    return nc


# ------------------------------------------------------------------ driver
_CACHE = {}


def kernel(x, edge_index, edge_attr, W1, b1, W2, b2, W3, b3, W4, b4, W5, b5,
           Wp, bp):
    apply_tile_patch()
    import os
    from concourse.bass_utils import run_bass_kernel_spmd

    Ws = [W1, W2, W3, W4, W5]
    bs = [b1, b2, b3, b4, b5]
    in_maps, meta = preprocess(x, edge_index, edge_attr, Ws, bs, Wp, bp)

    key = (meta["K"].tobytes(), meta["nch"], meta["idxcols"])
    nc = _CACHE.get(key)
    if nc is None:
        nc = build_program(meta)
        _CACHE[key] = nc

    res = run_bass_kernel_spmd(
        nc, in_maps, core_ids=list(range(NCORE)),
        trace=bool(int(os.environ.get("TRACE", "0"))))
    if res.exec_time_ns:
        print(f"HW exec time: {res.exec_time_ns} ns")
    out = np.concatenate(
        [res.results[c]["out"] for c in range(NCORE)], axis=0)
    return np.ascontiguousarray(out.astype(np.float32))
